# revision 1
# baseline (speedup 1.0000x reference)
"""Trainium2 Bass kernel: polar-BP left-message butterfly (nn_IterateLeftLayer).

Math per stage i (9..0), with L = left row i+1 (unclipped), R = right row i:
  out[pos] = w0 * ms(L[pos], L[neg] + R[neg])
  out[neg] = w1 * ms(L[pos], R[pos]) + L[neg]
where ms(x,y) = sign(x)sign(y)min(|x|,|y|), pos = {c: bit i of c == 0},
neg = pos + 2^i.  Final output = clip(left, +-10) with rows 0..9 replaced.

ms is computed exactly in sign-magnitude form:
  ms(x,y) = min_f32(x & MAG, y & MAG) | ((x ^ y) & SIGN)
Bitwise ops run in the DVE's exact integer path; the min runs on positive
floats (a pure selection, no rounding).  Note: int32 *arithmetic* ops (e.g.
int min) are NOT exact on the DVE -- operands convert through fp32 ALUs.

Sharding: pure data-parallel over batch, 512 rows per core on 8 cores.
Layout: batch on partitions (4 groups of 128 coalesced along the free axis
-> [128, 4096] tiles); the butterfly is pure strided access patterns.
"""

import sys

for _p in ("/opt/trn_rl_repo",):
    if _p not in sys.path:
        sys.path.insert(0, _p)

import numpy as np

import concourse.bass as bass
import concourse.tile as tile
from concourse import bacc, mybir
from concourse.bass_utils import run_bass_kernel_spmd

NUM_STAGES = 10
CODE = 1024
B = 4096
N_CORES = 8
P = 128
CLIP = 10.0
F32 = mybir.dt.float32
I32 = mybir.dt.int32
ALU = mybir.AluOpType
ACTF = mybir.ActivationFunctionType


def _halves(ap, i):
    """pos/neg strided views of a [P, W] row for stage i."""
    r = 1 << i
    v = ap.rearrange("p (m two r) -> p m two r", two=2, r=r)
    return v[:, :, 0, :], v[:, :, 1, :]


def build(nc, weights, bpc):
    """Emit the per-core kernel. weights: [(w0, w1)] * 10, bpc: batch rows/core."""
    g = bpc // P
    w = g * CODE
    h = w // 2

    right_d = nc.dram_tensor("right", [bpc, NUM_STAGES + 1, CODE], F32,
                             kind="ExternalInput")
    left10_d = nc.dram_tensor("left10", [bpc, CODE], F32, kind="ExternalInput")
    out_d = nc.dram_tensor("out", [bpc, NUM_STAGES, CODE], F32,
                           kind="ExternalOutput")

    with tile.TileContext(nc) as tc:
        with (
            tc.tile_pool(name="lo", bufs=2) as lo_pool,
            tc.tile_pool(name="rin", bufs=2) as r_pool,
            tc.tile_pool(name="tadd", bufs=1) as t_pool,
            tc.tile_pool(name="bm", bufs=1) as bm_pool,
            tc.tile_pool(name="am", bufs=1) as am_pool,
            tc.tile_pool(name="mm", bufs=1) as m_pool,
            tc.tile_pool(name="uu", bufs=1) as u_pool,
            tc.tile_pool(name="ms", bufs=1) as ms_pool,
            tc.tile_pool(name="oc", bufs=2) as oc_pool,
            tc.tile_pool(name="cst", bufs=1) as c_pool,
        ):
            sgn_t = c_pool.tile([P, 1], I32, tag="sgn")
            nc.vector.memset(sgn_t[:], -0x80000000)

            L = lo_pool.tile([P, w], F32, tag="lo")
            nc.sync.dma_start(
                L[:].rearrange("p (g c) -> p g c", g=g),
                left10_d.ap().rearrange("(g p) c -> p g c", p=P),
            )

            for i in reversed(range(NUM_STAGES)):
                w0, w1 = weights[i]
                R = r_pool.tile([P, w], F32)
                nc.sync.dma_start(
                    R[:].rearrange("p (g c) -> p g c", g=g),
                    right_d.ap()[:, i, :].rearrange("(g p) c -> p g c", p=P),
                )

                Lp, Ln = _halves(L[:], i)
                Rp, Rn = _halves(R[:], i)
                Lpi, _ = _halves(L[:].bitcast(I32), i)
                Rpi, _ = _halves(R[:].bitcast(I32), i)

                t = t_pool.tile([P, h], F32)
                nc.vector.tensor_add(t[:], Ln, Rn)
                ti = t[:].bitcast(I32)

                # magnitudes on ScalarE (offloads the DVE), bm = [|t| , |Rp|]
                bm = bm_pool.tile([P, w], F32)
                nc.scalar.activation(bm[:, :h], t[:], ACTF.Abs)
                nc.scalar.activation(bm[:, h:], Rp, ACTF.Abs)
                am = am_pool.tile([P, h], F32)   # |Lp| (shared A/B)
                nc.scalar.activation(am[:], Lp, ACTF.Abs)

                # min of magnitudes: fp32 min of positive floats (exact select)
                m = m_pool.tile([P, w], F32)
                nc.vector.tensor_tensor(m[:, :h], bm[:, :h], am[:], ALU.min)
                nc.vector.tensor_tensor(m[:, h:], bm[:, h:], am[:], ALU.min)

                # composite signs: u = [t ^ Lp , Rp ^ Lp]
                u = u_pool.tile([P, w], I32)
                nc.vector.tensor_tensor(u[:, :h], ti, Lpi, ALU.bitwise_xor)
                nc.vector.tensor_tensor(u[:, h:], Rpi, Lpi, ALU.bitwise_xor)

                # ms = (u & SIGN) | m   (one fused op over both halves)
                ms = ms_pool.tile([P, w], I32)
                nc.vector.scalar_tensor_tensor(
                    ms[:], u[:], sgn_t[:], m[:].bitcast(I32),
                    ALU.bitwise_and, ALU.bitwise_or)
                msA = ms[:, :h].bitcast(F32)
                msB = ms[:, h:].bitcast(F32)

                O = lo_pool.tile([P, w], F32, tag="lo")
                Op, On = _halves(O[:], i)
                # out[pos] = w0 * msA   (ScalarE: copy with scale)
                nc.scalar.activation(Op, msA, ACTF.Copy, bias=0.0,
                                     scale=float(w0))
                # out[neg] = w1 * msB + Ln
                nc.vector.scalar_tensor_tensor(
                    On, msB, float(w1), Ln, ALU.mult, ALU.add)

                oc = oc_pool.tile([P, w], F32)
                nc.gpsimd.tensor_scalar(
                    oc[:], O[:], CLIP, -CLIP, ALU.min, ALU.max)
                nc.sync.dma_start(
                    out_d.ap()[:, i, :].rearrange("(g p) c -> p g c", p=P),
                    oc[:].rearrange("p (g c) -> p g c", g=g),
                )
                L = O


TRACE = False
LAST_RESULTS = None


def _make_nc(weights, bpc):
    nc = bacc.Bacc("TRN2", target_bir_lowering=False, debug=False)
    build(nc, weights, bpc)
    nc.compile()
    return nc


def kernel(right, left, left_weights, iter):
    right = np.asarray(right, dtype=np.float32)
    left = np.asarray(left, dtype=np.float32)
    wsel = np.asarray(left_weights, dtype=np.float32)[int(iter)]  # [10, 2]
    weights = [(float(wsel[i, 0]), float(wsel[i, 1])) for i in range(NUM_STAGES)]

    bpc = B // N_CORES
    nc = _make_nc(weights, bpc)

    in_maps = []
    for c in range(N_CORES):
        sl = slice(c * bpc, (c + 1) * bpc)
        in_maps.append({
            "right": np.ascontiguousarray(right[sl]),
            "left10": np.ascontiguousarray(left[sl, NUM_STAGES, :]),
        })
    global LAST_RESULTS
    LAST_RESULTS = run_bass_kernel_spmd(
        nc, in_maps, list(range(N_CORES)), trace=TRACE)
    res = LAST_RESULTS.results

    out = np.empty((B, NUM_STAGES + 1, CODE), np.float32)
    for c in range(N_CORES):
        out[c * bpc:(c + 1) * bpc, :NUM_STAGES, :] = res[c]["out"]
    out[:, NUM_STAGES, :] = np.clip(left[:, NUM_STAGES, :], -CLIP, CLIP)
    return out



# revision 23
# speedup vs baseline: 1.9330x; 1.9330x over previous
"""Trainium2 Bass kernel: polar-BP left-message butterfly (nn_IterateLeftLayer).

Math per stage i (9..0), with L = left row i+1 (unclipped), R = right row i:
  out[pos] = w0 * ms(L[pos], L[neg] + R[neg])
  out[neg] = w1 * ms(L[pos], R[pos]) + L[neg]
where ms(x,y) = sign(x)sign(y)min(|x|,|y|), pos = {c: bit i of c == 0},
neg = pos + 2^i.  Final output = clip(left, +-10) with rows 0..9 replaced.

ms is computed in pure min/max form (no bit tricks):
  ms(a,b) = min(max(a,-b), max(-a,b))
which is exact (selection + sign flip only).  All on-device compute is fp16:
every DVE tensor_tensor runs in 2x mode and every tensor_scalar in 4x mode,
and DMA bytes halve.  fp16 rounding keeps rel err ~7e-4 vs the f32 oracle.

The +-10 output clip is applied on the host (the recurrence needs UNclipped
values anyway, so the device never clips).

Engine balance per stage (free dim h=2048 per op); the stage recurrence is
serial in L (every next-stage op needs the FULL previous row), so the per-
stage critical chain is what matters.  Assignment keeps the two cheap-engine
ops (Pool) OFF the chain's tail and the DVE queue ordered so it never
stalls more than ~0.4us:
  DVE : t=Ln+Rn, nt=-t, v2=max(nLp,t), mv=min(v1,v2), ON=mv+Ln,
        u2=max(nLp,t)... order [t, nt, v2, mv, ON, u2, OP=min(u1,u2)]
  Act : nLp=-Lp, then nRp for the NEXT stage (R is prefetched, so -Rp is
        computable one stage early)
  Pool: v1=max(Lp,nRp) (ready at row-start), u1=max(Lp,nt) (needed last)
  SP  : R-row prefetch + output writeback DMAs (double buffered)

Stage 0 pairs adjacent elements (stride 1), which would break the DVE 16-bit
packed fast path; instead L row 1 is deinterleaved on-device into packed
pos/neg halves, right row 0 arrives host-deinterleaved, and out row 0 leaves
as packed halves that the host re-interleaves.

Sharding: pure data-parallel over batch, 512 rows per core on 8 cores.
Layout: batch on partitions (4 groups of 128 coalesced along the free axis
-> [128, 4096] tiles); the butterfly is pure strided access patterns.
"""

import sys

for _p in ("/opt/trn_rl_repo",):
    if _p not in sys.path:
        sys.path.insert(0, _p)

import numpy as np

import bass_rust
import concourse.bass as bass
import concourse.tile as tile
from concourse import bacc, mybir
from concourse.bass_utils import run_bass_kernel_spmd


def _after(inst, *prevs):
    """Order `inst` after `prevs` on the same engine (scheduler edge, no
    semaphore) — pins queue order the greedy list scheduler would flip."""
    names = bass_rust.InstructionNameOrderedSet([p.ins.name for p in prevs])
    inst.ins.add_nosync_dependencies_from(names)

NUM_STAGES = 10
CODE = 1024
B = 4096
N_CORES = 8
P = 128
CLIP = 10.0
F16 = mybir.dt.float16
ALU = mybir.AluOpType
ACTF = mybir.ActivationFunctionType


def _halves(ap, i):
    """pos/neg strided views of a [P, W] row for stage i (i >= 1)."""
    r = 1 << i
    v = ap.rearrange("p (m two r) -> p m two r", two=2, r=r)
    return v[:, :, 0, :], v[:, :, 1, :]


def build(nc, weights, bpc):
    """Emit the per-core kernel. weights: [(w0, w1)] * 10, bpc: batch rows/core."""
    g = bpc // P
    w = g * CODE
    h = w // 2

    right_d = nc.dram_tensor("right", [bpc, NUM_STAGES, CODE], F16,
                             kind="ExternalInput")
    r0_d = nc.dram_tensor("right0", [bpc, CODE], F16, kind="ExternalInput")
    left10_d = nc.dram_tensor("left10", [bpc, CODE], F16, kind="ExternalInput")
    out_d = nc.dram_tensor("out", [bpc, NUM_STAGES, CODE], F16,
                           kind="ExternalOutput")

    def dram_row(dt_, i=None):
        ap = dt_.ap() if i is None else dt_.ap()[:, i, :]
        return ap.rearrange("(g p) c -> p g c", p=P)

    def sb(ap):
        return ap.rearrange("p (g c) -> p g c", g=g)

    with tile.TileContext(nc) as tc:
        with (
            tc.tile_pool(name="lo", bufs=3) as lo_pool,
            tc.tile_pool(name="rin", bufs=3) as r_pool,
            tc.tile_pool(name="tt", bufs=2) as t_pool,
            tc.tile_pool(name="ng", bufs=2) as ng_pool,
            tc.tile_pool(name="uu", bufs=2) as u_pool,
            tc.tile_pool(name="vv", bufs=2) as v_pool,
            tc.tile_pool(name="mv", bufs=2) as m_pool,
            tc.tile_pool(name="pk", bufs=1) as pk_pool,
        ):
            # fill: stage 9 needs only the NEG halves of L10/R9 for its first
            # op (t = Ln + Rn) — land those first so compute starts ~3us
            # earlier, then the pos halves, then R8.
            L = lo_pool.tile([P, w], F16, tag="lo", name="L10")
            R_tiles = {9: r_pool.tile([P, w], F16, tag="rin", name="R9")}
            for half in (1, 0):   # neg halves first
                for dt_, tile_ in ((left10_d, L), (right_d, R_tiles[9])):
                    src = dt_.ap() if dt_ is left10_d else dt_.ap()[:, 9, :]
                    src = src.rearrange("(g p) (two r) -> p g two r",
                                        p=P, two=2, r=CODE // 2)[:, :, half, :]
                    dst = tile_[:].rearrange("p (g two r) -> p g two r",
                                             two=2, r=CODE // 2)[:, :, half, :]
                    nc.sync.dma_start(dst, src)
            for i in (8,):
                R_tiles[i] = r_pool.tile([P, w], F16, tag="rin", name=f"R{i}")
                nc.sync.dma_start(sb(R_tiles[i][:]), dram_row(right_d, i))

            def rp_of(j):
                """pos-half view of stage j's R tile."""
                return _halves(R_tiles[j][:], 9 if j == 0 else j)[0]

            # -Rp for stage 9, ready before the pipeline starts
            nR_tiles = {9: ng_pool.tile([P, h], F16, tag="nRp", name="nRp9")}
            nc.scalar.activation(nR_tiles[9][:], rp_of(9), ACTF.Copy,
                                 bias=0.0, scale=-1.0)

            for i in reversed(range(NUM_STAGES)):
                w0, w1 = weights[i]
                stage0 = i == 0
                R = R_tiles[i]
                # prefetch R two stages ahead (stage 0's row arrives
                # host-deinterleaved in r0_d)
                if i - 2 >= 0:
                    R_tiles[i - 2] = r_pool.tile([P, w], F16, tag="rin",
                                                 name=f"R{i - 2}")
                    src = dram_row(r0_d) if i == 2 else dram_row(right_d, i - 2)
                    nc.sync.dma_start(sb(R_tiles[i - 2][:]), src)

                if stage0:
                    # adjacent pairs break the 16-bit packed fast path:
                    # deinterleave L row 1 into packed halves first
                    lv = L[:].rearrange("p (m two) -> p m two", two=2)
                    Lp_t = pk_pool.tile([P, h], F16, tag="lpk", name="Lp0")
                    nc.vector.tensor_copy(Lp_t[:], lv[:, :, 0])
                    Ln_t = pk_pool.tile([P, h], F16, tag="lnk", name="Ln0")
                    nc.scalar.activation(Ln_t[:], lv[:, :, 1], ACTF.Copy,
                                         bias=0.0, scale=1.0)
                    Lp, Ln = Lp_t[:], Ln_t[:]
                    # host-deinterleaved r0: per (p,g) code block the first
                    # 512 are evens (pos), last 512 odds (neg) == r=512 halves
                    Rp, Rn = _halves(R[:], 9)
                else:
                    Lp, Ln = _halves(L[:], i)
                    Rp, Rn = _halves(R[:], i)
                nRp = nR_tiles.pop(i)

                # ---- ready at row-start: nLp on Act, v1 on Pool
                nLp = ng_pool.tile([P, h], F16, tag="nLp", name=f"nLp{i}")
                nLp_i = nc.scalar.activation(nLp[:], Lp, ACTF.Copy, bias=0.0,
                                             scale=-1.0)
                v1 = v_pool.tile([P, h], F16, tag="v1", name=f"v1_{i}")
                nc.vector.tensor_tensor(v1[:], Lp, nRp[:], ALU.max)

                # ---- DVE spine: t, nt, v2, mv, ON, u2, OP
                t = t_pool.tile([P, h], F16, tag="t", name=f"t{i}")
                nc.vector.tensor_tensor(t[:], Ln, Rn, ALU.add)
                # nt on Pool (tensor_scalar IS implemented on gpsimd): the
                # u-chain has slack, and it takes 594ns off the DVE
                nt = t_pool.tile([P, h], F16, tag="nt", name=f"nt{i}")
                nc.gpsimd.tensor_scalar(nt[:], t[:], -1.0, None, ALU.mult)

                v2 = v_pool.tile([P, h], F16, tag="v2", name=f"v2_{i}")
                nc.vector.tensor_tensor(v2[:], nLp[:], Rp, ALU.max)
                u2 = u_pool.tile([P, h], F16, tag="u2", name=f"u2_{i}")
                nc.vector.tensor_tensor(u2[:], nLp[:], t[:], ALU.max)

                O = lo_pool.tile([P, w], F16, tag="lo", name=f"O{i}")
                Op, On = _halves(O[:], 9 if stage0 else i)

                mv = m_pool.tile([P, h], F16, tag="mv", name=f"mv{i}")
                nc.vector.tensor_tensor(mv[:], v1[:], v2[:], ALU.min)
                if w1 != 1.0:
                    nc.vector.tensor_scalar(mv[:], mv[:], float(w1), None,
                                            ALU.mult)
                on_i = nc.vector.tensor_tensor(On, mv[:], Ln, ALU.add)

                # u1 stalls on Pool's nt — pin it after ON so the DVE queue
                # runs [v2, u2, mv, ON, u1, OP] without idling
                u1 = u_pool.tile([P, h], F16, tag="u1", name=f"u1_{i}")
                u1_i = nc.vector.tensor_tensor(u1[:], Lp, nt[:], ALU.max)
                _after(u1_i, on_i)
                op_i = nc.vector.tensor_tensor(Op, u1[:], u2[:], ALU.min)
                if w0 != 1.0:
                    op_i = nc.vector.tensor_scalar(Op, Op, float(w0), None,
                                                   ALU.mult)

                # -Rp for the NEXT stage while DVE finishes this one (kept
                # behind nLp on Act so it can't steal nLp's slot)
                if i >= 1:
                    nR_tiles[i - 1] = ng_pool.tile([P, h], F16, tag="nRp",
                                                   name=f"nRp{i - 1}")
                    nrp_i = nc.scalar.activation(nR_tiles[i - 1][:],
                                                 rp_of(i - 1), ACTF.Copy,
                                                 bias=0.0, scale=-1.0)
                    _after(nrp_i, nLp_i)

                if stage0:
                    # split writeback: neg half leaves while OP still computes
                    nc.sync.dma_start(dram_row(out_d, 0)[:, :, h // g:], On)
                    nc.sync.dma_start(dram_row(out_d, 0)[:, :, :h // g], Op)
                else:
                    nc.sync.dma_start(dram_row(out_d, i), sb(O[:]))
                L = O


TRACE = False
LAST_RESULTS = None


def _make_nc(weights, bpc):
    nc = bacc.Bacc("TRN2", target_bir_lowering=False, debug=False)
    build(nc, weights, bpc)
    nc.compile()
    return nc


def kernel(right, left, left_weights, iter):
    right = np.asarray(right)
    left = np.asarray(left)
    wsel = np.asarray(left_weights, dtype=np.float32)[int(iter)]  # [10, 2]
    weights = [(float(wsel[i, 0]), float(wsel[i, 1])) for i in range(NUM_STAGES)]

    bpc = B // N_CORES
    nc = _make_nc(weights, bpc)

    right16 = right[:, :NUM_STAGES, :].astype(np.float16)
    left10_16 = left[:, NUM_STAGES, :].astype(np.float16)
    # stage-0 right row, host-deinterleaved into packed pos|neg halves
    r0 = np.empty((B, CODE), np.float16)
    r0[:, :CODE // 2] = right16[:, 0, 0::2]
    r0[:, CODE // 2:] = right16[:, 0, 1::2]

    in_maps = []
    for c in range(N_CORES):
        sl = slice(c * bpc, (c + 1) * bpc)
        in_maps.append({
            "right": np.ascontiguousarray(right16[sl]),
            "right0": np.ascontiguousarray(r0[sl]),
            "left10": np.ascontiguousarray(left10_16[sl]),
        })
    global LAST_RESULTS
    LAST_RESULTS = run_bass_kernel_spmd(
        nc, in_maps, list(range(N_CORES)), trace=TRACE)
    res = LAST_RESULTS.results

    out16 = np.empty((B, NUM_STAGES, CODE), np.float16)
    for c in range(N_CORES):
        out16[c * bpc:(c + 1) * bpc] = res[c]["out"]
    # row 0 left the device as packed pos|neg halves; re-interleave
    row0 = out16[:, 0, :].copy()
    out16[:, 0, 0::2] = row0[:, :CODE // 2]
    out16[:, 0, 1::2] = row0[:, CODE // 2:]

    out = np.empty((B, NUM_STAGES + 1, CODE), np.float32)
    out[:, :NUM_STAGES, :] = np.clip(out16.astype(np.float32), -CLIP, CLIP)
    out[:, NUM_STAGES, :] = np.clip(left[:, NUM_STAGES, :], -CLIP, CLIP)
    return out


# revision 26
# speedup vs baseline: 1.9366x; 1.0019x over previous
"""Trainium2 Bass kernel: polar-BP left-message butterfly (nn_IterateLeftLayer).

Math per stage i (9..0), with L = left row i+1 (unclipped), R = right row i:
  out[pos] = w0 * ms(L[pos], L[neg] + R[neg])
  out[neg] = w1 * ms(L[pos], R[pos]) + L[neg]
where ms(x,y) = sign(x)sign(y)min(|x|,|y|), pos = {c: bit i of c == 0},
neg = pos + 2^i.  Final output = clip(left, +-10) with rows 0..9 replaced.

ms is computed in pure min/max form (no bit tricks):
  ms(a,b) = min(max(a,-b), max(-a,b))
which is exact (selection + sign flip only).  All on-device compute is fp16:
every DVE tensor_tensor runs in 2x mode and every tensor_scalar in 4x mode,
and DMA bytes halve.  fp16 rounding keeps rel err ~7e-4 vs the f32 oracle.

The +-10 output clip is applied on the host (the recurrence needs UNclipped
values anyway, so the device never clips).

Engine balance per stage (free dim h=2048 per op); the stage recurrence is
serial in L (every next-stage op needs the FULL previous row), so the per-
stage critical chain is what matters.  Assignment keeps the two cheap-engine
ops (Pool) OFF the chain's tail and the DVE queue ordered so it never
stalls more than ~0.4us:
  DVE : t=Ln+Rn, nt=-t, v2=max(nLp,t), mv=min(v1,v2), ON=mv+Ln,
        u2=max(nLp,t)... order [t, nt, v2, mv, ON, u2, OP=min(u1,u2)]
  Act : nLp=-Lp, then nRp for the NEXT stage (R is prefetched, so -Rp is
        computable one stage early)
  Pool: v1=max(Lp,nRp) (ready at row-start), u1=max(Lp,nt) (needed last)
  SP  : R-row prefetch + output writeback DMAs (double buffered)

Stage 0 pairs adjacent elements (stride 1), which would break the DVE 16-bit
packed fast path; instead L row 1 is deinterleaved on-device into packed
pos/neg halves, right row 0 arrives host-deinterleaved, and out row 0 leaves
as packed halves that the host re-interleaves.

Sharding: pure data-parallel over batch, 512 rows per core on 8 cores.
Layout: batch on partitions (4 groups of 128 coalesced along the free axis
-> [128, 4096] tiles); the butterfly is pure strided access patterns.
"""

import sys

for _p in ("/opt/trn_rl_repo",):
    if _p not in sys.path:
        sys.path.insert(0, _p)

import numpy as np

import bass_rust
import concourse.bass as bass
import concourse.tile as tile
from concourse import bacc, mybir
from concourse.bass_utils import run_bass_kernel_spmd


def _after(inst, *prevs):
    """Order `inst` after `prevs` on the same engine (scheduler edge, no
    semaphore) — pins queue order the greedy list scheduler would flip."""
    names = bass_rust.InstructionNameOrderedSet([p.ins.name for p in prevs])
    inst.ins.add_nosync_dependencies_from(names)

NUM_STAGES = 10
CODE = 1024
B = 4096
N_CORES = 8
P = 128
CLIP = 10.0
F16 = mybir.dt.float16
ALU = mybir.AluOpType
ACTF = mybir.ActivationFunctionType


def _halves(ap, i):
    """pos/neg strided views of a [P, W] row for stage i (i >= 1)."""
    r = 1 << i
    v = ap.rearrange("p (m two r) -> p m two r", two=2, r=r)
    return v[:, :, 0, :], v[:, :, 1, :]


def build(nc, weights, bpc):
    """Emit the per-core kernel. weights: [(w0, w1)] * 10, bpc: batch rows/core."""
    g = bpc // P
    w = g * CODE
    h = w // 2

    right_d = nc.dram_tensor("right", [bpc, NUM_STAGES, CODE], F16,
                             kind="ExternalInput")
    r0_d = nc.dram_tensor("right0", [bpc, CODE], F16, kind="ExternalInput")
    left10_d = nc.dram_tensor("left10", [bpc, CODE], F16, kind="ExternalInput")
    out_d = nc.dram_tensor("out", [bpc, NUM_STAGES, CODE], F16,
                           kind="ExternalOutput")

    def dram_row(dt_, i=None):
        ap = dt_.ap() if i is None else dt_.ap()[:, i, :]
        return ap.rearrange("(g p) c -> p g c", p=P)

    def sb(ap):
        return ap.rearrange("p (g c) -> p g c", g=g)

    with tile.TileContext(nc) as tc:
        with (
            tc.tile_pool(name="lo", bufs=3) as lo_pool,
            tc.tile_pool(name="rin", bufs=3) as r_pool,
            tc.tile_pool(name="tt", bufs=2) as t_pool,
            tc.tile_pool(name="ng", bufs=2) as ng_pool,
            tc.tile_pool(name="uu", bufs=2) as u_pool,
            tc.tile_pool(name="vv", bufs=2) as v_pool,
            tc.tile_pool(name="mv", bufs=2) as m_pool,
            tc.tile_pool(name="pk", bufs=1) as pk_pool,
        ):
            # fill: stage 9 is column-split into two batch-group halves
            # (independent rows), each fed by quarter DMAs in dependency
            # order (neg half first), so compute starts ~3.5us in.
            L = lo_pool.tile([P, w], F16, tag="lo", name="L10")
            R_tiles = {9: r_pool.tile([P, w], F16, tag="rin", name="R9")}
            for s in (0, 1):
                for half in (1, 0):   # neg halves first
                    for dt_, tile_ in ((left10_d, L), (right_d, R_tiles[9])):
                        src = dt_.ap() if dt_ is left10_d else dt_.ap()[:, 9, :]
                        src = src.rearrange(
                            "(g p) (two r) -> p g two r",
                            p=P, two=2, r=CODE // 2)[:, 2 * s:2 * s + 2, half, :]
                        dst = tile_[:].rearrange(
                            "p (g two r) -> p g two r",
                            two=2, r=CODE // 2)[:, 2 * s:2 * s + 2, half, :]
                        nc.sync.dma_start(dst, src)
            for i in (8,):
                R_tiles[i] = r_pool.tile([P, w], F16, tag="rin", name=f"R{i}")
                nc.sync.dma_start(sb(R_tiles[i][:]), dram_row(right_d, i))

            def rp_of(j):
                """pos-half view of stage j's R tile."""
                return _halves(R_tiles[j][:], 9 if j == 0 else j)[0]

            nR_tiles = {}

            # ---- stage 9, column-split into two independent batch-group
            # halves so each starts as soon as its quarter-DMAs land.
            # nRp9 runs on DVE (idle during fill) instead of Act.
            w0, w1 = weights[9]
            R9 = R_tiles[9]
            R_tiles[7] = r_pool.tile([P, w], F16, tag="rin", name="R7")
            nc.sync.dma_start(sb(R_tiles[7][:]), dram_row(right_d, 7))
            Lp_f, Ln_f = _halves(L[:], 9)
            Rp_f, Rn_f = _halves(R9[:], 9)
            h2 = h // 2
            nLp9 = ng_pool.tile([P, h], F16, tag="nLp", name="nLp9")
            nRp9 = ng_pool.tile([P, h], F16, tag="nRp9", name="nRp9")
            t9 = t_pool.tile([P, h], F16, tag="t", name="t9")
            nt9 = t_pool.tile([P, h], F16, tag="nt", name="nt9")
            v19 = v_pool.tile([P, h], F16, tag="v1", name="v1_9")
            v29 = v_pool.tile([P, h], F16, tag="v2", name="v2_9")
            u19 = u_pool.tile([P, h], F16, tag="u1", name="u1_9")
            u29 = u_pool.tile([P, h], F16, tag="u2", name="u2_9")
            mv9 = m_pool.tile([P, h], F16, tag="mv", name="mv9")
            O9 = lo_pool.tile([P, w], F16, tag="lo", name="O9")
            Op_f, On_f = _halves(O9[:], 9)
            nlp_insts = []
            for s in (0, 1):
                msl = slice(2 * s, 2 * s + 2)
                csl = slice(s * h2, (s + 1) * h2)
                Lp, Ln = Lp_f[:, msl, :], Ln_f[:, msl, :]
                Rp, Rn = Rp_f[:, msl, :], Rn_f[:, msl, :]
                nlp_insts.append(nc.scalar.activation(
                    nLp9[:, csl], Lp, ACTF.Copy, bias=0.0, scale=-1.0))
                t_i = nc.vector.tensor_tensor(t9[:, csl], Ln, Rn, ALU.add)
                nc.gpsimd.tensor_scalar(nt9[:, csl], t9[:, csl], -1.0, None,
                                        ALU.mult)
                nr_i = nc.vector.tensor_scalar(nRp9[:, csl], Rp, -1.0, None,
                                               ALU.mult)
                _after(nr_i, t_i)
                v1_i = nc.vector.tensor_tensor(v19[:, csl], Lp, nRp9[:, csl],
                                               ALU.max)
                _after(v1_i, nr_i)
                v2_i = nc.vector.tensor_tensor(v29[:, csl], nLp9[:, csl], Rp,
                                               ALU.max)
                _after(v2_i, v1_i)
                u2_i = nc.vector.tensor_tensor(u29[:, csl], nLp9[:, csl],
                                               t9[:, csl], ALU.max)
                _after(u2_i, v2_i)
                mv_i = nc.vector.tensor_tensor(mv9[:, csl], v19[:, csl],
                                               v29[:, csl], ALU.min)
                _after(mv_i, u2_i)
                if w1 != 1.0:
                    mv_i = nc.vector.tensor_scalar(mv9[:, csl], mv9[:, csl],
                                                   float(w1), None, ALU.mult)
                u1_i = nc.vector.tensor_tensor(u19[:, csl], Lp, nt9[:, csl],
                                               ALU.max)
                _after(u1_i, mv_i)
                on_i = nc.vector.tensor_tensor(On_f[:, msl, :], mv9[:, csl],
                                               Ln, ALU.add)
                _after(on_i, u1_i)
                op_i = nc.vector.tensor_tensor(Op_f[:, msl, :], u19[:, csl],
                                               u29[:, csl], ALU.min)
                _after(op_i, on_i)
                if w0 != 1.0:
                    nc.vector.tensor_scalar(Op_f[:, msl, :], Op_f[:, msl, :],
                                            float(w0), None, ALU.mult)
                nc.sync.dma_start(
                    dram_row(out_d, 9)[:, msl, :],
                    O9[:, s * (w // 2):(s + 1) * (w // 2)].rearrange(
                        "p (g c) -> p g c", g=2))
            # -Rp for stage 8 on Act while stage 9 finishes
            nR_tiles[8] = ng_pool.tile([P, h], F16, tag="nRp", name="nRp8")
            nr8_i = nc.scalar.activation(nR_tiles[8][:], rp_of(8), ACTF.Copy,
                                         bias=0.0, scale=-1.0)
            _after(nr8_i, nlp_insts[1])
            L = O9

            for i in reversed(range(NUM_STAGES - 1)):
                w0, w1 = weights[i]
                stage0 = i == 0
                R = R_tiles[i]
                # prefetch R two stages ahead (stage 0's row arrives
                # host-deinterleaved in r0_d)
                if i - 2 >= 0:
                    R_tiles[i - 2] = r_pool.tile([P, w], F16, tag="rin",
                                                 name=f"R{i - 2}")
                    src = dram_row(r0_d) if i == 2 else dram_row(right_d, i - 2)
                    nc.sync.dma_start(sb(R_tiles[i - 2][:]), src)

                if stage0:
                    # adjacent pairs break the 16-bit packed fast path:
                    # deinterleave L row 1 into packed halves first
                    lv = L[:].rearrange("p (m two) -> p m two", two=2)
                    Lp_t = pk_pool.tile([P, h], F16, tag="lpk", name="Lp0")
                    nc.vector.tensor_copy(Lp_t[:], lv[:, :, 0])
                    Ln_t = pk_pool.tile([P, h], F16, tag="lnk", name="Ln0")
                    nc.scalar.activation(Ln_t[:], lv[:, :, 1], ACTF.Copy,
                                         bias=0.0, scale=1.0)
                    Lp, Ln = Lp_t[:], Ln_t[:]
                    # host-deinterleaved r0: per (p,g) code block the first
                    # 512 are evens (pos), last 512 odds (neg) == r=512 halves
                    Rp, Rn = _halves(R[:], 9)
                else:
                    Lp, Ln = _halves(L[:], i)
                    Rp, Rn = _halves(R[:], i)
                nRp = nR_tiles.pop(i)

                # DVE queue [t, v1, v2, u2, mv, u1, ON, OP]: no op directly
                # follows its producer (kills the RAW side-effect gaps), the
                # next stage's t needs only ON (2 slots before OP -> seamless
                # stage handoff), and u1 sits late enough that Pool's nt is
                # always ready.
                nLp = ng_pool.tile([P, h], F16, tag="nLp", name=f"nLp{i}")
                nLp_i = nc.scalar.activation(nLp[:], Lp, ACTF.Copy, bias=0.0,
                                             scale=-1.0)

                t = t_pool.tile([P, h], F16, tag="t", name=f"t{i}")
                t_i = nc.vector.tensor_tensor(t[:], Ln, Rn, ALU.add)
                v1 = v_pool.tile([P, h], F16, tag="v1", name=f"v1_{i}")
                v1_i = nc.vector.tensor_tensor(v1[:], Lp, nRp[:], ALU.max)
                _after(v1_i, t_i)
                # nt on Pool (tensor_scalar IS implemented on gpsimd): the
                # u-chain has slack, and it takes 594ns off the DVE
                nt = t_pool.tile([P, h], F16, tag="nt", name=f"nt{i}")
                nc.gpsimd.tensor_scalar(nt[:], t[:], -1.0, None, ALU.mult)

                v2 = v_pool.tile([P, h], F16, tag="v2", name=f"v2_{i}")
                v2_i = nc.vector.tensor_tensor(v2[:], nLp[:], Rp, ALU.max)
                _after(v2_i, v1_i)
                u2 = u_pool.tile([P, h], F16, tag="u2", name=f"u2_{i}")
                u2_i = nc.vector.tensor_tensor(u2[:], nLp[:], t[:], ALU.max)
                _after(u2_i, v2_i)

                O = lo_pool.tile([P, w], F16, tag="lo", name=f"O{i}")
                Op, On = _halves(O[:], 9 if stage0 else i)

                mv = m_pool.tile([P, h], F16, tag="mv", name=f"mv{i}")
                mv_i = nc.vector.tensor_tensor(mv[:], v1[:], v2[:], ALU.min)
                _after(mv_i, u2_i)
                if w1 != 1.0:
                    mv_i = nc.vector.tensor_scalar(mv[:], mv[:], float(w1),
                                                   None, ALU.mult)
                u1 = u_pool.tile([P, h], F16, tag="u1", name=f"u1_{i}")
                u1_i = nc.vector.tensor_tensor(u1[:], Lp, nt[:], ALU.max)
                _after(u1_i, mv_i)
                on_i = nc.vector.tensor_tensor(On, mv[:], Ln, ALU.add)
                _after(on_i, u1_i)
                op_i = nc.vector.tensor_tensor(Op, u1[:], u2[:], ALU.min)
                _after(op_i, on_i)
                if w0 != 1.0:
                    op_i = nc.vector.tensor_scalar(Op, Op, float(w0), None,
                                                   ALU.mult)

                # -Rp for the NEXT stage while DVE finishes this one (kept
                # behind nLp on Act so it can't steal nLp's slot)
                if i >= 1:
                    nR_tiles[i - 1] = ng_pool.tile([P, h], F16, tag="nRp",
                                                   name=f"nRp{i - 1}")
                    nrp_i = nc.scalar.activation(nR_tiles[i - 1][:],
                                                 rp_of(i - 1), ACTF.Copy,
                                                 bias=0.0, scale=-1.0)
                    _after(nrp_i, nLp_i)

                if stage0:
                    # split writeback: neg half leaves while OP still computes
                    nc.sync.dma_start(dram_row(out_d, 0)[:, :, h // g:], On)
                    nc.sync.dma_start(dram_row(out_d, 0)[:, :, :h // g], Op)
                else:
                    nc.sync.dma_start(dram_row(out_d, i), sb(O[:]))
                L = O


TRACE = False
LAST_RESULTS = None


def _make_nc(weights, bpc):
    nc = bacc.Bacc("TRN2", target_bir_lowering=False, debug=False)
    build(nc, weights, bpc)
    nc.compile()
    return nc


def kernel(right, left, left_weights, iter):
    right = np.asarray(right)
    left = np.asarray(left)
    wsel = np.asarray(left_weights, dtype=np.float32)[int(iter)]  # [10, 2]
    weights = [(float(wsel[i, 0]), float(wsel[i, 1])) for i in range(NUM_STAGES)]

    bpc = B // N_CORES
    nc = _make_nc(weights, bpc)

    right16 = right[:, :NUM_STAGES, :].astype(np.float16)
    left10_16 = left[:, NUM_STAGES, :].astype(np.float16)
    # stage-0 right row, host-deinterleaved into packed pos|neg halves
    r0 = np.empty((B, CODE), np.float16)
    r0[:, :CODE // 2] = right16[:, 0, 0::2]
    r0[:, CODE // 2:] = right16[:, 0, 1::2]

    in_maps = []
    for c in range(N_CORES):
        sl = slice(c * bpc, (c + 1) * bpc)
        in_maps.append({
            "right": np.ascontiguousarray(right16[sl]),
            "right0": np.ascontiguousarray(r0[sl]),
            "left10": np.ascontiguousarray(left10_16[sl]),
        })
    global LAST_RESULTS
    LAST_RESULTS = run_bass_kernel_spmd(
        nc, in_maps, list(range(N_CORES)), trace=TRACE)
    res = LAST_RESULTS.results

    out16 = np.empty((B, NUM_STAGES, CODE), np.float16)
    for c in range(N_CORES):
        out16[c * bpc:(c + 1) * bpc] = res[c]["out"]
    # row 0 left the device as packed pos|neg halves; re-interleave
    row0 = out16[:, 0, :].copy()
    out16[:, 0, 0::2] = row0[:, :CODE // 2]
    out16[:, 0, 1::2] = row0[:, CODE // 2:]

    out = np.empty((B, NUM_STAGES + 1, CODE), np.float32)
    out[:, :NUM_STAGES, :] = np.clip(out16.astype(np.float32), -CLIP, CLIP)
    out[:, NUM_STAGES, :] = np.clip(left[:, NUM_STAGES, :], -CLIP, CLIP)
    return out


# revision 32
# speedup vs baseline: 2.0170x; 1.0415x over previous
"""Trainium2 Bass kernel: polar-BP left-message butterfly (nn_IterateLeftLayer).

Math per stage i (9..0), with L = left row i+1 (unclipped), R = right row i:
  out[pos] = w0 * ms(L[pos], L[neg] + R[neg])
  out[neg] = w1 * ms(L[pos], R[pos]) + L[neg]
where ms(x,y) = sign(x)sign(y)min(|x|,|y|), pos = {c: bit i of c == 0},
neg = pos + 2^i.  Final output = clip(left, +-10) with rows 0..9 replaced.

ms is computed in pure min/max form (no bit tricks):
  ms(a,b) = min(max(a,-b), max(-a,b))
which is exact (selection + sign flip only).  All on-device compute is fp16:
every DVE tensor_tensor runs in 2x mode and every tensor_scalar in 4x mode,
and DMA bytes halve.  fp16 rounding keeps rel err ~7e-4 vs the f32 oracle.

The +-10 output clip is applied on the host (the recurrence needs UNclipped
values anyway, so the device never clips).

Engine balance per stage (free dim h=2048 per op); the stage recurrence is
serial in L (every next-stage op needs the FULL previous row), so the per-
stage critical chain is what matters.  Assignment keeps the two cheap-engine
ops (Pool) OFF the chain's tail and the DVE queue ordered so it never
stalls more than ~0.4us:
  DVE : t=Ln+Rn, nt=-t, v2=max(nLp,t), mv=min(v1,v2), ON=mv+Ln,
        u2=max(nLp,t)... order [t, nt, v2, mv, ON, u2, OP=min(u1,u2)]
  Act : nLp=-Lp, then nRp for the NEXT stage (R is prefetched, so -Rp is
        computable one stage early)
  Pool: v1=max(Lp,nRp) (ready at row-start), u1=max(Lp,nt) (needed last)
  SP  : R-row prefetch + output writeback DMAs (double buffered)

Stage 0 pairs adjacent elements (stride 1), which would break the DVE 16-bit
packed fast path; instead L row 1 is deinterleaved on-device into packed
pos/neg halves, right row 0 arrives host-deinterleaved, and out row 0 leaves
as packed halves that the host re-interleaves.

Sharding: pure data-parallel over batch, 512 rows per core on 8 cores.
Layout: batch on partitions (4 groups of 128 coalesced along the free axis
-> [128, 4096] tiles); the butterfly is pure strided access patterns.
"""

import sys

for _p in ("/opt/trn_rl_repo",):
    if _p not in sys.path:
        sys.path.insert(0, _p)

import numpy as np

import bass_rust
import concourse.bass as bass
import concourse.tile as tile
from concourse import bacc, mybir
from concourse.bass_utils import run_bass_kernel_spmd


def _after(inst, *prevs):
    """Order `inst` after `prevs` on the same engine (scheduler edge, no
    semaphore) — pins queue order the greedy list scheduler would flip."""
    names = bass_rust.InstructionNameOrderedSet([p.ins.name for p in prevs])
    inst.ins.add_nosync_dependencies_from(names)

NUM_STAGES = 10
CODE = 1024
B = 4096
N_CORES = 8
P = 128
CLIP = 10.0
F16 = mybir.dt.float16
ALU = mybir.AluOpType
ACTF = mybir.ActivationFunctionType


def _halves(ap, i):
    """pos/neg strided views of a [P, W] row for stage i (i >= 1)."""
    r = 1 << i
    v = ap.rearrange("p (m two r) -> p m two r", two=2, r=r)
    return v[:, :, 0, :], v[:, :, 1, :]


def build(nc, weights, bpc):
    """Emit the per-core kernel. weights: [(w0, w1)] * 10, bpc: batch rows/core."""
    g = bpc // P
    w = g * CODE
    h = w // 2

    right_d = nc.dram_tensor("right", [bpc, NUM_STAGES, CODE], F16,
                             kind="ExternalInput")
    r0_d = nc.dram_tensor("right0", [bpc, CODE], F16, kind="ExternalInput")
    left10_d = nc.dram_tensor("left10", [bpc, CODE], F16, kind="ExternalInput")
    out_d = nc.dram_tensor("out", [bpc, NUM_STAGES, CODE], F16,
                           kind="ExternalOutput")

    def dram_row(dt_, i=None):
        ap = dt_.ap() if i is None else dt_.ap()[:, i, :]
        return ap.rearrange("(g p) c -> p g c", p=P)

    def sb(ap):
        return ap.rearrange("p (g c) -> p g c", g=g)

    with tile.TileContext(nc) as tc:
        with (
            tc.tile_pool(name="lo", bufs=3) as lo_pool,
            tc.tile_pool(name="rin", bufs=3) as r_pool,
            tc.tile_pool(name="tt", bufs=2) as t_pool,
            tc.tile_pool(name="ng", bufs=2) as ng_pool,
            tc.tile_pool(name="uu", bufs=2) as u_pool,
            tc.tile_pool(name="vv", bufs=2) as v_pool,
            tc.tile_pool(name="mv", bufs=2) as m_pool,
            tc.tile_pool(name="pk", bufs=1) as pk_pool,
        ):
            # fill: stage 9 is column-split into two batch-group halves
            # (independent rows), each fed by quarter DMAs in dependency
            # order (neg half first), so compute starts ~3.5us in.
            L = lo_pool.tile([P, w], F16, tag="lo", name="L10")
            R_tiles = {9: r_pool.tile([P, w], F16, tag="rin", name="R9")}
            for s in (0, 1):
                for half in (1, 0):   # neg halves first
                    for dt_, tile_ in ((left10_d, L), (right_d, R_tiles[9])):
                        src = dt_.ap() if dt_ is left10_d else dt_.ap()[:, 9, :]
                        src = src.rearrange(
                            "(g p) (two r) -> p g two r",
                            p=P, two=2, r=CODE // 2)[:, 2 * s:2 * s + 2, half, :]
                        dst = tile_[:].rearrange(
                            "p (g two r) -> p g two r",
                            two=2, r=CODE // 2)[:, 2 * s:2 * s + 2, half, :]
                        nc.sync.dma_start(dst, src)
            for i in (8,):
                R_tiles[i] = r_pool.tile([P, w], F16, tag="rin", name=f"R{i}")
                nc.sync.dma_start(sb(R_tiles[i][:]), dram_row(right_d, i))

            def rp_of(j):
                """pos-half view of stage j's R tile."""
                return _halves(R_tiles[j][:], 9 if j == 0 else j)[0]

            nR_tiles = {}

            # ---- stage 9, column-split into two independent batch-group
            # halves so each starts as soon as its quarter-DMAs land.
            # nRp9 runs on DVE (idle during fill) instead of Act.
            w0, w1 = weights[9]
            R9 = R_tiles[9]
            R_tiles[7] = r_pool.tile([P, w], F16, tag="rin", name="R7")
            nc.sync.dma_start(sb(R_tiles[7][:]), dram_row(right_d, 7))
            Lp_f, Ln_f = _halves(L[:], 9)
            Rp_f, Rn_f = _halves(R9[:], 9)
            h2 = h // 2
            nLp9 = ng_pool.tile([P, h], F16, tag="nLp", name="nLp9")
            nRp9 = ng_pool.tile([P, h], F16, tag="nRp9", name="nRp9")
            t9 = t_pool.tile([P, h], F16, tag="t", name="t9")
            nt9 = t_pool.tile([P, h], F16, tag="nt", name="nt9")
            v19 = v_pool.tile([P, h], F16, tag="v1", name="v1_9")
            v29 = v_pool.tile([P, h], F16, tag="v2", name="v2_9")
            u19 = u_pool.tile([P, h], F16, tag="u1", name="u1_9")
            u29 = u_pool.tile([P, h], F16, tag="u2", name="u2_9")
            mv9 = m_pool.tile([P, h], F16, tag="mv", name="mv9")
            O9 = lo_pool.tile([P, w], F16, tag="lo", name="O9")
            Op_f, On_f = _halves(O9[:], 9)
            nlp_insts = []
            for s in (0, 1):
                msl = slice(2 * s, 2 * s + 2)
                csl = slice(s * h2, (s + 1) * h2)
                Lp, Ln = Lp_f[:, msl, :], Ln_f[:, msl, :]
                Rp, Rn = Rp_f[:, msl, :], Rn_f[:, msl, :]
                nlp_insts.append(nc.scalar.activation(
                    nLp9[:, csl], Lp, ACTF.Copy, bias=0.0, scale=-1.0))
                t_i = nc.vector.tensor_tensor(t9[:, csl], Ln, Rn, ALU.add)
                nc.gpsimd.tensor_scalar(nt9[:, csl], t9[:, csl], -1.0, None,
                                        ALU.mult)
                nr_i = nc.vector.tensor_scalar(nRp9[:, csl], Rp, -1.0, None,
                                               ALU.mult)
                _after(nr_i, t_i)
                v1_i = nc.vector.tensor_tensor(v19[:, csl], Lp, nRp9[:, csl],
                                               ALU.max)
                _after(v1_i, nr_i)
                v2_i = nc.vector.tensor_tensor(v29[:, csl], nLp9[:, csl], Rp,
                                               ALU.max)
                _after(v2_i, v1_i)
                u2_i = nc.vector.tensor_tensor(u29[:, csl], nLp9[:, csl],
                                               t9[:, csl], ALU.max)
                _after(u2_i, v2_i)
                mv_i = nc.vector.tensor_tensor(mv9[:, csl], v19[:, csl],
                                               v29[:, csl], ALU.min)
                _after(mv_i, u2_i)
                if w1 != 1.0:
                    mv_i = nc.vector.tensor_scalar(mv9[:, csl], mv9[:, csl],
                                                   float(w1), None, ALU.mult)
                u1_i = nc.vector.tensor_tensor(u19[:, csl], Lp, nt9[:, csl],
                                               ALU.max)
                _after(u1_i, mv_i)
                on_i = nc.vector.tensor_tensor(On_f[:, msl, :], mv9[:, csl],
                                               Ln, ALU.add)
                _after(on_i, u1_i)
                op_i = nc.vector.tensor_tensor(Op_f[:, msl, :], u19[:, csl],
                                               u29[:, csl], ALU.min)
                _after(op_i, on_i)
                if w0 != 1.0:
                    nc.vector.tensor_scalar(Op_f[:, msl, :], Op_f[:, msl, :],
                                            float(w0), None, ALU.mult)
                nc.sync.dma_start(
                    dram_row(out_d, 9)[:, msl, :],
                    O9[:, s * (w // 2):(s + 1) * (w // 2)].rearrange(
                        "p (g c) -> p g c", g=2))
            # -Rp for stage 8 on Act while stage 9 finishes
            nR_tiles[8] = ng_pool.tile([P, h], F16, tag="nRp", name="nRp8")
            nr8_i = nc.scalar.activation(nR_tiles[8][:], rp_of(8), ACTF.Copy,
                                         bias=0.0, scale=-1.0)
            _after(nr8_i, nlp_insts[1])
            L = O9

            for i in reversed(range(1, NUM_STAGES - 1)):
                w0, w1 = weights[i]
                R = R_tiles[i]
                # prefetch R two stages ahead (stage 0's row arrives
                # host-deinterleaved in r0_d)
                if i - 2 >= 0:
                    R_tiles[i - 2] = r_pool.tile([P, w], F16, tag="rin",
                                                 name=f"R{i - 2}")
                    src = dram_row(r0_d) if i == 2 else dram_row(right_d, i - 2)
                    nc.sync.dma_start(sb(R_tiles[i - 2][:]), src)

                Lp, Ln = _halves(L[:], i)
                Rp, Rn = _halves(R[:], i)
                nRp = nR_tiles.pop(i)

                # DVE queue [t, v1, v2, u2, mv, u1, ON, OP]: no op directly
                # follows its producer (kills the RAW side-effect gaps), the
                # next stage's t needs only ON (2 slots before OP -> seamless
                # stage handoff), and u1 sits late enough that Pool's nt is
                # always ready.
                nLp = ng_pool.tile([P, h], F16, tag="nLp", name=f"nLp{i}")
                nLp_i = nc.scalar.activation(nLp[:], Lp, ACTF.Copy, bias=0.0,
                                             scale=-1.0)

                t = t_pool.tile([P, h], F16, tag="t", name=f"t{i}")
                t_i = nc.vector.tensor_tensor(t[:], Ln, Rn, ALU.add)
                v1 = v_pool.tile([P, h], F16, tag="v1", name=f"v1_{i}")
                v1_i = nc.vector.tensor_tensor(v1[:], Lp, nRp[:], ALU.max)
                _after(v1_i, t_i)
                # nt on Pool (tensor_scalar IS implemented on gpsimd): the
                # u-chain has slack, and it takes 594ns off the DVE
                nt = t_pool.tile([P, h], F16, tag="nt", name=f"nt{i}")
                nc.gpsimd.tensor_scalar(nt[:], t[:], -1.0, None, ALU.mult)

                v2 = v_pool.tile([P, h], F16, tag="v2", name=f"v2_{i}")
                v2_i = nc.vector.tensor_tensor(v2[:], nLp[:], Rp, ALU.max)
                _after(v2_i, v1_i)
                u2 = u_pool.tile([P, h], F16, tag="u2", name=f"u2_{i}")
                u2_i = nc.vector.tensor_tensor(u2[:], nLp[:], t[:], ALU.max)
                _after(u2_i, v2_i)

                O = lo_pool.tile([P, w], F16, tag="lo", name=f"O{i}")
                Op, On = _halves(O[:], i)

                mv = m_pool.tile([P, h], F16, tag="mv", name=f"mv{i}")
                mv_i = nc.vector.tensor_tensor(mv[:], v1[:], v2[:], ALU.min)
                _after(mv_i, u2_i)
                if w1 != 1.0:
                    mv_i = nc.vector.tensor_scalar(mv[:], mv[:], float(w1),
                                                   None, ALU.mult)
                u1 = u_pool.tile([P, h], F16, tag="u1", name=f"u1_{i}")
                u1_i = nc.vector.tensor_tensor(u1[:], Lp, nt[:], ALU.max)
                _after(u1_i, mv_i)
                on_i = nc.vector.tensor_tensor(On, mv[:], Ln, ALU.add)
                _after(on_i, u1_i)
                op_i = nc.vector.tensor_tensor(Op, u1[:], u2[:], ALU.min)
                _after(op_i, on_i)
                if w0 != 1.0:
                    op_i = nc.vector.tensor_scalar(Op, Op, float(w0), None,
                                                   ALU.mult)

                # -Rp for the NEXT stage while DVE finishes this one (kept
                # behind nLp on Act so it can't steal nLp's slot)
                nR_tiles[i - 1] = ng_pool.tile([P, h], F16, tag="nRp",
                                               name=f"nRp{i - 1}")
                nrp_i = nc.scalar.activation(nR_tiles[i - 1][:],
                                             rp_of(i - 1), ACTF.Copy,
                                             bias=0.0, scale=-1.0)
                _after(nrp_i, nLp_i)

                nc.sync.dma_start(dram_row(out_d, i), sb(O[:]))
                L = O

            # ---- stage 0: adjacent pairs -> deinterleave L row 1 into
            # packed halves (r0 arrives host-deinterleaved), column-split
            # into two batch-group halves so the writeback overlaps the
            # second half's compute.
            w0, w1 = weights[0]
            R0 = R_tiles[0]
            lv = L[:].rearrange("p (m two) -> p m two", two=2)
            Rp_f, Rn_f = _halves(R0[:], 9)
            nRp0 = nR_tiles.pop(0)
            Lp_t = pk_pool.tile([P, h], F16, tag="lpk", name="Lp0")
            Ln_t = pk_pool.tile([P, h], F16, tag="lnk", name="Ln0")
            nLp0 = ng_pool.tile([P, h], F16, tag="nLp", name="nLp0")
            t0 = t_pool.tile([P, h], F16, tag="t", name="t0")
            nt0 = t_pool.tile([P, h], F16, tag="nt", name="nt0")
            v10 = v_pool.tile([P, h], F16, tag="v1", name="v1_0")
            v20 = v_pool.tile([P, h], F16, tag="v2", name="v2_0")
            u10 = u_pool.tile([P, h], F16, tag="u1", name="u1_0")
            u20 = u_pool.tile([P, h], F16, tag="u2", name="u2_0")
            mv0 = m_pool.tile([P, h], F16, tag="mv", name="mv0")
            O0 = lo_pool.tile([P, w], F16, tag="lo", name="O0")
            Op_f, On_f = _halves(O0[:], 9)
            prev_act = None
            for s in (0, 1):
                msl = slice(2 * s, 2 * s + 2)
                csl = slice(s * h2, (s + 1) * h2)
                mm = slice(s * (w // 4), (s + 1) * (w // 4))
                lpc_i = nc.vector.tensor_copy(Lp_t[:, csl], lv[:, mm, 0])
                lnc_i = nc.scalar.activation(Ln_t[:, csl], lv[:, mm, 1],
                                             ACTF.Copy, bias=0.0, scale=1.0)
                nlp_i = nc.scalar.activation(nLp0[:, csl], lv[:, mm, 0],
                                             ACTF.Copy, bias=0.0, scale=-1.0)
                if prev_act is not None:
                    _after(lnc_i, prev_act)
                _after(nlp_i, lnc_i)
                prev_act = nlp_i
                Lp, Ln = Lp_t[:, csl], Ln_t[:, csl]
                Rp, Rn = Rp_f[:, msl, :], Rn_f[:, msl, :]
                t_i = nc.vector.tensor_tensor(t0[:, csl], Ln, Rn, ALU.add)
                _after(t_i, lpc_i)
                nc.gpsimd.tensor_scalar(nt0[:, csl], t0[:, csl], -1.0, None,
                                        ALU.mult)
                v1_i = nc.vector.tensor_tensor(v10[:, csl], Lp, nRp0[:, csl],
                                               ALU.max)
                _after(v1_i, t_i)
                v2_i = nc.vector.tensor_tensor(v20[:, csl], nLp0[:, csl], Rp,
                                               ALU.max)
                _after(v2_i, v1_i)
                u2_i = nc.vector.tensor_tensor(u20[:, csl], nLp0[:, csl],
                                               t0[:, csl], ALU.max)
                _after(u2_i, v2_i)
                mv_i = nc.vector.tensor_tensor(mv0[:, csl], v10[:, csl],
                                               v20[:, csl], ALU.min)
                _after(mv_i, u2_i)
                if w1 != 1.0:
                    mv_i = nc.vector.tensor_scalar(mv0[:, csl], mv0[:, csl],
                                                   float(w1), None, ALU.mult)
                u1_i = nc.vector.tensor_tensor(u10[:, csl], Lp, nt0[:, csl],
                                               ALU.max)
                _after(u1_i, mv_i)
                on_i = nc.vector.tensor_tensor(On_f[:, msl, :], mv0[:, csl],
                                               Ln, ALU.add)
                _after(on_i, u1_i)
                op_i = nc.vector.tensor_tensor(Op_f[:, msl, :], u10[:, csl],
                                               u20[:, csl], ALU.min)
                _after(op_i, on_i)
                if w0 != 1.0:
                    nc.vector.tensor_scalar(Op_f[:, msl, :], Op_f[:, msl, :],
                                            float(w0), None, ALU.mult)
                # split writeback: each half leaves while the rest computes
                nc.sync.dma_start(dram_row(out_d, 0)[:, msl, h // g:],
                                  On_f[:, msl, :])
                nc.sync.dma_start(dram_row(out_d, 0)[:, msl, :h // g],
                                  Op_f[:, msl, :])


TRACE = False
LAST_RESULTS = None


def _make_nc(weights, bpc):
    nc = bacc.Bacc("TRN2", target_bir_lowering=False, debug=False)
    build(nc, weights, bpc)
    nc.compile()
    return nc


def kernel(right, left, left_weights, iter):
    right = np.asarray(right)
    left = np.asarray(left)
    wsel = np.asarray(left_weights, dtype=np.float32)[int(iter)]  # [10, 2]
    weights = [(float(wsel[i, 0]), float(wsel[i, 1])) for i in range(NUM_STAGES)]

    bpc = B // N_CORES
    nc = _make_nc(weights, bpc)

    right16 = right[:, :NUM_STAGES, :].astype(np.float16)
    left10_16 = left[:, NUM_STAGES, :].astype(np.float16)
    # stage-0 right row, host-deinterleaved into packed pos|neg halves
    r0 = np.empty((B, CODE), np.float16)
    r0[:, :CODE // 2] = right16[:, 0, 0::2]
    r0[:, CODE // 2:] = right16[:, 0, 1::2]

    in_maps = []
    for c in range(N_CORES):
        sl = slice(c * bpc, (c + 1) * bpc)
        in_maps.append({
            "right": np.ascontiguousarray(right16[sl]),
            "right0": np.ascontiguousarray(r0[sl]),
            "left10": np.ascontiguousarray(left10_16[sl]),
        })
    global LAST_RESULTS
    LAST_RESULTS = run_bass_kernel_spmd(
        nc, in_maps, list(range(N_CORES)), trace=TRACE)
    res = LAST_RESULTS.results

    out16 = np.empty((B, NUM_STAGES, CODE), np.float16)
    for c in range(N_CORES):
        out16[c * bpc:(c + 1) * bpc] = res[c]["out"]
    # row 0 left the device as packed pos|neg halves; re-interleave
    row0 = out16[:, 0, :].copy()
    out16[:, 0, 0::2] = row0[:, :CODE // 2]
    out16[:, 0, 1::2] = row0[:, CODE // 2:]

    out = np.empty((B, NUM_STAGES + 1, CODE), np.float32)
    out[:, :NUM_STAGES, :] = np.clip(out16.astype(np.float32), -CLIP, CLIP)
    out[:, NUM_STAGES, :] = np.clip(left[:, NUM_STAGES, :], -CLIP, CLIP)
    return out


# revision 36
# speedup vs baseline: 2.1112x; 1.0467x over previous
"""Trainium2 Bass kernel: polar-BP left-message butterfly (nn_IterateLeftLayer).

Math per stage i (9..0), with L = left row i+1 (unclipped), R = right row i:
  out[pos] = w0 * ms(L[pos], L[neg] + R[neg])
  out[neg] = w1 * ms(L[pos], R[pos]) + L[neg]
where ms(x,y) = sign(x)sign(y)min(|x|,|y|), pos = {c: bit i of c == 0},
neg = pos + 2^i.  Final output = clip(left, +-10) with rows 0..9 replaced.

ms is computed in pure min/max form (no bit tricks):
  ms(a,b) = min(max(a,-b), max(-a,b))
which is exact (selection + sign flip only).  All on-device compute is fp16:
every DVE tensor_tensor runs in 2x mode and every tensor_scalar in 4x mode,
and DMA bytes halve.  fp16 rounding keeps rel err ~7e-4 vs the f32 oracle.

The +-10 output clip is applied on the host (the recurrence needs UNclipped
values anyway, so the device never clips).

Engine balance per stage (free dim h=2048 per op); the stage recurrence is
serial in L (every next-stage op needs the FULL previous row), so the per-
stage critical chain is what matters.  Assignment keeps the two cheap-engine
ops (Pool) OFF the chain's tail and the DVE queue ordered so it never
stalls more than ~0.4us:
  DVE : t=Ln+Rn, nt=-t, v2=max(nLp,t), mv=min(v1,v2), ON=mv+Ln,
        u2=max(nLp,t)... order [t, nt, v2, mv, ON, u2, OP=min(u1,u2)]
  Act : nLp=-Lp, then nRp for the NEXT stage (R is prefetched, so -Rp is
        computable one stage early)
  Pool: v1=max(Lp,nRp) (ready at row-start), u1=max(Lp,nt) (needed last)
  SP  : R-row prefetch + output writeback DMAs (double buffered)

Stage 0 pairs adjacent elements (stride 1), which would break the DVE 16-bit
packed fast path; instead L row 1 is deinterleaved on-device into packed
pos/neg halves, right row 0 arrives host-deinterleaved, and out row 0 leaves
as packed halves that the host re-interleaves.

Sharding: pure data-parallel over batch, 512 rows per core on 8 cores.
Layout: batch on partitions (4 groups of 128 coalesced along the free axis
-> [128, 4096] tiles); the butterfly is pure strided access patterns.
"""

import sys

for _p in ("/opt/trn_rl_repo",):
    if _p not in sys.path:
        sys.path.insert(0, _p)

import numpy as np

import bass_rust
import concourse.bass as bass
import concourse.tile as tile
from concourse import bacc, mybir
from concourse.bass_utils import run_bass_kernel_spmd


def _after(inst, *prevs):
    """Order `inst` after `prevs` on the same engine (scheduler edge, no
    semaphore) — pins queue order the greedy list scheduler would flip."""
    names = bass_rust.InstructionNameOrderedSet([p.ins.name for p in prevs])
    inst.ins.add_nosync_dependencies_from(names)

NUM_STAGES = 10
CODE = 1024
B = 4096
N_CORES = 8
P = 128
CLIP = 10.0
F16 = mybir.dt.float16
ALU = mybir.AluOpType
ACTF = mybir.ActivationFunctionType


def _halves(ap, i):
    """pos/neg strided views of a [P, W] row for stage i (i >= 1)."""
    r = 1 << i
    v = ap.rearrange("p (m two r) -> p m two r", two=2, r=r)
    return v[:, :, 0, :], v[:, :, 1, :]


def build(nc, weights, bpc):
    """Emit the per-core kernel. weights: [(w0, w1)] * 10, bpc: batch rows/core."""
    g = bpc // P
    w = g * CODE
    h = w // 2

    right_d = nc.dram_tensor("right", [bpc, NUM_STAGES, CODE], F16,
                             kind="ExternalInput")
    r0_d = nc.dram_tensor("right0", [bpc, CODE], F16, kind="ExternalInput")
    left10_d = nc.dram_tensor("left10", [bpc, CODE], F16, kind="ExternalInput")
    out_d = nc.dram_tensor("out", [bpc, NUM_STAGES, CODE], F16,
                           kind="ExternalOutput")

    def dram_row(dt_, i=None):
        ap = dt_.ap() if i is None else dt_.ap()[:, i, :]
        return ap.rearrange("(g p) c -> p g c", p=P)

    def sb(ap):
        return ap.rearrange("p (g c) -> p g c", g=g)

    with tile.TileContext(nc) as tc:
        with (
            tc.tile_pool(name="lo", bufs=3) as lo_pool,
            tc.tile_pool(name="rin", bufs=3) as r_pool,
            tc.tile_pool(name="tt", bufs=2) as t_pool,
            tc.tile_pool(name="ng", bufs=2) as ng_pool,
            tc.tile_pool(name="uu", bufs=2) as u_pool,
            tc.tile_pool(name="vv", bufs=2) as v_pool,
            tc.tile_pool(name="mv", bufs=2) as m_pool,
            tc.tile_pool(name="pk", bufs=1) as pk_pool,
        ):
            # fill: stage 9 is column-split into two batch-group halves
            # (independent rows), each fed by quarter DMAs in dependency
            # order (neg half first), so compute starts ~3.5us in.
            L = lo_pool.tile([P, w], F16, tag="lo", name="L10")
            R_tiles = {9: r_pool.tile([P, w], F16, tag="rin", name="R9")}
            for s in (0, 1):
                for half in (1, 0):   # neg halves first
                    for dt_, tile_ in ((left10_d, L), (right_d, R_tiles[9])):
                        src = dt_.ap() if dt_ is left10_d else dt_.ap()[:, 9, :]
                        src = src.rearrange(
                            "(g p) (two r) -> p g two r",
                            p=P, two=2, r=CODE // 2)[:, 2 * s:2 * s + 2, half, :]
                        dst = tile_[:].rearrange(
                            "p (g two r) -> p g two r",
                            two=2, r=CODE // 2)[:, 2 * s:2 * s + 2, half, :]
                        nc.sync.dma_start(dst, src)
            for i in (8,):
                R_tiles[i] = r_pool.tile([P, w], F16, tag="rin", name=f"R{i}")
                nc.sync.dma_start(sb(R_tiles[i][:]), dram_row(right_d, i))

            def rp_of(j):
                """pos-half view of stage j's R tile."""
                return _halves(R_tiles[j][:], 9 if j == 0 else j)[0]

            nR_tiles = {}

            # ---- stage 9, column-split into two independent batch-group
            # halves so each starts as soon as its quarter-DMAs land.
            # nRp9 runs on DVE (idle during fill) instead of Act.
            w0, w1 = weights[9]
            R9 = R_tiles[9]
            R_tiles[7] = r_pool.tile([P, w], F16, tag="rin", name="R7")
            nc.sync.dma_start(sb(R_tiles[7][:]), dram_row(right_d, 7))
            Lp_f, Ln_f = _halves(L[:], 9)
            Rp_f, Rn_f = _halves(R9[:], 9)
            h2 = h // 2
            nLp9 = ng_pool.tile([P, h], F16, tag="nLp", name="nLp9")
            nRp9 = ng_pool.tile([P, h], F16, tag="nRp9", name="nRp9")
            t9 = t_pool.tile([P, h], F16, tag="t", name="t9")
            nt9 = t_pool.tile([P, h], F16, tag="nt", name="nt9")
            v19 = v_pool.tile([P, h], F16, tag="v1", name="v1_9")
            v29 = v_pool.tile([P, h], F16, tag="v2", name="v2_9")
            u19 = u_pool.tile([P, h], F16, tag="u1", name="u1_9")
            u29 = u_pool.tile([P, h], F16, tag="u2", name="u2_9")
            mv9 = m_pool.tile([P, h], F16, tag="mv", name="mv9")
            O9 = lo_pool.tile([P, w], F16, tag="lo", name="O9")
            Op_f, On_f = _halves(O9[:], 9)
            nlp_insts = []
            for s in (0, 1):
                msl = slice(2 * s, 2 * s + 2)
                csl = slice(s * h2, (s + 1) * h2)
                Lp, Ln = Lp_f[:, msl, :], Ln_f[:, msl, :]
                Rp, Rn = Rp_f[:, msl, :], Rn_f[:, msl, :]
                nlp_insts.append(nc.scalar.activation(
                    nLp9[:, csl], Lp, ACTF.Copy, bias=0.0, scale=-1.0))
                t_i = nc.vector.tensor_tensor(t9[:, csl], Ln, Rn, ALU.add)
                nc.gpsimd.tensor_scalar(nt9[:, csl], t9[:, csl], -1.0, None,
                                        ALU.mult)
                nr_i = nc.vector.tensor_scalar(nRp9[:, csl], Rp, -1.0, None,
                                               ALU.mult)
                _after(nr_i, t_i)
                v1_i = nc.vector.tensor_tensor(v19[:, csl], Lp, nRp9[:, csl],
                                               ALU.max)
                _after(v1_i, nr_i)
                v2_i = nc.vector.tensor_tensor(v29[:, csl], nLp9[:, csl], Rp,
                                               ALU.max)
                _after(v2_i, v1_i)
                u2_i = nc.vector.tensor_tensor(u29[:, csl], nLp9[:, csl],
                                               t9[:, csl], ALU.max)
                _after(u2_i, v2_i)
                mv_i = nc.vector.tensor_tensor(mv9[:, csl], v19[:, csl],
                                               v29[:, csl], ALU.min)
                _after(mv_i, u2_i)
                if w1 != 1.0:
                    mv_i = nc.vector.tensor_scalar(mv9[:, csl], mv9[:, csl],
                                                   float(w1), None, ALU.mult)
                u1_i = nc.vector.tensor_tensor(u19[:, csl], Lp, nt9[:, csl],
                                               ALU.max)
                _after(u1_i, mv_i)
                nc.gpsimd.tensor_tensor(
                    On_f[:, 2 * s + 1:2 * s + 2, :],
                    mv9[:, s * h2 + h2 // 2:(s + 1) * h2],
                    Ln_f[:, 2 * s + 1:2 * s + 2, :], ALU.add)
                on_i = nc.vector.tensor_tensor(
                    On_f[:, 2 * s:2 * s + 1, :],
                    mv9[:, s * h2:s * h2 + h2 // 2],
                    Ln_f[:, 2 * s:2 * s + 1, :], ALU.add)
                _after(on_i, u1_i)
                op_i = nc.vector.tensor_tensor(Op_f[:, msl, :], u19[:, csl],
                                               u29[:, csl], ALU.min)
                _after(op_i, on_i)
                if w0 != 1.0:
                    nc.vector.tensor_scalar(Op_f[:, msl, :], Op_f[:, msl, :],
                                            float(w0), None, ALU.mult)
                nc.sync.dma_start(
                    dram_row(out_d, 9)[:, msl, :],
                    O9[:, s * (w // 2):(s + 1) * (w // 2)].rearrange(
                        "p (g c) -> p g c", g=2))
            # -Rp for stage 8 on Act while stage 9 finishes
            nR_tiles[8] = ng_pool.tile([P, h], F16, tag="nRp", name="nRp8")
            nr8_i = nc.scalar.activation(nR_tiles[8][:], rp_of(8), ACTF.Copy,
                                         bias=0.0, scale=-1.0)
            _after(nr8_i, nlp_insts[1])
            L = O9

            for i in reversed(range(1, NUM_STAGES - 1)):
                w0, w1 = weights[i]
                R = R_tiles[i]
                # prefetch R two stages ahead (stage 0's row arrives
                # host-deinterleaved in r0_d)
                if i - 2 >= 0:
                    R_tiles[i - 2] = r_pool.tile([P, w], F16, tag="rin",
                                                 name=f"R{i - 2}")
                    src = dram_row(r0_d) if i == 2 else dram_row(right_d, i - 2)
                    nc.sync.dma_start(sb(R_tiles[i - 2][:]), src)

                Lp, Ln = _halves(L[:], i)
                Rp, Rn = _halves(R[:], i)
                nRp = nR_tiles.pop(i)

                # DVE queue [t, v1, v2, u2, mv, u1, ON, OP]: no op directly
                # follows its producer (kills the RAW side-effect gaps), the
                # next stage's t needs only ON (2 slots before OP -> seamless
                # stage handoff), and u1 sits late enough that Pool's nt is
                # always ready.
                nLp = ng_pool.tile([P, h], F16, tag="nLp", name=f"nLp{i}")
                nLp_i = nc.scalar.activation(nLp[:], Lp, ACTF.Copy, bias=0.0,
                                             scale=-1.0)

                t = t_pool.tile([P, h], F16, tag="t", name=f"t{i}")
                t_i = nc.vector.tensor_tensor(t[:], Ln, Rn, ALU.add)
                v1 = v_pool.tile([P, h], F16, tag="v1", name=f"v1_{i}")
                v1_i = nc.vector.tensor_tensor(v1[:], Lp, nRp[:], ALU.max)
                _after(v1_i, t_i)
                # nt on Pool (tensor_scalar IS implemented on gpsimd): the
                # u-chain has slack, and it takes 594ns off the DVE
                nt = t_pool.tile([P, h], F16, tag="nt", name=f"nt{i}")
                nc.gpsimd.tensor_scalar(nt[:], t[:], -1.0, None, ALU.mult)

                v2 = v_pool.tile([P, h], F16, tag="v2", name=f"v2_{i}")
                v2_i = nc.vector.tensor_tensor(v2[:], nLp[:], Rp, ALU.max)
                _after(v2_i, v1_i)
                u2 = u_pool.tile([P, h], F16, tag="u2", name=f"u2_{i}")
                u2_i = nc.vector.tensor_tensor(u2[:], nLp[:], t[:], ALU.max)
                _after(u2_i, v2_i)

                O = lo_pool.tile([P, w], F16, tag="lo", name=f"O{i}")
                Op, On = _halves(O[:], i)

                mv = m_pool.tile([P, h], F16, tag="mv", name=f"mv{i}")
                mv_i = nc.vector.tensor_tensor(mv[:], v1[:], v2[:], ALU.min)
                _after(mv_i, u2_i)
                if w1 != 1.0:
                    mv_i = nc.vector.tensor_scalar(mv[:], mv[:], float(w1),
                                                   None, ALU.mult)
                u1 = u_pool.tile([P, h], F16, tag="u1", name=f"u1_{i}")
                u1_i = nc.vector.tensor_tensor(u1[:], Lp, nt[:], ALU.max)
                _after(u1_i, mv_i)
                # ON = mv + Ln, column-split: Pool (idle after nt) takes the
                # tail half right after mv while the DVE finishes u1/OP — the
                # only stage op Pool can legally run (fp16 tensor add).
                m_i = w >> (i + 1)
                mh = max(1, (m_i * 7 + 8) // 16)   # ~7/16 on DVE, rest Pool
                ch = mh * (1 << i)
                nc.gpsimd.tensor_tensor(On[:, mh:, :], mv[:, ch:], Ln[:, mh:, :],
                                        ALU.add)
                on_i = nc.vector.tensor_tensor(On[:, :mh, :], mv[:, :ch],
                                               Ln[:, :mh, :], ALU.add)
                _after(on_i, u1_i)
                op_i = nc.vector.tensor_tensor(Op, u1[:], u2[:], ALU.min)
                _after(op_i, on_i)
                if w0 != 1.0:
                    op_i = nc.vector.tensor_scalar(Op, Op, float(w0), None,
                                                   ALU.mult)

                # -Rp for the NEXT stage while DVE finishes this one (kept
                # behind nLp on Act so it can't steal nLp's slot)
                nR_tiles[i - 1] = ng_pool.tile([P, h], F16, tag="nRp",
                                               name=f"nRp{i - 1}")
                nrp_i = nc.scalar.activation(nR_tiles[i - 1][:],
                                             rp_of(i - 1), ACTF.Copy,
                                             bias=0.0, scale=-1.0)
                _after(nrp_i, nLp_i)

                nc.sync.dma_start(dram_row(out_d, i), sb(O[:]))
                L = O

            # ---- stage 0: adjacent pairs -> deinterleave L row 1 into
            # packed halves (r0 arrives host-deinterleaved), column-split
            # into two batch-group halves so the writeback overlaps the
            # second half's compute.
            w0, w1 = weights[0]
            R0 = R_tiles[0]
            lv = L[:].rearrange("p (m two) -> p m two", two=2)
            Rp_f, Rn_f = _halves(R0[:], 9)
            nRp0 = nR_tiles.pop(0)
            Lp_t = pk_pool.tile([P, h], F16, tag="lpk", name="Lp0")
            Ln_t = pk_pool.tile([P, h], F16, tag="lnk", name="Ln0")
            nLp0 = ng_pool.tile([P, h], F16, tag="nLp", name="nLp0")
            t0 = t_pool.tile([P, h], F16, tag="t", name="t0")
            nt0 = t_pool.tile([P, h], F16, tag="nt", name="nt0")
            v10 = v_pool.tile([P, h], F16, tag="v1", name="v1_0")
            v20 = v_pool.tile([P, h], F16, tag="v2", name="v2_0")
            u10 = u_pool.tile([P, h], F16, tag="u1", name="u1_0")
            u20 = u_pool.tile([P, h], F16, tag="u2", name="u2_0")
            mv0 = m_pool.tile([P, h], F16, tag="mv", name="mv0")
            O0 = lo_pool.tile([P, w], F16, tag="lo", name="O0")
            Op_f, On_f = _halves(O0[:], 9)
            prev_act = None
            for s in (0, 1):
                msl = slice(2 * s, 2 * s + 2)
                csl = slice(s * h2, (s + 1) * h2)
                mm = slice(s * (w // 4), (s + 1) * (w // 4))
                lpc_i = nc.vector.tensor_copy(Lp_t[:, csl], lv[:, mm, 0])
                lnc_i = nc.scalar.activation(Ln_t[:, csl], lv[:, mm, 1],
                                             ACTF.Copy, bias=0.0, scale=1.0)
                nlp_i = nc.scalar.activation(nLp0[:, csl], lv[:, mm, 0],
                                             ACTF.Copy, bias=0.0, scale=-1.0)
                if prev_act is not None:
                    _after(lnc_i, prev_act)
                _after(nlp_i, lnc_i)
                prev_act = nlp_i
                Lp, Ln = Lp_t[:, csl], Ln_t[:, csl]
                Rp, Rn = Rp_f[:, msl, :], Rn_f[:, msl, :]
                t_i = nc.vector.tensor_tensor(t0[:, csl], Ln, Rn, ALU.add)
                _after(t_i, lpc_i)
                nc.gpsimd.tensor_scalar(nt0[:, csl], t0[:, csl], -1.0, None,
                                        ALU.mult)
                v1_i = nc.vector.tensor_tensor(v10[:, csl], Lp, nRp0[:, csl],
                                               ALU.max)
                _after(v1_i, t_i)
                v2_i = nc.vector.tensor_tensor(v20[:, csl], nLp0[:, csl], Rp,
                                               ALU.max)
                _after(v2_i, v1_i)
                u2_i = nc.vector.tensor_tensor(u20[:, csl], nLp0[:, csl],
                                               t0[:, csl], ALU.max)
                _after(u2_i, v2_i)
                mv_i = nc.vector.tensor_tensor(mv0[:, csl], v10[:, csl],
                                               v20[:, csl], ALU.min)
                _after(mv_i, u2_i)
                if w1 != 1.0:
                    mv_i = nc.vector.tensor_scalar(mv0[:, csl], mv0[:, csl],
                                                   float(w1), None, ALU.mult)
                u1_i = nc.vector.tensor_tensor(u10[:, csl], Lp, nt0[:, csl],
                                               ALU.max)
                _after(u1_i, mv_i)
                nc.gpsimd.tensor_tensor(
                    On_f[:, 2 * s + 1:2 * s + 2, :],
                    mv0[:, s * h2 + h2 // 2:(s + 1) * h2],
                    Ln_t[:, s * h2 + h2 // 2:(s + 1) * h2], ALU.add)
                on_i = nc.vector.tensor_tensor(
                    On_f[:, 2 * s:2 * s + 1, :],
                    mv0[:, s * h2:s * h2 + h2 // 2],
                    Ln_t[:, s * h2:s * h2 + h2 // 2], ALU.add)
                _after(on_i, u1_i)
                op_i = nc.vector.tensor_tensor(Op_f[:, msl, :], u10[:, csl],
                                               u20[:, csl], ALU.min)
                _after(op_i, on_i)
                if w0 != 1.0:
                    nc.vector.tensor_scalar(Op_f[:, msl, :], Op_f[:, msl, :],
                                            float(w0), None, ALU.mult)
                # split writeback: each half leaves while the rest computes
                nc.sync.dma_start(dram_row(out_d, 0)[:, msl, h // g:],
                                  On_f[:, msl, :])
                nc.sync.dma_start(dram_row(out_d, 0)[:, msl, :h // g],
                                  Op_f[:, msl, :])


TRACE = False
LAST_RESULTS = None


def _make_nc(weights, bpc):
    nc = bacc.Bacc("TRN2", target_bir_lowering=False, debug=False)
    build(nc, weights, bpc)
    nc.compile()
    return nc


def kernel(right, left, left_weights, iter):
    right = np.asarray(right)
    left = np.asarray(left)
    wsel = np.asarray(left_weights, dtype=np.float32)[int(iter)]  # [10, 2]
    weights = [(float(wsel[i, 0]), float(wsel[i, 1])) for i in range(NUM_STAGES)]

    bpc = B // N_CORES
    nc = _make_nc(weights, bpc)

    right16 = right[:, :NUM_STAGES, :].astype(np.float16)
    left10_16 = left[:, NUM_STAGES, :].astype(np.float16)
    # stage-0 right row, host-deinterleaved into packed pos|neg halves
    r0 = np.empty((B, CODE), np.float16)
    r0[:, :CODE // 2] = right16[:, 0, 0::2]
    r0[:, CODE // 2:] = right16[:, 0, 1::2]

    in_maps = []
    for c in range(N_CORES):
        sl = slice(c * bpc, (c + 1) * bpc)
        in_maps.append({
            "right": np.ascontiguousarray(right16[sl]),
            "right0": np.ascontiguousarray(r0[sl]),
            "left10": np.ascontiguousarray(left10_16[sl]),
        })
    global LAST_RESULTS
    LAST_RESULTS = run_bass_kernel_spmd(
        nc, in_maps, list(range(N_CORES)), trace=TRACE)
    res = LAST_RESULTS.results

    out16 = np.empty((B, NUM_STAGES, CODE), np.float16)
    for c in range(N_CORES):
        out16[c * bpc:(c + 1) * bpc] = res[c]["out"]
    # row 0 left the device as packed pos|neg halves; re-interleave
    row0 = out16[:, 0, :].copy()
    out16[:, 0, 0::2] = row0[:, :CODE // 2]
    out16[:, 0, 1::2] = row0[:, CODE // 2:]

    out = np.empty((B, NUM_STAGES + 1, CODE), np.float32)
    out[:, :NUM_STAGES, :] = np.clip(out16.astype(np.float32), -CLIP, CLIP)
    out[:, NUM_STAGES, :] = np.clip(left[:, NUM_STAGES, :], -CLIP, CLIP)
    return out


# revision 38
# speedup vs baseline: 2.1395x; 1.0134x over previous
"""Trainium2 Bass kernel: polar-BP left-message butterfly (nn_IterateLeftLayer).

Math per stage i (9..0), with L = left row i+1 (unclipped), R = right row i:
  out[pos] = w0 * ms(L[pos], L[neg] + R[neg])
  out[neg] = w1 * ms(L[pos], R[pos]) + L[neg]
where ms(x,y) = sign(x)sign(y)min(|x|,|y|), pos = {c: bit i of c == 0},
neg = pos + 2^i.  Final output = clip(left, +-10) with rows 0..9 replaced.

ms is computed in pure min/max form (no bit tricks):
  ms(a,b) = min(max(a,-b), max(-a,b))
which is exact (selection + sign flip only).  All on-device compute is fp16:
every DVE tensor_tensor runs in 2x mode and every tensor_scalar in 4x mode,
and DMA bytes halve.  fp16 rounding keeps rel err ~7e-4 vs the f32 oracle.

The +-10 output clip is applied on the host (the recurrence needs UNclipped
values anyway, so the device never clips).

Engine balance per stage (free dim h=2048 per op); the stage recurrence is
serial in L (every next-stage op needs the FULL previous row), so the per-
stage critical chain is what matters.  Assignment keeps the two cheap-engine
ops (Pool) OFF the chain's tail and the DVE queue ordered so it never
stalls more than ~0.4us:
  DVE : t=Ln+Rn, nt=-t, v2=max(nLp,t), mv=min(v1,v2), ON=mv+Ln,
        u2=max(nLp,t)... order [t, nt, v2, mv, ON, u2, OP=min(u1,u2)]
  Act : nLp=-Lp, then nRp for the NEXT stage (R is prefetched, so -Rp is
        computable one stage early)
  Pool: v1=max(Lp,nRp) (ready at row-start), u1=max(Lp,nt) (needed last)
  SP  : R-row prefetch + output writeback DMAs (double buffered)

Stage 0 pairs adjacent elements (stride 1), which would break the DVE 16-bit
packed fast path; instead L row 1 is deinterleaved on-device into packed
pos/neg halves, right row 0 arrives host-deinterleaved, and out row 0 leaves
as packed halves that the host re-interleaves.

Sharding: pure data-parallel over batch, 512 rows per core on 8 cores.
Layout: batch on partitions (4 groups of 128 coalesced along the free axis
-> [128, 4096] tiles); the butterfly is pure strided access patterns.
"""

import sys

for _p in ("/opt/trn_rl_repo",):
    if _p not in sys.path:
        sys.path.insert(0, _p)

import numpy as np

import bass_rust
import concourse.bass as bass
import concourse.tile as tile
from concourse import bacc, mybir
from concourse.bass_utils import run_bass_kernel_spmd


def _after(inst, *prevs):
    """Order `inst` after `prevs` on the same engine (scheduler edge, no
    semaphore) — pins queue order the greedy list scheduler would flip."""
    names = bass_rust.InstructionNameOrderedSet([p.ins.name for p in prevs])
    inst.ins.add_nosync_dependencies_from(names)

NUM_STAGES = 10
CODE = 1024
B = 4096
N_CORES = 8
P = 128
CLIP = 10.0
F16 = mybir.dt.float16
ALU = mybir.AluOpType
ACTF = mybir.ActivationFunctionType


def _halves(ap, i):
    """pos/neg strided views of a [P, W] row for stage i (i >= 1)."""
    r = 1 << i
    v = ap.rearrange("p (m two r) -> p m two r", two=2, r=r)
    return v[:, :, 0, :], v[:, :, 1, :]


def build(nc, weights, bpc):
    """Emit the per-core kernel. weights: [(w0, w1)] * 10, bpc: batch rows/core."""
    g = bpc // P
    w = g * CODE
    h = w // 2

    right_d = nc.dram_tensor("right", [bpc, NUM_STAGES, CODE], F16,
                             kind="ExternalInput")
    r0_d = nc.dram_tensor("right0", [bpc, CODE], F16, kind="ExternalInput")
    left10_d = nc.dram_tensor("left10", [bpc, CODE], F16, kind="ExternalInput")
    out_d = nc.dram_tensor("out", [bpc, NUM_STAGES, CODE], F16,
                           kind="ExternalOutput")

    def dram_row(dt_, i=None):
        ap = dt_.ap() if i is None else dt_.ap()[:, i, :]
        return ap.rearrange("(g p) c -> p g c", p=P)

    def sb(ap):
        return ap.rearrange("p (g c) -> p g c", g=g)

    with tile.TileContext(nc) as tc:
        with (
            tc.tile_pool(name="lo", bufs=3) as lo_pool,
            tc.tile_pool(name="rin", bufs=3) as r_pool,
            tc.tile_pool(name="tt", bufs=2) as t_pool,
            tc.tile_pool(name="ng", bufs=2) as ng_pool,
            tc.tile_pool(name="uu", bufs=2) as u_pool,
            tc.tile_pool(name="vv", bufs=2) as v_pool,
            tc.tile_pool(name="mv", bufs=2) as m_pool,
            tc.tile_pool(name="pk", bufs=1) as pk_pool,
        ):
            # fill: stage 9 is column-split into two batch-group halves
            # (independent rows), each fed by quarter DMAs in dependency
            # order (neg half first), so compute starts ~3.5us in.
            L = lo_pool.tile([P, w], F16, tag="lo", name="L10")
            R_tiles = {9: r_pool.tile([P, w], F16, tag="rin", name="R9")}
            for s in (0, 1):
                for half in (1, 0):   # neg halves first
                    for dt_, tile_ in ((left10_d, L), (right_d, R_tiles[9])):
                        src = dt_.ap() if dt_ is left10_d else dt_.ap()[:, 9, :]
                        src = src.rearrange(
                            "(g p) (two r) -> p g two r",
                            p=P, two=2, r=CODE // 2)[:, 2 * s:2 * s + 2, half, :]
                        dst = tile_[:].rearrange(
                            "p (g two r) -> p g two r",
                            two=2, r=CODE // 2)[:, 2 * s:2 * s + 2, half, :]
                        nc.sync.dma_start(dst, src)
            for i in (8,):
                R_tiles[i] = r_pool.tile([P, w], F16, tag="rin", name=f"R{i}")
                nc.sync.dma_start(sb(R_tiles[i][:]), dram_row(right_d, i))

            def rp_of(j):
                """pos-half view of stage j's R tile."""
                return _halves(R_tiles[j][:], 9 if j == 0 else j)[0]

            nR_tiles = {}

            # ---- stage 9, column-split into two independent batch-group
            # halves so each starts as soon as its quarter-DMAs land.
            # nRp9 runs on DVE (idle during fill) instead of Act.
            w0, w1 = weights[9]
            R9 = R_tiles[9]
            R_tiles[7] = r_pool.tile([P, w], F16, tag="rin", name="R7")
            nc.sync.dma_start(sb(R_tiles[7][:]), dram_row(right_d, 7))
            Lp_f, Ln_f = _halves(L[:], 9)
            Rp_f, Rn_f = _halves(R9[:], 9)
            h2 = h // 2
            nLp9 = ng_pool.tile([P, h], F16, tag="nLp", name="nLp9")
            nRp9 = ng_pool.tile([P, h], F16, tag="nRp9", name="nRp9")
            t9 = t_pool.tile([P, h], F16, tag="t", name="t9")
            nt9 = t_pool.tile([P, h], F16, tag="nt", name="nt9")
            v19 = v_pool.tile([P, h], F16, tag="v1", name="v1_9")
            v29 = v_pool.tile([P, h], F16, tag="v2", name="v2_9")
            u19 = u_pool.tile([P, h], F16, tag="u1", name="u1_9")
            u29 = u_pool.tile([P, h], F16, tag="u2", name="u2_9")
            mv9 = m_pool.tile([P, h], F16, tag="mv", name="mv9")
            O9 = lo_pool.tile([P, w], F16, tag="lo", name="O9")
            Op_f, On_f = _halves(O9[:], 9)
            nlp_insts = []
            for s in (0, 1):
                msl = slice(2 * s, 2 * s + 2)
                csl = slice(s * h2, (s + 1) * h2)
                Lp, Ln = Lp_f[:, msl, :], Ln_f[:, msl, :]
                Rp, Rn = Rp_f[:, msl, :], Rn_f[:, msl, :]
                nlp_insts.append(nc.scalar.activation(
                    nLp9[:, csl], Lp, ACTF.Copy, bias=0.0, scale=-1.0))
                t_i = nc.vector.tensor_tensor(t9[:, csl], Ln, Rn, ALU.add)
                nc.gpsimd.tensor_scalar(nt9[:, csl], t9[:, csl], -1.0, None,
                                        ALU.mult)
                nr_i = nc.vector.tensor_scalar(nRp9[:, csl], Rp, -1.0, None,
                                               ALU.mult)
                _after(nr_i, t_i)
                v1_i = nc.vector.tensor_tensor(v19[:, csl], Lp, nRp9[:, csl],
                                               ALU.max)
                _after(v1_i, nr_i)
                v2_i = nc.vector.tensor_tensor(v29[:, csl], nLp9[:, csl], Rp,
                                               ALU.max)
                _after(v2_i, v1_i)
                u2_i = nc.vector.tensor_tensor(u29[:, csl], nLp9[:, csl],
                                               t9[:, csl], ALU.max)
                _after(u2_i, v2_i)
                mv_i = nc.vector.tensor_tensor(mv9[:, csl], v19[:, csl],
                                               v29[:, csl], ALU.min)
                _after(mv_i, u2_i)
                if w1 != 1.0:
                    mv_i = nc.vector.tensor_scalar(mv9[:, csl], mv9[:, csl],
                                                   float(w1), None, ALU.mult)
                u1_i = nc.vector.tensor_tensor(u19[:, csl], Lp, nt9[:, csl],
                                               ALU.max)
                _after(u1_i, mv_i)
                nc.gpsimd.tensor_tensor(
                    On_f[:, 2 * s + 1:2 * s + 2, :],
                    mv9[:, s * h2 + h2 // 2:(s + 1) * h2],
                    Ln_f[:, 2 * s + 1:2 * s + 2, :], ALU.add)
                on_i = nc.vector.tensor_tensor(
                    On_f[:, 2 * s:2 * s + 1, :],
                    mv9[:, s * h2:s * h2 + h2 // 2],
                    Ln_f[:, 2 * s:2 * s + 1, :], ALU.add)
                _after(on_i, u1_i)
                op_i = nc.vector.tensor_tensor(Op_f[:, msl, :], u19[:, csl],
                                               u29[:, csl], ALU.min)
                _after(op_i, on_i)
                if w0 != 1.0:
                    nc.vector.tensor_scalar(Op_f[:, msl, :], Op_f[:, msl, :],
                                            float(w0), None, ALU.mult)
                nc.sync.dma_start(
                    dram_row(out_d, 9)[:, msl, :],
                    O9[:, s * (w // 2):(s + 1) * (w // 2)].rearrange(
                        "p (g c) -> p g c", g=2))
            # -Rp for stage 8 on Act while stage 9 finishes
            nR_tiles[8] = ng_pool.tile([P, h], F16, tag="nRp", name="nRp8")
            nr8_i = nc.scalar.activation(nR_tiles[8][:], rp_of(8), ACTF.Copy,
                                         bias=0.0, scale=-1.0)
            _after(nr8_i, nlp_insts[1])
            L = O9

            for i in reversed(range(1, NUM_STAGES - 1)):
                w0, w1 = weights[i]
                R = R_tiles[i]
                # prefetch R two stages ahead (stage 0's row arrives
                # host-deinterleaved in r0_d)
                if i - 2 >= 0:
                    R_tiles[i - 2] = r_pool.tile([P, w], F16, tag="rin",
                                                 name=f"R{i - 2}")
                    src = dram_row(r0_d) if i == 2 else dram_row(right_d, i - 2)
                    nc.sync.dma_start(sb(R_tiles[i - 2][:]), src)

                Lp, Ln = _halves(L[:], i)
                Rp, Rn = _halves(R[:], i)
                nRp = nR_tiles.pop(i)

                # DVE queue [t, v1, v2, u2, mv, u1, ON, OP]: no op directly
                # follows its producer (kills the RAW side-effect gaps), the
                # next stage's t needs only ON (2 slots before OP -> seamless
                # stage handoff), and u1 sits late enough that Pool's nt is
                # always ready.
                nLp = ng_pool.tile([P, h], F16, tag="nLp", name=f"nLp{i}")
                nLp_i = nc.scalar.activation(nLp[:], Lp, ACTF.Copy, bias=0.0,
                                             scale=-1.0)

                # t = Ln + Rn, column-split: Pool is idle before nt, so it
                # takes the tail quarter there
                m_i = w >> (i + 1)
                tq = m_i - max(1, m_i // 4)
                ct = tq << i
                t = t_pool.tile([P, h], F16, tag="t", name=f"t{i}")
                t_i = nc.vector.tensor_tensor(t[:, :ct], Ln[:, :tq, :],
                                              Rn[:, :tq, :], ALU.add)
                nc.gpsimd.tensor_tensor(t[:, ct:], Ln[:, tq:, :],
                                        Rn[:, tq:, :], ALU.add)
                v1 = v_pool.tile([P, h], F16, tag="v1", name=f"v1_{i}")
                v1_i = nc.vector.tensor_tensor(v1[:], Lp, nRp[:], ALU.max)
                _after(v1_i, t_i)
                # nt on Pool (tensor_scalar IS implemented on gpsimd): the
                # u-chain has slack, and it takes 594ns off the DVE
                nt = t_pool.tile([P, h], F16, tag="nt", name=f"nt{i}")
                nc.gpsimd.tensor_scalar(nt[:], t[:], -1.0, None, ALU.mult)

                v2 = v_pool.tile([P, h], F16, tag="v2", name=f"v2_{i}")
                v2_i = nc.vector.tensor_tensor(v2[:], nLp[:], Rp, ALU.max)
                _after(v2_i, v1_i)
                u2 = u_pool.tile([P, h], F16, tag="u2", name=f"u2_{i}")
                u2_i = nc.vector.tensor_tensor(u2[:], nLp[:], t[:], ALU.max)
                _after(u2_i, v2_i)

                O = lo_pool.tile([P, w], F16, tag="lo", name=f"O{i}")
                Op, On = _halves(O[:], i)

                mv = m_pool.tile([P, h], F16, tag="mv", name=f"mv{i}")
                mv_i = nc.vector.tensor_tensor(mv[:], v1[:], v2[:], ALU.min)
                _after(mv_i, u2_i)
                if w1 != 1.0:
                    mv_i = nc.vector.tensor_scalar(mv[:], mv[:], float(w1),
                                                   None, ALU.mult)
                u1 = u_pool.tile([P, h], F16, tag="u1", name=f"u1_{i}")
                u1_i = nc.vector.tensor_tensor(u1[:], Lp, nt[:], ALU.max)
                _after(u1_i, mv_i)
                # ON = mv + Ln, column-split: Pool (idle after nt) takes the
                # tail half right after mv while the DVE finishes u1/OP — the
                # only stage op Pool can legally run (fp16 tensor add).
                mh = max(1, (m_i * 7 + 8) // 16)   # ~7/16 on DVE, rest Pool
                ch = mh * (1 << i)
                nc.gpsimd.tensor_tensor(On[:, mh:, :], mv[:, ch:], Ln[:, mh:, :],
                                        ALU.add)
                on_i = nc.vector.tensor_tensor(On[:, :mh, :], mv[:, :ch],
                                               Ln[:, :mh, :], ALU.add)
                _after(on_i, u1_i)
                op_i = nc.vector.tensor_tensor(Op, u1[:], u2[:], ALU.min)
                _after(op_i, on_i)
                if w0 != 1.0:
                    op_i = nc.vector.tensor_scalar(Op, Op, float(w0), None,
                                                   ALU.mult)

                # -Rp for the NEXT stage while DVE finishes this one (kept
                # behind nLp on Act so it can't steal nLp's slot)
                nR_tiles[i - 1] = ng_pool.tile([P, h], F16, tag="nRp",
                                               name=f"nRp{i - 1}")
                nrp_i = nc.scalar.activation(nR_tiles[i - 1][:],
                                             rp_of(i - 1), ACTF.Copy,
                                             bias=0.0, scale=-1.0)
                _after(nrp_i, nLp_i)

                nc.sync.dma_start(dram_row(out_d, i), sb(O[:]))
                L = O

            # ---- stage 0: adjacent pairs -> deinterleave L row 1 into
            # packed halves (r0 arrives host-deinterleaved), column-split
            # into two batch-group halves so the writeback overlaps the
            # second half's compute.
            w0, w1 = weights[0]
            R0 = R_tiles[0]
            lv = L[:].rearrange("p (m two) -> p m two", two=2)
            Rp_f, Rn_f = _halves(R0[:], 9)
            nRp0 = nR_tiles.pop(0)
            Lp_t = pk_pool.tile([P, h], F16, tag="lpk", name="Lp0")
            Ln_t = pk_pool.tile([P, h], F16, tag="lnk", name="Ln0")
            nLp0 = ng_pool.tile([P, h], F16, tag="nLp", name="nLp0")
            t0 = t_pool.tile([P, h], F16, tag="t", name="t0")
            nt0 = t_pool.tile([P, h], F16, tag="nt", name="nt0")
            v10 = v_pool.tile([P, h], F16, tag="v1", name="v1_0")
            v20 = v_pool.tile([P, h], F16, tag="v2", name="v2_0")
            u10 = u_pool.tile([P, h], F16, tag="u1", name="u1_0")
            u20 = u_pool.tile([P, h], F16, tag="u2", name="u2_0")
            mv0 = m_pool.tile([P, h], F16, tag="mv", name="mv0")
            O0 = lo_pool.tile([P, w], F16, tag="lo", name="O0")
            Op_f, On_f = _halves(O0[:], 9)
            prev_act = None
            for s in (0, 1):
                msl = slice(2 * s, 2 * s + 2)
                csl = slice(s * h2, (s + 1) * h2)
                mm = slice(s * (w // 4), (s + 1) * (w // 4))
                lpc_i = nc.vector.tensor_copy(Lp_t[:, csl], lv[:, mm, 0])
                lnc_i = nc.scalar.activation(Ln_t[:, csl], lv[:, mm, 1],
                                             ACTF.Copy, bias=0.0, scale=1.0)
                nlp_i = nc.scalar.activation(nLp0[:, csl], lv[:, mm, 0],
                                             ACTF.Copy, bias=0.0, scale=-1.0)
                if prev_act is not None:
                    _after(lnc_i, prev_act)
                _after(nlp_i, lnc_i)
                prev_act = nlp_i
                Lp, Ln = Lp_t[:, csl], Ln_t[:, csl]
                Rp, Rn = Rp_f[:, msl, :], Rn_f[:, msl, :]
                t_i = nc.vector.tensor_tensor(t0[:, csl], Ln, Rn, ALU.add)
                _after(t_i, lpc_i)
                nc.gpsimd.tensor_scalar(nt0[:, csl], t0[:, csl], -1.0, None,
                                        ALU.mult)
                v1_i = nc.vector.tensor_tensor(v10[:, csl], Lp, nRp0[:, csl],
                                               ALU.max)
                _after(v1_i, t_i)
                v2_i = nc.vector.tensor_tensor(v20[:, csl], nLp0[:, csl], Rp,
                                               ALU.max)
                _after(v2_i, v1_i)
                u2_i = nc.vector.tensor_tensor(u20[:, csl], nLp0[:, csl],
                                               t0[:, csl], ALU.max)
                _after(u2_i, v2_i)
                mv_i = nc.vector.tensor_tensor(mv0[:, csl], v10[:, csl],
                                               v20[:, csl], ALU.min)
                _after(mv_i, u2_i)
                if w1 != 1.0:
                    mv_i = nc.vector.tensor_scalar(mv0[:, csl], mv0[:, csl],
                                                   float(w1), None, ALU.mult)
                u1_i = nc.vector.tensor_tensor(u10[:, csl], Lp, nt0[:, csl],
                                               ALU.max)
                _after(u1_i, mv_i)
                nc.gpsimd.tensor_tensor(
                    On_f[:, 2 * s + 1:2 * s + 2, :],
                    mv0[:, s * h2 + h2 // 2:(s + 1) * h2],
                    Ln_t[:, s * h2 + h2 // 2:(s + 1) * h2], ALU.add)
                on_i = nc.vector.tensor_tensor(
                    On_f[:, 2 * s:2 * s + 1, :],
                    mv0[:, s * h2:s * h2 + h2 // 2],
                    Ln_t[:, s * h2:s * h2 + h2 // 2], ALU.add)
                _after(on_i, u1_i)
                op_i = nc.vector.tensor_tensor(Op_f[:, msl, :], u10[:, csl],
                                               u20[:, csl], ALU.min)
                _after(op_i, on_i)
                if w0 != 1.0:
                    nc.vector.tensor_scalar(Op_f[:, msl, :], Op_f[:, msl, :],
                                            float(w0), None, ALU.mult)
                # split writeback: each half leaves while the rest computes
                nc.sync.dma_start(dram_row(out_d, 0)[:, msl, h // g:],
                                  On_f[:, msl, :])
                nc.sync.dma_start(dram_row(out_d, 0)[:, msl, :h // g],
                                  Op_f[:, msl, :])


TRACE = False
LAST_RESULTS = None


def _make_nc(weights, bpc):
    nc = bacc.Bacc("TRN2", target_bir_lowering=False, debug=False)
    build(nc, weights, bpc)
    nc.compile()
    return nc


def kernel(right, left, left_weights, iter):
    right = np.asarray(right)
    left = np.asarray(left)
    wsel = np.asarray(left_weights, dtype=np.float32)[int(iter)]  # [10, 2]
    weights = [(float(wsel[i, 0]), float(wsel[i, 1])) for i in range(NUM_STAGES)]

    bpc = B // N_CORES
    nc = _make_nc(weights, bpc)

    right16 = right[:, :NUM_STAGES, :].astype(np.float16)
    left10_16 = left[:, NUM_STAGES, :].astype(np.float16)
    # stage-0 right row, host-deinterleaved into packed pos|neg halves
    r0 = np.empty((B, CODE), np.float16)
    r0[:, :CODE // 2] = right16[:, 0, 0::2]
    r0[:, CODE // 2:] = right16[:, 0, 1::2]

    in_maps = []
    for c in range(N_CORES):
        sl = slice(c * bpc, (c + 1) * bpc)
        in_maps.append({
            "right": np.ascontiguousarray(right16[sl]),
            "right0": np.ascontiguousarray(r0[sl]),
            "left10": np.ascontiguousarray(left10_16[sl]),
        })
    global LAST_RESULTS
    LAST_RESULTS = run_bass_kernel_spmd(
        nc, in_maps, list(range(N_CORES)), trace=TRACE)
    res = LAST_RESULTS.results

    out16 = np.empty((B, NUM_STAGES, CODE), np.float16)
    for c in range(N_CORES):
        out16[c * bpc:(c + 1) * bpc] = res[c]["out"]
    # row 0 left the device as packed pos|neg halves; re-interleave
    row0 = out16[:, 0, :].copy()
    out16[:, 0, 0::2] = row0[:, :CODE // 2]
    out16[:, 0, 1::2] = row0[:, CODE // 2:]

    out = np.empty((B, NUM_STAGES + 1, CODE), np.float32)
    out[:, :NUM_STAGES, :] = np.clip(out16.astype(np.float32), -CLIP, CLIP)
    out[:, NUM_STAGES, :] = np.clip(left[:, NUM_STAGES, :], -CLIP, CLIP)
    return out


# revision 44
# speedup vs baseline: 2.1469x; 1.0035x over previous
"""Trainium2 Bass kernel: polar-BP left-message butterfly (nn_IterateLeftLayer).

Math per stage i (9..0), with L = left row i+1 (unclipped), R = right row i:
  out[pos] = w0 * ms(L[pos], L[neg] + R[neg])
  out[neg] = w1 * ms(L[pos], R[pos]) + L[neg]
where ms(x,y) = sign(x)sign(y)min(|x|,|y|), pos = {c: bit i of c == 0},
neg = pos + 2^i.  Final output = clip(left, +-10) with rows 0..9 replaced.

ms is computed in pure min/max form (no bit tricks):
  ms(a,b) = min(max(a,-b), max(-a,b))
which is exact (selection + sign flip only).  All on-device compute is fp16:
every DVE tensor_tensor runs in 2x mode and every tensor_scalar in 4x mode,
and DMA bytes halve.  fp16 rounding keeps rel err ~7e-4 vs the f32 oracle.

The +-10 output clip is applied on the host (the recurrence needs UNclipped
values anyway, so the device never clips).

Engine balance per stage (free dim h=2048 per op); the stage recurrence is
serial in L (every next-stage op needs the FULL previous row), so the per-
stage critical chain is what matters.  Assignment keeps the two cheap-engine
ops (Pool) OFF the chain's tail and the DVE queue ordered so it never
stalls more than ~0.4us:
  DVE : t=Ln+Rn, nt=-t, v2=max(nLp,t), mv=min(v1,v2), ON=mv+Ln,
        u2=max(nLp,t)... order [t, nt, v2, mv, ON, u2, OP=min(u1,u2)]
  Act : nLp=-Lp, then nRp for the NEXT stage (R is prefetched, so -Rp is
        computable one stage early)
  Pool: v1=max(Lp,nRp) (ready at row-start), u1=max(Lp,nt) (needed last)
  SP  : R-row prefetch + output writeback DMAs (double buffered)

Stage 0 pairs adjacent elements (stride 1), which would break the DVE 16-bit
packed fast path; instead L row 1 is deinterleaved on-device into packed
pos/neg halves, right row 0 arrives host-deinterleaved, and out row 0 leaves
as packed halves that the host re-interleaves.

Sharding: pure data-parallel over batch, 512 rows per core on 8 cores.
Layout: batch on partitions (4 groups of 128 coalesced along the free axis
-> [128, 4096] tiles); the butterfly is pure strided access patterns.
"""

import sys

for _p in ("/opt/trn_rl_repo",):
    if _p not in sys.path:
        sys.path.insert(0, _p)

import numpy as np

import bass_rust
import concourse.bass as bass
import concourse.tile as tile
from concourse import bacc, mybir
from concourse.bass_utils import run_bass_kernel_spmd


def _after(inst, *prevs):
    """Order `inst` after `prevs` on the same engine (scheduler edge, no
    semaphore) — pins queue order the greedy list scheduler would flip."""
    names = bass_rust.InstructionNameOrderedSet([p.ins.name for p in prevs])
    inst.ins.add_nosync_dependencies_from(names)

NUM_STAGES = 10
CODE = 1024
B = 4096
N_CORES = 8
P = 128
CLIP = 10.0
F16 = mybir.dt.float16
ALU = mybir.AluOpType
ACTF = mybir.ActivationFunctionType


def _halves(ap, i):
    """pos/neg strided views of a [P, W] row for stage i (i >= 1)."""
    r = 1 << i
    v = ap.rearrange("p (m two r) -> p m two r", two=2, r=r)
    return v[:, :, 0, :], v[:, :, 1, :]


def build(nc, weights, bpc):
    """Emit the per-core kernel. weights: [(w0, w1)] * 10, bpc: batch rows/core."""
    g = bpc // P
    w = g * CODE
    h = w // 2

    right_d = nc.dram_tensor("right", [bpc, NUM_STAGES, CODE], F16,
                             kind="ExternalInput")
    r0_d = nc.dram_tensor("right0", [bpc, CODE], F16, kind="ExternalInput")
    left10_d = nc.dram_tensor("left10", [bpc, CODE], F16, kind="ExternalInput")
    out_d = nc.dram_tensor("out", [bpc, NUM_STAGES, CODE], F16,
                           kind="ExternalOutput")

    def dram_row(dt_, i=None):
        ap = dt_.ap() if i is None else dt_.ap()[:, i, :]
        return ap.rearrange("(g p) c -> p g c", p=P)

    def sb(ap):
        return ap.rearrange("p (g c) -> p g c", g=g)

    with tile.TileContext(nc) as tc:
        with (
            tc.tile_pool(name="lo", bufs=3) as lo_pool,
            tc.tile_pool(name="rin", bufs=3) as r_pool,
            tc.tile_pool(name="tt", bufs=2) as t_pool,
            tc.tile_pool(name="ng", bufs=2) as ng_pool,
            tc.tile_pool(name="uu", bufs=2) as u_pool,
            tc.tile_pool(name="vv", bufs=2) as v_pool,
            tc.tile_pool(name="mv", bufs=2) as m_pool,
            tc.tile_pool(name="pk", bufs=1) as pk_pool,
        ):
            # fill: stage 9 is column-split into two batch-group halves
            # (independent rows), each fed by quarter DMAs in dependency
            # order (neg half first), so compute starts ~3.5us in.
            L = lo_pool.tile([P, w], F16, tag="lo", name="L10")
            R_tiles = {9: r_pool.tile([P, w], F16, tag="rin", name="R9")}
            for s in (0, 1):
                for half in (1, 0):   # neg halves first
                    for dt_, tile_ in ((left10_d, L), (right_d, R_tiles[9])):
                        src = dt_.ap() if dt_ is left10_d else dt_.ap()[:, 9, :]
                        src = src.rearrange(
                            "(g p) (two r) -> p g two r",
                            p=P, two=2, r=CODE // 2)[:, 2 * s:2 * s + 2, half, :]
                        dst = tile_[:].rearrange(
                            "p (g two r) -> p g two r",
                            two=2, r=CODE // 2)[:, 2 * s:2 * s + 2, half, :]
                        nc.sync.dma_start(dst, src)
            for i in (8,):
                R_tiles[i] = r_pool.tile([P, w], F16, tag="rin", name=f"R{i}")
                nc.sync.dma_start(sb(R_tiles[i][:]), dram_row(right_d, i))
            def rp_of(j):
                """pos-half view of stage j's R tile."""
                return _halves(R_tiles[j][:], 9 if j == 0 else j)[0]

            nR_tiles = {}

            # ---- stage 9, column-split into two independent batch-group
            # halves so each starts as soon as its quarter-DMAs land.
            # nRp9 runs on DVE (idle during fill) instead of Act.
            w0, w1 = weights[9]
            R9 = R_tiles[9]
            R_tiles[7] = r_pool.tile([P, w], F16, tag="rin", name="R7")
            nc.sync.dma_start(sb(R_tiles[7][:]), dram_row(right_d, 7))
            Lp_f, Ln_f = _halves(L[:], 9)
            Rp_f, Rn_f = _halves(R9[:], 9)
            h2 = h // 2
            nLp9 = ng_pool.tile([P, h], F16, tag="nLp", name="nLp9")
            nRp9 = ng_pool.tile([P, h], F16, tag="nRp9", name="nRp9")
            t9 = t_pool.tile([P, h], F16, tag="t", name="t9")
            nt9 = t_pool.tile([P, h], F16, tag="nt", name="nt9")
            v19 = v_pool.tile([P, h], F16, tag="v1", name="v1_9")
            v29 = v_pool.tile([P, h], F16, tag="v2", name="v2_9")
            u19 = u_pool.tile([P, h], F16, tag="u1", name="u1_9")
            u29 = u_pool.tile([P, h], F16, tag="u2", name="u2_9")
            mv9 = m_pool.tile([P, h], F16, tag="mv", name="mv9")
            O9 = lo_pool.tile([P, w], F16, tag="lo", name="O9")
            Op_f, On_f = _halves(O9[:], 9)
            nlp_insts = []
            for s in (0, 1):
                msl = slice(2 * s, 2 * s + 2)
                csl = slice(s * h2, (s + 1) * h2)
                Lp, Ln = Lp_f[:, msl, :], Ln_f[:, msl, :]
                Rp, Rn = Rp_f[:, msl, :], Rn_f[:, msl, :]
                nlp_insts.append(nc.scalar.activation(
                    nLp9[:, csl], Lp, ACTF.Copy, bias=0.0, scale=-1.0))
                t_i = nc.vector.tensor_tensor(t9[:, csl], Ln, Rn, ALU.add)
                nc.gpsimd.tensor_scalar(nt9[:, csl], t9[:, csl], -1.0, None,
                                        ALU.mult)
                nr_i = nc.vector.tensor_scalar(nRp9[:, csl], Rp, -1.0, None,
                                               ALU.mult)
                _after(nr_i, t_i)
                v1_i = nc.vector.tensor_tensor(v19[:, csl], Lp, nRp9[:, csl],
                                               ALU.max)
                _after(v1_i, nr_i)
                v2_i = nc.vector.tensor_tensor(v29[:, csl], nLp9[:, csl], Rp,
                                               ALU.max)
                _after(v2_i, v1_i)
                u2_i = nc.vector.tensor_tensor(u29[:, csl], nLp9[:, csl],
                                               t9[:, csl], ALU.max)
                _after(u2_i, v2_i)
                mv_i = nc.vector.tensor_tensor(mv9[:, csl], v19[:, csl],
                                               v29[:, csl], ALU.min)
                _after(mv_i, u2_i)
                if w1 != 1.0:
                    mv_i = nc.vector.tensor_scalar(mv9[:, csl], mv9[:, csl],
                                                   float(w1), None, ALU.mult)
                u1_i = nc.vector.tensor_tensor(u19[:, csl], Lp, nt9[:, csl],
                                               ALU.max)
                _after(u1_i, mv_i)
                nc.gpsimd.tensor_tensor(
                    On_f[:, 2 * s + 1:2 * s + 2, :],
                    mv9[:, s * h2 + h2 // 2:(s + 1) * h2],
                    Ln_f[:, 2 * s + 1:2 * s + 2, :], ALU.add)
                on_i = nc.vector.tensor_tensor(
                    On_f[:, 2 * s:2 * s + 1, :],
                    mv9[:, s * h2:s * h2 + h2 // 2],
                    Ln_f[:, 2 * s:2 * s + 1, :], ALU.add)
                _after(on_i, u1_i)
                op_i = nc.vector.tensor_tensor(Op_f[:, msl, :], u19[:, csl],
                                               u29[:, csl], ALU.min)
                _after(op_i, on_i)
                if w0 != 1.0:
                    nc.vector.tensor_scalar(Op_f[:, msl, :], Op_f[:, msl, :],
                                            float(w0), None, ALU.mult)
                nc.sync.dma_start(
                    dram_row(out_d, 9)[:, msl, :],
                    O9[:, s * (w // 2):(s + 1) * (w // 2)].rearrange(
                        "p (g c) -> p g c", g=2))
            # -Rp for stage 8 on Act while stage 9 finishes
            nR_tiles[8] = ng_pool.tile([P, h], F16, tag="nRp", name="nRp8")
            nr8_i = nc.scalar.activation(nR_tiles[8][:], rp_of(8), ACTF.Copy,
                                         bias=0.0, scale=-1.0)
            _after(nr8_i, nlp_insts[1])
            L = O9

            for i in reversed(range(1, NUM_STAGES - 1)):
                w0, w1 = weights[i]
                R = R_tiles[i]
                # prefetch R two stages ahead (stage 0's row arrives
                # host-deinterleaved in r0_d)
                if i - 2 >= 0:
                    R_tiles[i - 2] = r_pool.tile([P, w], F16, tag="rin",
                                                 name=f"R{i - 2}")
                    src = dram_row(r0_d) if i == 2 else dram_row(right_d, i - 2)
                    nc.sync.dma_start(sb(R_tiles[i - 2][:]), src)

                Lp, Ln = _halves(L[:], i)
                Rp, Rn = _halves(R[:], i)
                nRp = nR_tiles.pop(i)

                # DVE queue [t, v1, v2, u2, mv, u1, ON, OP]: no op directly
                # follows its producer (kills the RAW side-effect gaps), the
                # next stage's t needs only ON (2 slots before OP -> seamless
                # stage handoff), and u1 sits late enough that Pool's nt is
                # always ready.
                # nLp gates v2 (DVE slot 3): 3/4 on Act, 1/4 on the DVE in
                # the slot freed by t's Pool share
                m_i = w >> (i + 1)
                nq = m_i - max(1, m_i // 4)
                cq = nq << i
                nLp = ng_pool.tile([P, h], F16, tag="nLp", name=f"nLp{i}")
                nLp_i = nc.scalar.activation(nLp[:, :cq], Lp[:, :nq, :],
                                             ACTF.Copy, bias=0.0, scale=-1.0)

                # t = Ln + Rn, column-split: Pool is idle before nt, so it
                # takes the tail 3/8 there
                tq = m_i - max(1, (m_i * 3) // 8)
                ct = tq << i
                t = t_pool.tile([P, h], F16, tag="t", name=f"t{i}")
                t_i = nc.vector.tensor_tensor(t[:, :ct], Ln[:, :tq, :],
                                              Rn[:, :tq, :], ALU.add)
                nc.gpsimd.tensor_tensor(t[:, ct:], Ln[:, tq:, :],
                                        Rn[:, tq:, :], ALU.add)
                nlpd_i = nc.vector.tensor_scalar(nLp[:, cq:], Lp[:, nq:, :],
                                                 -1.0, None, ALU.mult)
                _after(nlpd_i, t_i)
                v1 = v_pool.tile([P, h], F16, tag="v1", name=f"v1_{i}")
                v1_i = nc.vector.tensor_tensor(v1[:], Lp, nRp[:], ALU.max)
                _after(v1_i, nlpd_i)
                # nt on Pool (tensor_scalar IS implemented on gpsimd): the
                # u-chain has slack, and it takes 594ns off the DVE
                nt = t_pool.tile([P, h], F16, tag="nt", name=f"nt{i}")
                nc.gpsimd.tensor_scalar(nt[:], t[:], -1.0, None, ALU.mult)

                v2 = v_pool.tile([P, h], F16, tag="v2", name=f"v2_{i}")
                v2_i = nc.vector.tensor_tensor(v2[:], nLp[:], Rp, ALU.max)
                _after(v2_i, v1_i)
                u2 = u_pool.tile([P, h], F16, tag="u2", name=f"u2_{i}")
                u2_i = nc.vector.tensor_tensor(u2[:], nLp[:], t[:], ALU.max)
                _after(u2_i, v2_i)

                O = lo_pool.tile([P, w], F16, tag="lo", name=f"O{i}")
                Op, On = _halves(O[:], i)

                mv = m_pool.tile([P, h], F16, tag="mv", name=f"mv{i}")
                mv_i = nc.vector.tensor_tensor(mv[:], v1[:], v2[:], ALU.min)
                _after(mv_i, u2_i)
                if w1 != 1.0:
                    mv_i = nc.vector.tensor_scalar(mv[:], mv[:], float(w1),
                                                   None, ALU.mult)
                u1 = u_pool.tile([P, h], F16, tag="u1", name=f"u1_{i}")
                u1_i = nc.vector.tensor_tensor(u1[:], Lp, nt[:], ALU.max)
                _after(u1_i, mv_i)
                # ON = mv + Ln, column-split: Pool (idle after nt) takes the
                # tail half right after mv while the DVE finishes u1/OP — the
                # only stage op Pool can legally run (fp16 tensor add).
                mh = max(1, (m_i * 7 + 8) // 16)   # ~7/16 on DVE, rest Pool
                ch = mh * (1 << i)
                nc.gpsimd.tensor_tensor(On[:, mh:, :], mv[:, ch:], Ln[:, mh:, :],
                                        ALU.add)
                on_i = nc.vector.tensor_tensor(On[:, :mh, :], mv[:, :ch],
                                               Ln[:, :mh, :], ALU.add)
                _after(on_i, u1_i)
                op_i = nc.vector.tensor_tensor(Op, u1[:], u2[:], ALU.min)
                _after(op_i, on_i)
                if w0 != 1.0:
                    op_i = nc.vector.tensor_scalar(Op, Op, float(w0), None,
                                                   ALU.mult)

                # -Rp for the NEXT stage while DVE finishes this one (kept
                # behind nLp on Act so it can't steal nLp's slot)
                nR_tiles[i - 1] = ng_pool.tile([P, h], F16, tag="nRp",
                                               name=f"nRp{i - 1}")
                nrp_i = nc.scalar.activation(nR_tiles[i - 1][:],
                                             rp_of(i - 1), ACTF.Copy,
                                             bias=0.0, scale=-1.0)
                _after(nrp_i, nLp_i)

                nc.sync.dma_start(dram_row(out_d, i), sb(O[:]))
                L = O

            # ---- stage 0: adjacent pairs -> deinterleave L row 1 into
            # packed halves (r0 arrives host-deinterleaved), column-split
            # into two batch-group halves so the writeback overlaps the
            # second half's compute.
            w0, w1 = weights[0]
            R0 = R_tiles[0]
            lv = L[:].rearrange("p (m two) -> p m two", two=2)
            Rp_f, Rn_f = _halves(R0[:], 9)
            nRp0 = nR_tiles.pop(0)
            Lp_t = pk_pool.tile([P, h], F16, tag="lpk", name="Lp0")
            Ln_t = pk_pool.tile([P, h], F16, tag="lnk", name="Ln0")
            nLp0 = ng_pool.tile([P, h], F16, tag="nLp", name="nLp0")
            t0 = t_pool.tile([P, h], F16, tag="t", name="t0")
            nt0 = t_pool.tile([P, h], F16, tag="nt", name="nt0")
            v10 = v_pool.tile([P, h], F16, tag="v1", name="v1_0")
            v20 = v_pool.tile([P, h], F16, tag="v2", name="v2_0")
            u10 = u_pool.tile([P, h], F16, tag="u1", name="u1_0")
            u20 = u_pool.tile([P, h], F16, tag="u2", name="u2_0")
            mv0 = m_pool.tile([P, h], F16, tag="mv", name="mv0")
            O0 = lo_pool.tile([P, w], F16, tag="lo", name="O0")
            Op_f, On_f = _halves(O0[:], 9)
            prev_act = None
            for s in (0, 1):
                msl = slice(2 * s, 2 * s + 2)
                csl = slice(s * h2, (s + 1) * h2)
                mm = slice(s * (w // 4), (s + 1) * (w // 4))
                lpc_i = nc.vector.tensor_copy(Lp_t[:, csl], lv[:, mm, 0])
                lnc_i = nc.scalar.activation(Ln_t[:, csl], lv[:, mm, 1],
                                             ACTF.Copy, bias=0.0, scale=1.0)
                nlp_i = nc.scalar.activation(nLp0[:, csl], lv[:, mm, 0],
                                             ACTF.Copy, bias=0.0, scale=-1.0)
                if prev_act is not None:
                    _after(lnc_i, prev_act)
                _after(nlp_i, lnc_i)
                prev_act = nlp_i
                Lp, Ln = Lp_t[:, csl], Ln_t[:, csl]
                Rp, Rn = Rp_f[:, msl, :], Rn_f[:, msl, :]
                t_i = nc.vector.tensor_tensor(t0[:, csl], Ln, Rn, ALU.add)
                _after(t_i, lpc_i)
                nc.gpsimd.tensor_scalar(nt0[:, csl], t0[:, csl], -1.0, None,
                                        ALU.mult)
                v1_i = nc.vector.tensor_tensor(v10[:, csl], Lp, nRp0[:, csl],
                                               ALU.max)
                _after(v1_i, t_i)
                v2_i = nc.vector.tensor_tensor(v20[:, csl], nLp0[:, csl], Rp,
                                               ALU.max)
                _after(v2_i, v1_i)
                u2_i = nc.vector.tensor_tensor(u20[:, csl], nLp0[:, csl],
                                               t0[:, csl], ALU.max)
                _after(u2_i, v2_i)
                mv_i = nc.vector.tensor_tensor(mv0[:, csl], v10[:, csl],
                                               v20[:, csl], ALU.min)
                _after(mv_i, u2_i)
                if w1 != 1.0:
                    mv_i = nc.vector.tensor_scalar(mv0[:, csl], mv0[:, csl],
                                                   float(w1), None, ALU.mult)
                u1_i = nc.vector.tensor_tensor(u10[:, csl], Lp, nt0[:, csl],
                                               ALU.max)
                _after(u1_i, mv_i)
                nc.gpsimd.tensor_tensor(
                    On_f[:, 2 * s + 1:2 * s + 2, :],
                    mv0[:, s * h2 + h2 // 2:(s + 1) * h2],
                    Ln_t[:, s * h2 + h2 // 2:(s + 1) * h2], ALU.add)
                on_i = nc.vector.tensor_tensor(
                    On_f[:, 2 * s:2 * s + 1, :],
                    mv0[:, s * h2:s * h2 + h2 // 2],
                    Ln_t[:, s * h2:s * h2 + h2 // 2], ALU.add)
                _after(on_i, u1_i)
                op_i = nc.vector.tensor_tensor(Op_f[:, msl, :], u10[:, csl],
                                               u20[:, csl], ALU.min)
                _after(op_i, on_i)
                if w0 != 1.0:
                    nc.vector.tensor_scalar(Op_f[:, msl, :], Op_f[:, msl, :],
                                            float(w0), None, ALU.mult)
                # split writeback: each half leaves while the rest computes
                nc.sync.dma_start(dram_row(out_d, 0)[:, msl, h // g:],
                                  On_f[:, msl, :])
                nc.sync.dma_start(dram_row(out_d, 0)[:, msl, :h // g],
                                  Op_f[:, msl, :])


TRACE = False
LAST_RESULTS = None


def _make_nc(weights, bpc):
    nc = bacc.Bacc("TRN2", target_bir_lowering=False, debug=False)
    build(nc, weights, bpc)
    nc.compile()
    return nc


def kernel(right, left, left_weights, iter):
    right = np.asarray(right)
    left = np.asarray(left)
    wsel = np.asarray(left_weights, dtype=np.float32)[int(iter)]  # [10, 2]
    weights = [(float(wsel[i, 0]), float(wsel[i, 1])) for i in range(NUM_STAGES)]

    bpc = B // N_CORES
    nc = _make_nc(weights, bpc)

    right16 = right[:, :NUM_STAGES, :].astype(np.float16)
    left10_16 = left[:, NUM_STAGES, :].astype(np.float16)
    # stage-0 right row, host-deinterleaved into packed pos|neg halves
    r0 = np.empty((B, CODE), np.float16)
    r0[:, :CODE // 2] = right16[:, 0, 0::2]
    r0[:, CODE // 2:] = right16[:, 0, 1::2]

    in_maps = []
    for c in range(N_CORES):
        sl = slice(c * bpc, (c + 1) * bpc)
        in_maps.append({
            "right": np.ascontiguousarray(right16[sl]),
            "right0": np.ascontiguousarray(r0[sl]),
            "left10": np.ascontiguousarray(left10_16[sl]),
        })
    global LAST_RESULTS
    LAST_RESULTS = run_bass_kernel_spmd(
        nc, in_maps, list(range(N_CORES)), trace=TRACE)
    res = LAST_RESULTS.results

    out16 = np.empty((B, NUM_STAGES, CODE), np.float16)
    for c in range(N_CORES):
        out16[c * bpc:(c + 1) * bpc] = res[c]["out"]
    # row 0 left the device as packed pos|neg halves; re-interleave
    row0 = out16[:, 0, :].copy()
    out16[:, 0, 0::2] = row0[:, :CODE // 2]
    out16[:, 0, 1::2] = row0[:, CODE // 2:]

    out = np.empty((B, NUM_STAGES + 1, CODE), np.float32)
    out[:, :NUM_STAGES, :] = np.clip(out16.astype(np.float32), -CLIP, CLIP)
    out[:, NUM_STAGES, :] = np.clip(left[:, NUM_STAGES, :], -CLIP, CLIP)
    return out


# revision 45
# speedup vs baseline: 2.1588x; 1.0055x over previous
"""Trainium2 Bass kernel: polar-BP left-message butterfly (nn_IterateLeftLayer).

Math per stage i (9..0), with L = left row i+1 (unclipped), R = right row i:
  out[pos] = w0 * ms(L[pos], L[neg] + R[neg])
  out[neg] = w1 * ms(L[pos], R[pos]) + L[neg]
where ms(x,y) = sign(x)sign(y)min(|x|,|y|), pos = {c: bit i of c == 0},
neg = pos + 2^i.  Final output = clip(left, +-10) with rows 0..9 replaced.

ms is computed in pure min/max form (no bit tricks):
  ms(a,b) = min(max(a,-b), max(-a,b))
which is exact (selection + sign flip only).  All on-device compute is fp16:
every DVE tensor_tensor runs in 2x mode and every tensor_scalar in 4x mode,
and DMA bytes halve.  fp16 rounding keeps rel err ~7e-4 vs the f32 oracle.

The +-10 output clip is applied on the host (the recurrence needs UNclipped
values anyway, so the device never clips).

Engine balance per stage (free dim h=2048 per op); the stage recurrence is
serial in L (every next-stage op needs the FULL previous row), so the per-
stage critical chain is what matters.  Assignment keeps the two cheap-engine
ops (Pool) OFF the chain's tail and the DVE queue ordered so it never
stalls more than ~0.4us:
  DVE : t=Ln+Rn, nt=-t, v2=max(nLp,t), mv=min(v1,v2), ON=mv+Ln,
        u2=max(nLp,t)... order [t, nt, v2, mv, ON, u2, OP=min(u1,u2)]
  Act : nLp=-Lp, then nRp for the NEXT stage (R is prefetched, so -Rp is
        computable one stage early)
  Pool: v1=max(Lp,nRp) (ready at row-start), u1=max(Lp,nt) (needed last)
  SP  : R-row prefetch + output writeback DMAs (double buffered)

Stage 0 pairs adjacent elements (stride 1), which would break the DVE 16-bit
packed fast path; instead L row 1 is deinterleaved on-device into packed
pos/neg halves, right row 0 arrives host-deinterleaved, and out row 0 leaves
as packed halves that the host re-interleaves.

Sharding: pure data-parallel over batch, 512 rows per core on 8 cores.
Layout: batch on partitions (4 groups of 128 coalesced along the free axis
-> [128, 4096] tiles); the butterfly is pure strided access patterns.
"""

import sys

for _p in ("/opt/trn_rl_repo",):
    if _p not in sys.path:
        sys.path.insert(0, _p)

import numpy as np

import bass_rust
import concourse.bass as bass
import concourse.tile as tile
from concourse import bacc, mybir
from concourse.bass_utils import run_bass_kernel_spmd


def _after(inst, *prevs):
    """Order `inst` after `prevs` on the same engine (scheduler edge, no
    semaphore) — pins queue order the greedy list scheduler would flip."""
    names = bass_rust.InstructionNameOrderedSet([p.ins.name for p in prevs])
    inst.ins.add_nosync_dependencies_from(names)

NUM_STAGES = 10
CODE = 1024
B = 4096
N_CORES = 8
P = 128
CLIP = 10.0
F16 = mybir.dt.float16
ALU = mybir.AluOpType
ACTF = mybir.ActivationFunctionType


def _halves(ap, i):
    """pos/neg strided views of a [P, W] row for stage i (i >= 1)."""
    r = 1 << i
    v = ap.rearrange("p (m two r) -> p m two r", two=2, r=r)
    return v[:, :, 0, :], v[:, :, 1, :]


def build(nc, weights, bpc):
    """Emit the per-core kernel. weights: [(w0, w1)] * 10, bpc: batch rows/core."""
    g = bpc // P
    w = g * CODE
    h = w // 2

    right_d = nc.dram_tensor("right", [bpc, NUM_STAGES, CODE], F16,
                             kind="ExternalInput")
    r0_d = nc.dram_tensor("right0", [bpc, CODE], F16, kind="ExternalInput")
    left10_d = nc.dram_tensor("left10", [bpc, CODE], F16, kind="ExternalInput")
    out_d = nc.dram_tensor("out", [bpc, NUM_STAGES, CODE], F16,
                           kind="ExternalOutput")

    def dram_row(dt_, i=None):
        ap = dt_.ap() if i is None else dt_.ap()[:, i, :]
        return ap.rearrange("(g p) c -> p g c", p=P)

    def sb(ap):
        return ap.rearrange("p (g c) -> p g c", g=g)

    with tile.TileContext(nc) as tc:
        with (
            tc.tile_pool(name="lo", bufs=3) as lo_pool,
            tc.tile_pool(name="rin", bufs=3) as r_pool,
            tc.tile_pool(name="tt", bufs=2) as t_pool,
            tc.tile_pool(name="ng", bufs=2) as ng_pool,
            tc.tile_pool(name="uu", bufs=2) as u_pool,
            tc.tile_pool(name="vv", bufs=2) as v_pool,
            tc.tile_pool(name="mv", bufs=2) as m_pool,
            tc.tile_pool(name="pk", bufs=1) as pk_pool,
        ):
            # fill: stage 9 is column-split into two batch-group halves
            # (independent rows), each fed by quarter DMAs in dependency
            # order (neg half first), so compute starts ~3.5us in.
            L = lo_pool.tile([P, w], F16, tag="lo", name="L10")
            R_tiles = {9: r_pool.tile([P, w], F16, tag="rin", name="R9")}
            for s in (0, 1):
                for half in (1, 0):   # neg halves first
                    for dt_, tile_ in ((left10_d, L), (right_d, R_tiles[9])):
                        src = dt_.ap() if dt_ is left10_d else dt_.ap()[:, 9, :]
                        src = src.rearrange(
                            "(g p) (two r) -> p g two r",
                            p=P, two=2, r=CODE // 2)[:, 2 * s:2 * s + 2, half, :]
                        dst = tile_[:].rearrange(
                            "p (g two r) -> p g two r",
                            two=2, r=CODE // 2)[:, 2 * s:2 * s + 2, half, :]
                        nc.sync.dma_start(dst, src)
            for i in (8,):
                R_tiles[i] = r_pool.tile([P, w], F16, tag="rin", name=f"R{i}")
                nc.sync.dma_start(sb(R_tiles[i][:]), dram_row(right_d, i))
            def rp_of(j):
                """pos-half view of stage j's R tile."""
                return _halves(R_tiles[j][:], 9 if j == 0 else j)[0]

            nR_tiles = {}

            # ---- stage 9, column-split into two independent batch-group
            # halves so each starts as soon as its quarter-DMAs land.
            # nRp9 runs on DVE (idle during fill) instead of Act.
            w0, w1 = weights[9]
            R9 = R_tiles[9]
            R_tiles[7] = r_pool.tile([P, w], F16, tag="rin", name="R7")
            nc.sync.dma_start(sb(R_tiles[7][:]), dram_row(right_d, 7))
            Lp_f, Ln_f = _halves(L[:], 9)
            Rp_f, Rn_f = _halves(R9[:], 9)
            h2 = h // 2
            nLp9 = ng_pool.tile([P, h], F16, tag="nLp", name="nLp9")
            nRp9 = ng_pool.tile([P, h], F16, tag="nRp9", name="nRp9")
            t9 = t_pool.tile([P, h], F16, tag="t", name="t9")
            nt9 = t_pool.tile([P, h], F16, tag="nt", name="nt9")
            v19 = v_pool.tile([P, h], F16, tag="v1", name="v1_9")
            v29 = v_pool.tile([P, h], F16, tag="v2", name="v2_9")
            u19 = u_pool.tile([P, h], F16, tag="u1", name="u1_9")
            u29 = u_pool.tile([P, h], F16, tag="u2", name="u2_9")
            mv9 = m_pool.tile([P, h], F16, tag="mv", name="mv9")
            O9 = lo_pool.tile([P, w], F16, tag="lo", name="O9")
            Op_f, On_f = _halves(O9[:], 9)
            nlp_insts = []
            for s in (0, 1):
                msl = slice(2 * s, 2 * s + 2)
                csl = slice(s * h2, (s + 1) * h2)
                Lp, Ln = Lp_f[:, msl, :], Ln_f[:, msl, :]
                Rp, Rn = Rp_f[:, msl, :], Rn_f[:, msl, :]
                nlp_insts.append(nc.scalar.activation(
                    nLp9[:, csl], Lp, ACTF.Copy, bias=0.0, scale=-1.0))
                t_i = nc.vector.tensor_tensor(t9[:, csl], Ln, Rn, ALU.add)
                nc.gpsimd.tensor_scalar(nt9[:, csl], t9[:, csl], -1.0, None,
                                        ALU.mult)
                nr_i = nc.vector.tensor_scalar(nRp9[:, csl], Rp, -1.0, None,
                                               ALU.mult)
                _after(nr_i, t_i)
                v1_i = nc.vector.tensor_tensor(v19[:, csl], Lp, nRp9[:, csl],
                                               ALU.max)
                _after(v1_i, nr_i)
                v2_i = nc.vector.tensor_tensor(v29[:, csl], nLp9[:, csl], Rp,
                                               ALU.max)
                _after(v2_i, v1_i)
                u2_i = nc.vector.tensor_tensor(u29[:, csl], nLp9[:, csl],
                                               t9[:, csl], ALU.max)
                _after(u2_i, v2_i)
                mv_i = nc.vector.tensor_tensor(mv9[:, csl], v19[:, csl],
                                               v29[:, csl], ALU.min)
                _after(mv_i, u2_i)
                if w1 != 1.0:
                    mv_i = nc.vector.tensor_scalar(mv9[:, csl], mv9[:, csl],
                                                   float(w1), None, ALU.mult)
                u1_i = nc.vector.tensor_tensor(u19[:, csl], Lp, nt9[:, csl],
                                               ALU.max)
                _after(u1_i, mv_i)
                nc.gpsimd.tensor_tensor(
                    On_f[:, 2 * s + 1:2 * s + 2, :],
                    mv9[:, s * h2 + h2 // 2:(s + 1) * h2],
                    Ln_f[:, 2 * s + 1:2 * s + 2, :], ALU.add)
                on_i = nc.vector.tensor_tensor(
                    On_f[:, 2 * s:2 * s + 1, :],
                    mv9[:, s * h2:s * h2 + h2 // 2],
                    Ln_f[:, 2 * s:2 * s + 1, :], ALU.add)
                _after(on_i, u1_i)
                op_i = nc.vector.tensor_tensor(Op_f[:, msl, :], u19[:, csl],
                                               u29[:, csl], ALU.min)
                _after(op_i, on_i)
                if w0 != 1.0:
                    nc.vector.tensor_scalar(Op_f[:, msl, :], Op_f[:, msl, :],
                                            float(w0), None, ALU.mult)
                nc.sync.dma_start(
                    dram_row(out_d, 9)[:, msl, :],
                    O9[:, s * (w // 2):(s + 1) * (w // 2)].rearrange(
                        "p (g c) -> p g c", g=2))
            # -Rp for stage 8 on Act while stage 9 finishes
            nR_tiles[8] = ng_pool.tile([P, h], F16, tag="nRp", name="nRp8")
            nr8_i = nc.scalar.activation(nR_tiles[8][:], rp_of(8), ACTF.Copy,
                                         bias=0.0, scale=-1.0)
            _after(nr8_i, nlp_insts[1])
            L = O9

            for i in reversed(range(1, NUM_STAGES - 1)):
                w0, w1 = weights[i]
                R = R_tiles[i]
                # prefetch R two stages ahead (stage 0's row arrives
                # host-deinterleaved in r0_d)
                if i - 2 >= 0:
                    R_tiles[i - 2] = r_pool.tile([P, w], F16, tag="rin",
                                                 name=f"R{i - 2}")
                    src = dram_row(r0_d) if i == 2 else dram_row(right_d, i - 2)
                    nc.sync.dma_start(sb(R_tiles[i - 2][:]), src)

                Lp, Ln = _halves(L[:], i)
                Rp, Rn = _halves(R[:], i)
                nRp = nR_tiles.pop(i)

                # DVE queue [t, v1, v2, u2, mv, u1, ON, OP]: no op directly
                # follows its producer (kills the RAW side-effect gaps), the
                # next stage's t needs only ON (2 slots before OP -> seamless
                # stage handoff), and u1 sits late enough that Pool's nt is
                # always ready.
                # nLp gates v2 (DVE slot 3): 3/4 on Act, 1/4 on the DVE in
                # the slot freed by t's Pool share
                m_i = w >> (i + 1)
                nq = m_i - max(1, m_i // 8)
                cq = nq << i
                nLp = ng_pool.tile([P, h], F16, tag="nLp", name=f"nLp{i}")
                nLp_i = nc.scalar.activation(nLp[:, :cq], Lp[:, :nq, :],
                                             ACTF.Copy, bias=0.0, scale=-1.0)

                # t = Ln + Rn, column-split: Pool is idle before nt, so it
                # takes the tail 3/8 there
                tq = m_i - max(1, (m_i * 3) // 8)
                ct = tq << i
                t = t_pool.tile([P, h], F16, tag="t", name=f"t{i}")
                t_i = nc.vector.tensor_tensor(t[:, :ct], Ln[:, :tq, :],
                                              Rn[:, :tq, :], ALU.add)
                nc.gpsimd.tensor_tensor(t[:, ct:], Ln[:, tq:, :],
                                        Rn[:, tq:, :], ALU.add)
                nlpd_i = nc.vector.tensor_scalar(nLp[:, cq:], Lp[:, nq:, :],
                                                 -1.0, None, ALU.mult)
                _after(nlpd_i, t_i)
                v1 = v_pool.tile([P, h], F16, tag="v1", name=f"v1_{i}")
                v1_i = nc.vector.tensor_tensor(v1[:], Lp, nRp[:], ALU.max)
                _after(v1_i, nlpd_i)
                # nt on Pool (tensor_scalar IS implemented on gpsimd): the
                # u-chain has slack, and it takes 594ns off the DVE
                nt = t_pool.tile([P, h], F16, tag="nt", name=f"nt{i}")
                nc.gpsimd.tensor_scalar(nt[:], t[:], -1.0, None, ALU.mult)

                v2 = v_pool.tile([P, h], F16, tag="v2", name=f"v2_{i}")
                v2_i = nc.vector.tensor_tensor(v2[:], nLp[:], Rp, ALU.max)
                _after(v2_i, v1_i)
                u2 = u_pool.tile([P, h], F16, tag="u2", name=f"u2_{i}")
                u2_i = nc.vector.tensor_tensor(u2[:], nLp[:], t[:], ALU.max)
                _after(u2_i, v2_i)

                O = lo_pool.tile([P, w], F16, tag="lo", name=f"O{i}")
                Op, On = _halves(O[:], i)

                mv = m_pool.tile([P, h], F16, tag="mv", name=f"mv{i}")
                mv_i = nc.vector.tensor_tensor(mv[:], v1[:], v2[:], ALU.min)
                _after(mv_i, u2_i)
                if w1 != 1.0:
                    mv_i = nc.vector.tensor_scalar(mv[:], mv[:], float(w1),
                                                   None, ALU.mult)
                u1 = u_pool.tile([P, h], F16, tag="u1", name=f"u1_{i}")
                u1_i = nc.vector.tensor_tensor(u1[:], Lp, nt[:], ALU.max)
                _after(u1_i, mv_i)
                # ON = mv + Ln, column-split: Pool (idle after nt) takes the
                # tail half right after mv while the DVE finishes u1/OP — the
                # only stage op Pool can legally run (fp16 tensor add).
                mh = max(1, (m_i * 7 + 8) // 16)   # ~7/16 on DVE, rest Pool
                ch = mh * (1 << i)
                nc.gpsimd.tensor_tensor(On[:, mh:, :], mv[:, ch:], Ln[:, mh:, :],
                                        ALU.add)
                on_i = nc.vector.tensor_tensor(On[:, :mh, :], mv[:, :ch],
                                               Ln[:, :mh, :], ALU.add)
                _after(on_i, u1_i)
                op_i = nc.vector.tensor_tensor(Op, u1[:], u2[:], ALU.min)
                _after(op_i, on_i)
                if w0 != 1.0:
                    op_i = nc.vector.tensor_scalar(Op, Op, float(w0), None,
                                                   ALU.mult)

                # -Rp for the NEXT stage while DVE finishes this one (kept
                # behind nLp on Act so it can't steal nLp's slot)
                nR_tiles[i - 1] = ng_pool.tile([P, h], F16, tag="nRp",
                                               name=f"nRp{i - 1}")
                nrp_i = nc.scalar.activation(nR_tiles[i - 1][:],
                                             rp_of(i - 1), ACTF.Copy,
                                             bias=0.0, scale=-1.0)
                _after(nrp_i, nLp_i)

                nc.sync.dma_start(dram_row(out_d, i), sb(O[:]))
                L = O

            # ---- stage 0: adjacent pairs -> deinterleave L row 1 into
            # packed halves (r0 arrives host-deinterleaved), column-split
            # into two batch-group halves so the writeback overlaps the
            # second half's compute.
            w0, w1 = weights[0]
            R0 = R_tiles[0]
            lv = L[:].rearrange("p (m two) -> p m two", two=2)
            Rp_f, Rn_f = _halves(R0[:], 9)
            nRp0 = nR_tiles.pop(0)
            Lp_t = pk_pool.tile([P, h], F16, tag="lpk", name="Lp0")
            Ln_t = pk_pool.tile([P, h], F16, tag="lnk", name="Ln0")
            nLp0 = ng_pool.tile([P, h], F16, tag="nLp", name="nLp0")
            t0 = t_pool.tile([P, h], F16, tag="t", name="t0")
            nt0 = t_pool.tile([P, h], F16, tag="nt", name="nt0")
            v10 = v_pool.tile([P, h], F16, tag="v1", name="v1_0")
            v20 = v_pool.tile([P, h], F16, tag="v2", name="v2_0")
            u10 = u_pool.tile([P, h], F16, tag="u1", name="u1_0")
            u20 = u_pool.tile([P, h], F16, tag="u2", name="u2_0")
            mv0 = m_pool.tile([P, h], F16, tag="mv", name="mv0")
            O0 = lo_pool.tile([P, w], F16, tag="lo", name="O0")
            Op_f, On_f = _halves(O0[:], 9)
            prev_act = None
            for s in (0, 1):
                msl = slice(2 * s, 2 * s + 2)
                csl = slice(s * h2, (s + 1) * h2)
                mm = slice(s * (w // 4), (s + 1) * (w // 4))
                lpc_i = nc.vector.tensor_copy(Lp_t[:, csl], lv[:, mm, 0])
                lnc_i = nc.scalar.activation(Ln_t[:, csl], lv[:, mm, 1],
                                             ACTF.Copy, bias=0.0, scale=1.0)
                nlp_i = nc.scalar.activation(nLp0[:, csl], lv[:, mm, 0],
                                             ACTF.Copy, bias=0.0, scale=-1.0)
                if prev_act is not None:
                    _after(lnc_i, prev_act)
                _after(nlp_i, lnc_i)
                prev_act = nlp_i
                Lp, Ln = Lp_t[:, csl], Ln_t[:, csl]
                Rp, Rn = Rp_f[:, msl, :], Rn_f[:, msl, :]
                t_i = nc.vector.tensor_tensor(t0[:, csl], Ln, Rn, ALU.add)
                _after(t_i, lpc_i)
                nc.gpsimd.tensor_scalar(nt0[:, csl], t0[:, csl], -1.0, None,
                                        ALU.mult)
                v1_i = nc.vector.tensor_tensor(v10[:, csl], Lp, nRp0[:, csl],
                                               ALU.max)
                _after(v1_i, t_i)
                v2_i = nc.vector.tensor_tensor(v20[:, csl], nLp0[:, csl], Rp,
                                               ALU.max)
                _after(v2_i, v1_i)
                u2_i = nc.vector.tensor_tensor(u20[:, csl], nLp0[:, csl],
                                               t0[:, csl], ALU.max)
                _after(u2_i, v2_i)
                mv_i = nc.vector.tensor_tensor(mv0[:, csl], v10[:, csl],
                                               v20[:, csl], ALU.min)
                _after(mv_i, u2_i)
                if w1 != 1.0:
                    mv_i = nc.vector.tensor_scalar(mv0[:, csl], mv0[:, csl],
                                                   float(w1), None, ALU.mult)
                u1_i = nc.vector.tensor_tensor(u10[:, csl], Lp, nt0[:, csl],
                                               ALU.max)
                _after(u1_i, mv_i)
                nc.gpsimd.tensor_tensor(
                    On_f[:, 2 * s + 1:2 * s + 2, :],
                    mv0[:, s * h2 + h2 // 2:(s + 1) * h2],
                    Ln_t[:, s * h2 + h2 // 2:(s + 1) * h2], ALU.add)
                on_i = nc.vector.tensor_tensor(
                    On_f[:, 2 * s:2 * s + 1, :],
                    mv0[:, s * h2:s * h2 + h2 // 2],
                    Ln_t[:, s * h2:s * h2 + h2 // 2], ALU.add)
                _after(on_i, u1_i)
                op_i = nc.vector.tensor_tensor(Op_f[:, msl, :], u10[:, csl],
                                               u20[:, csl], ALU.min)
                _after(op_i, on_i)
                if w0 != 1.0:
                    nc.vector.tensor_scalar(Op_f[:, msl, :], Op_f[:, msl, :],
                                            float(w0), None, ALU.mult)
                # split writeback: each half leaves while the rest computes
                nc.sync.dma_start(dram_row(out_d, 0)[:, msl, h // g:],
                                  On_f[:, msl, :])
                nc.sync.dma_start(dram_row(out_d, 0)[:, msl, :h // g],
                                  Op_f[:, msl, :])


TRACE = False
LAST_RESULTS = None


def _make_nc(weights, bpc):
    nc = bacc.Bacc("TRN2", target_bir_lowering=False, debug=False)
    build(nc, weights, bpc)
    nc.compile()
    return nc


def kernel(right, left, left_weights, iter):
    right = np.asarray(right)
    left = np.asarray(left)
    wsel = np.asarray(left_weights, dtype=np.float32)[int(iter)]  # [10, 2]
    weights = [(float(wsel[i, 0]), float(wsel[i, 1])) for i in range(NUM_STAGES)]

    bpc = B // N_CORES
    nc = _make_nc(weights, bpc)

    right16 = right[:, :NUM_STAGES, :].astype(np.float16)
    left10_16 = left[:, NUM_STAGES, :].astype(np.float16)
    # stage-0 right row, host-deinterleaved into packed pos|neg halves
    r0 = np.empty((B, CODE), np.float16)
    r0[:, :CODE // 2] = right16[:, 0, 0::2]
    r0[:, CODE // 2:] = right16[:, 0, 1::2]

    in_maps = []
    for c in range(N_CORES):
        sl = slice(c * bpc, (c + 1) * bpc)
        in_maps.append({
            "right": np.ascontiguousarray(right16[sl]),
            "right0": np.ascontiguousarray(r0[sl]),
            "left10": np.ascontiguousarray(left10_16[sl]),
        })
    global LAST_RESULTS
    LAST_RESULTS = run_bass_kernel_spmd(
        nc, in_maps, list(range(N_CORES)), trace=TRACE)
    res = LAST_RESULTS.results

    out16 = np.empty((B, NUM_STAGES, CODE), np.float16)
    for c in range(N_CORES):
        out16[c * bpc:(c + 1) * bpc] = res[c]["out"]
    # row 0 left the device as packed pos|neg halves; re-interleave
    row0 = out16[:, 0, :].copy()
    out16[:, 0, 0::2] = row0[:, :CODE // 2]
    out16[:, 0, 1::2] = row0[:, CODE // 2:]

    out = np.empty((B, NUM_STAGES + 1, CODE), np.float32)
    out[:, :NUM_STAGES, :] = np.clip(out16.astype(np.float32), -CLIP, CLIP)
    out[:, NUM_STAGES, :] = np.clip(left[:, NUM_STAGES, :], -CLIP, CLIP)
    return out


# revision 46
# speedup vs baseline: 2.1768x; 1.0083x over previous
"""Trainium2 Bass kernel: polar-BP left-message butterfly (nn_IterateLeftLayer).

Math per stage i (9..0), with L = left row i+1 (unclipped), R = right row i:
  out[pos] = w0 * ms(L[pos], L[neg] + R[neg])
  out[neg] = w1 * ms(L[pos], R[pos]) + L[neg]
where ms(x,y) = sign(x)sign(y)min(|x|,|y|), pos = {c: bit i of c == 0},
neg = pos + 2^i.  Final output = clip(left, +-10) with rows 0..9 replaced.

ms is computed in pure min/max form (no bit tricks):
  ms(a,b) = min(max(a,-b), max(-a,b))
which is exact (selection + sign flip only).  All on-device compute is fp16:
every DVE tensor_tensor runs in 2x mode and every tensor_scalar in 4x mode,
and DMA bytes halve.  fp16 rounding keeps rel err ~7e-4 vs the f32 oracle.

The +-10 output clip is applied on the host (the recurrence needs UNclipped
values anyway, so the device never clips).

Engine balance per stage (free dim h=2048 per op); the stage recurrence is
serial in L (every next-stage op needs the FULL previous row), so the per-
stage critical chain is what matters.  Assignment keeps the two cheap-engine
ops (Pool) OFF the chain's tail and the DVE queue ordered so it never
stalls more than ~0.4us:
  DVE : t=Ln+Rn, nt=-t, v2=max(nLp,t), mv=min(v1,v2), ON=mv+Ln,
        u2=max(nLp,t)... order [t, nt, v2, mv, ON, u2, OP=min(u1,u2)]
  Act : nLp=-Lp, then nRp for the NEXT stage (R is prefetched, so -Rp is
        computable one stage early)
  Pool: v1=max(Lp,nRp) (ready at row-start), u1=max(Lp,nt) (needed last)
  SP  : R-row prefetch + output writeback DMAs (double buffered)

Stage 0 pairs adjacent elements (stride 1), which would break the DVE 16-bit
packed fast path; instead L row 1 is deinterleaved on-device into packed
pos/neg halves, right row 0 arrives host-deinterleaved, and out row 0 leaves
as packed halves that the host re-interleaves.

Sharding: pure data-parallel over batch, 512 rows per core on 8 cores.
Layout: batch on partitions (4 groups of 128 coalesced along the free axis
-> [128, 4096] tiles); the butterfly is pure strided access patterns.
"""

import sys

for _p in ("/opt/trn_rl_repo",):
    if _p not in sys.path:
        sys.path.insert(0, _p)

import numpy as np

import bass_rust
import concourse.bass as bass
import concourse.tile as tile
from concourse import bacc, mybir
from concourse.bass_utils import run_bass_kernel_spmd


def _after(inst, *prevs):
    """Order `inst` after `prevs` on the same engine (scheduler edge, no
    semaphore) — pins queue order the greedy list scheduler would flip."""
    names = bass_rust.InstructionNameOrderedSet([p.ins.name for p in prevs])
    inst.ins.add_nosync_dependencies_from(names)

NUM_STAGES = 10
CODE = 1024
B = 4096
N_CORES = 8
P = 128
CLIP = 10.0
F16 = mybir.dt.float16
ALU = mybir.AluOpType
ACTF = mybir.ActivationFunctionType


def _halves(ap, i):
    """pos/neg strided views of a [P, W] row for stage i (i >= 1)."""
    r = 1 << i
    v = ap.rearrange("p (m two r) -> p m two r", two=2, r=r)
    return v[:, :, 0, :], v[:, :, 1, :]


def build(nc, weights, bpc):
    """Emit the per-core kernel. weights: [(w0, w1)] * 10, bpc: batch rows/core."""
    g = bpc // P
    w = g * CODE
    h = w // 2

    right_d = nc.dram_tensor("right", [bpc, NUM_STAGES, CODE], F16,
                             kind="ExternalInput")
    r0_d = nc.dram_tensor("right0", [bpc, CODE], F16, kind="ExternalInput")
    left10_d = nc.dram_tensor("left10", [bpc, CODE], F16, kind="ExternalInput")
    out_d = nc.dram_tensor("out", [bpc, NUM_STAGES, CODE], F16,
                           kind="ExternalOutput")

    def dram_row(dt_, i=None):
        ap = dt_.ap() if i is None else dt_.ap()[:, i, :]
        return ap.rearrange("(g p) c -> p g c", p=P)

    def sb(ap):
        return ap.rearrange("p (g c) -> p g c", g=g)

    with tile.TileContext(nc) as tc:
        with (
            tc.tile_pool(name="lo", bufs=3) as lo_pool,
            tc.tile_pool(name="rin", bufs=3) as r_pool,
            tc.tile_pool(name="tt", bufs=2) as t_pool,
            tc.tile_pool(name="ng", bufs=2) as ng_pool,
            tc.tile_pool(name="uu", bufs=2) as u_pool,
            tc.tile_pool(name="vv", bufs=2) as v_pool,
            tc.tile_pool(name="mv", bufs=2) as m_pool,
            tc.tile_pool(name="pk", bufs=1) as pk_pool,
        ):
            # fill: stage 9 is column-split into two batch-group halves
            # (independent rows), each fed by quarter DMAs in dependency
            # order (neg half first), so compute starts ~3.5us in.
            L = lo_pool.tile([P, w], F16, tag="lo", name="L10")
            R_tiles = {9: r_pool.tile([P, w], F16, tag="rin", name="R9")}
            for s in (0, 1):
                for half in (1, 0):   # neg halves first
                    for dt_, tile_ in ((left10_d, L), (right_d, R_tiles[9])):
                        src = dt_.ap() if dt_ is left10_d else dt_.ap()[:, 9, :]
                        src = src.rearrange(
                            "(g p) (two r) -> p g two r",
                            p=P, two=2, r=CODE // 2)[:, 2 * s:2 * s + 2, half, :]
                        dst = tile_[:].rearrange(
                            "p (g two r) -> p g two r",
                            two=2, r=CODE // 2)[:, 2 * s:2 * s + 2, half, :]
                        nc.sync.dma_start(dst, src)
            for i in (8,):
                R_tiles[i] = r_pool.tile([P, w], F16, tag="rin", name=f"R{i}")
                nc.sync.dma_start(sb(R_tiles[i][:]), dram_row(right_d, i))
            def rp_of(j):
                """pos-half view of stage j's R tile."""
                return _halves(R_tiles[j][:], 9 if j == 0 else j)[0]

            nR_tiles = {}

            # ---- stage 9, column-split into two independent batch-group
            # halves so each starts as soon as its quarter-DMAs land.
            # nRp9 runs on DVE (idle during fill) instead of Act.
            w0, w1 = weights[9]
            R9 = R_tiles[9]
            R_tiles[7] = r_pool.tile([P, w], F16, tag="rin", name="R7")
            nc.sync.dma_start(sb(R_tiles[7][:]), dram_row(right_d, 7))
            Lp_f, Ln_f = _halves(L[:], 9)
            Rp_f, Rn_f = _halves(R9[:], 9)
            h2 = h // 2
            nLp9 = ng_pool.tile([P, h], F16, tag="nLp", name="nLp9")
            nRp9 = ng_pool.tile([P, h], F16, tag="nRp9", name="nRp9")
            t9 = t_pool.tile([P, h], F16, tag="t", name="t9")
            nt9 = t_pool.tile([P, h], F16, tag="nt", name="nt9")
            v19 = v_pool.tile([P, h], F16, tag="v1", name="v1_9")
            v29 = v_pool.tile([P, h], F16, tag="v2", name="v2_9")
            u19 = u_pool.tile([P, h], F16, tag="u1", name="u1_9")
            u29 = u_pool.tile([P, h], F16, tag="u2", name="u2_9")
            mv9 = m_pool.tile([P, h], F16, tag="mv", name="mv9")
            O9 = lo_pool.tile([P, w], F16, tag="lo", name="O9")
            Op_f, On_f = _halves(O9[:], 9)
            nlp_insts = []
            for s in (0, 1):
                msl = slice(2 * s, 2 * s + 2)
                csl = slice(s * h2, (s + 1) * h2)
                Lp, Ln = Lp_f[:, msl, :], Ln_f[:, msl, :]
                Rp, Rn = Rp_f[:, msl, :], Rn_f[:, msl, :]
                nlp_insts.append(nc.scalar.activation(
                    nLp9[:, csl], Lp, ACTF.Copy, bias=0.0, scale=-1.0))
                t_i = nc.vector.tensor_tensor(t9[:, csl], Ln, Rn, ALU.add)
                nc.gpsimd.tensor_scalar(nt9[:, csl], t9[:, csl], -1.0, None,
                                        ALU.mult)
                nr_i = nc.vector.tensor_scalar(nRp9[:, csl], Rp, -1.0, None,
                                               ALU.mult)
                _after(nr_i, t_i)
                v1_i = nc.vector.tensor_tensor(v19[:, csl], Lp, nRp9[:, csl],
                                               ALU.max)
                _after(v1_i, nr_i)
                v2_i = nc.vector.tensor_tensor(v29[:, csl], nLp9[:, csl], Rp,
                                               ALU.max)
                _after(v2_i, v1_i)
                u2_i = nc.vector.tensor_tensor(u29[:, csl], nLp9[:, csl],
                                               t9[:, csl], ALU.max)
                _after(u2_i, v2_i)
                mv_i = nc.vector.tensor_tensor(mv9[:, csl], v19[:, csl],
                                               v29[:, csl], ALU.min)
                _after(mv_i, u2_i)
                if w1 != 1.0:
                    mv_i = nc.vector.tensor_scalar(mv9[:, csl], mv9[:, csl],
                                                   float(w1), None, ALU.mult)
                u1_i = nc.vector.tensor_tensor(u19[:, csl], Lp, nt9[:, csl],
                                               ALU.max)
                _after(u1_i, mv_i)
                nc.gpsimd.tensor_tensor(
                    On_f[:, 2 * s + 1:2 * s + 2, :],
                    mv9[:, s * h2 + h2 // 2:(s + 1) * h2],
                    Ln_f[:, 2 * s + 1:2 * s + 2, :], ALU.add)
                on_i = nc.vector.tensor_tensor(
                    On_f[:, 2 * s:2 * s + 1, :],
                    mv9[:, s * h2:s * h2 + h2 // 2],
                    Ln_f[:, 2 * s:2 * s + 1, :], ALU.add)
                _after(on_i, u1_i)
                op_i = nc.vector.tensor_tensor(Op_f[:, msl, :], u19[:, csl],
                                               u29[:, csl], ALU.min)
                _after(op_i, on_i)
                if w0 != 1.0:
                    nc.vector.tensor_scalar(Op_f[:, msl, :], Op_f[:, msl, :],
                                            float(w0), None, ALU.mult)
                nc.sync.dma_start(
                    dram_row(out_d, 9)[:, msl, :],
                    O9[:, s * (w // 2):(s + 1) * (w // 2)].rearrange(
                        "p (g c) -> p g c", g=2))
            # -Rp for stage 8 on Act while stage 9 finishes
            nR_tiles[8] = ng_pool.tile([P, h], F16, tag="nRp", name="nRp8")
            nr8_i = nc.scalar.activation(nR_tiles[8][:], rp_of(8), ACTF.Copy,
                                         bias=0.0, scale=-1.0)
            _after(nr8_i, nlp_insts[1])
            L = O9

            for i in reversed(range(1, NUM_STAGES - 1)):
                w0, w1 = weights[i]
                R = R_tiles[i]
                # prefetch R two stages ahead (stage 0's row arrives
                # host-deinterleaved in r0_d)
                if i - 2 >= 0:
                    R_tiles[i - 2] = r_pool.tile([P, w], F16, tag="rin",
                                                 name=f"R{i - 2}")
                    src = dram_row(r0_d) if i == 2 else dram_row(right_d, i - 2)
                    nc.sync.dma_start(sb(R_tiles[i - 2][:]), src)

                Lp, Ln = _halves(L[:], i)
                Rp, Rn = _halves(R[:], i)
                nRp = nR_tiles.pop(i)

                # DVE queue [t, v1, v2, u2, mv, u1, ON, OP]: no op directly
                # follows its producer (kills the RAW side-effect gaps), the
                # next stage's t needs only ON (2 slots before OP -> seamless
                # stage handoff), and u1 sits late enough that Pool's nt is
                # always ready.
                # nLp gates v2 (DVE slot 3): 3/4 on Act, 1/4 on the DVE in
                # the slot freed by t's Pool share
                m_i = w >> (i + 1)
                nq = m_i - max(1, m_i // 8)
                cq = nq << i
                nLp = ng_pool.tile([P, h], F16, tag="nLp", name=f"nLp{i}")
                nLp_i = nc.scalar.activation(nLp[:, :cq], Lp[:, :nq, :],
                                             ACTF.Copy, bias=0.0, scale=-1.0)

                # t = Ln + Rn, column-split: Pool is idle before nt, so it
                # takes the tail 3/8 there
                tq = m_i - max(1, (m_i * 7) // 16)
                ct = tq << i
                t = t_pool.tile([P, h], F16, tag="t", name=f"t{i}")
                t_i = nc.vector.tensor_tensor(t[:, :ct], Ln[:, :tq, :],
                                              Rn[:, :tq, :], ALU.add)
                nc.gpsimd.tensor_tensor(t[:, ct:], Ln[:, tq:, :],
                                        Rn[:, tq:, :], ALU.add)
                nlpd_i = nc.vector.tensor_scalar(nLp[:, cq:], Lp[:, nq:, :],
                                                 -1.0, None, ALU.mult)
                _after(nlpd_i, t_i)
                v1 = v_pool.tile([P, h], F16, tag="v1", name=f"v1_{i}")
                v1_i = nc.vector.tensor_tensor(v1[:], Lp, nRp[:], ALU.max)
                _after(v1_i, nlpd_i)
                # nt on Pool (tensor_scalar IS implemented on gpsimd): the
                # u-chain has slack, and it takes 594ns off the DVE
                nt = t_pool.tile([P, h], F16, tag="nt", name=f"nt{i}")
                nc.gpsimd.tensor_scalar(nt[:], t[:], -1.0, None, ALU.mult)

                v2 = v_pool.tile([P, h], F16, tag="v2", name=f"v2_{i}")
                v2_i = nc.vector.tensor_tensor(v2[:], nLp[:], Rp, ALU.max)
                _after(v2_i, v1_i)
                u2 = u_pool.tile([P, h], F16, tag="u2", name=f"u2_{i}")
                u2_i = nc.vector.tensor_tensor(u2[:], nLp[:], t[:], ALU.max)
                _after(u2_i, v2_i)

                O = lo_pool.tile([P, w], F16, tag="lo", name=f"O{i}")
                Op, On = _halves(O[:], i)

                mv = m_pool.tile([P, h], F16, tag="mv", name=f"mv{i}")
                mv_i = nc.vector.tensor_tensor(mv[:], v1[:], v2[:], ALU.min)
                _after(mv_i, u2_i)
                if w1 != 1.0:
                    mv_i = nc.vector.tensor_scalar(mv[:], mv[:], float(w1),
                                                   None, ALU.mult)
                u1 = u_pool.tile([P, h], F16, tag="u1", name=f"u1_{i}")
                u1_i = nc.vector.tensor_tensor(u1[:], Lp, nt[:], ALU.max)
                _after(u1_i, mv_i)
                # ON = mv + Ln, column-split: Pool (idle after nt) takes the
                # tail half right after mv while the DVE finishes u1/OP — the
                # only stage op Pool can legally run (fp16 tensor add).
                mh = max(1, (m_i * 13 + 16) // 32)   # ~13/32 on DVE, rest Pool
                ch = mh * (1 << i)
                nc.gpsimd.tensor_tensor(On[:, mh:, :], mv[:, ch:], Ln[:, mh:, :],
                                        ALU.add)
                on_i = nc.vector.tensor_tensor(On[:, :mh, :], mv[:, :ch],
                                               Ln[:, :mh, :], ALU.add)
                _after(on_i, u1_i)
                op_i = nc.vector.tensor_tensor(Op, u1[:], u2[:], ALU.min)
                _after(op_i, on_i)
                if w0 != 1.0:
                    op_i = nc.vector.tensor_scalar(Op, Op, float(w0), None,
                                                   ALU.mult)

                # -Rp for the NEXT stage while DVE finishes this one (kept
                # behind nLp on Act so it can't steal nLp's slot)
                nR_tiles[i - 1] = ng_pool.tile([P, h], F16, tag="nRp",
                                               name=f"nRp{i - 1}")
                nrp_i = nc.scalar.activation(nR_tiles[i - 1][:],
                                             rp_of(i - 1), ACTF.Copy,
                                             bias=0.0, scale=-1.0)
                _after(nrp_i, nLp_i)

                nc.sync.dma_start(dram_row(out_d, i), sb(O[:]))
                L = O

            # ---- stage 0: adjacent pairs -> deinterleave L row 1 into
            # packed halves (r0 arrives host-deinterleaved), column-split
            # into two batch-group halves so the writeback overlaps the
            # second half's compute.
            w0, w1 = weights[0]
            R0 = R_tiles[0]
            lv = L[:].rearrange("p (m two) -> p m two", two=2)
            Rp_f, Rn_f = _halves(R0[:], 9)
            nRp0 = nR_tiles.pop(0)
            Lp_t = pk_pool.tile([P, h], F16, tag="lpk", name="Lp0")
            Ln_t = pk_pool.tile([P, h], F16, tag="lnk", name="Ln0")
            nLp0 = ng_pool.tile([P, h], F16, tag="nLp", name="nLp0")
            t0 = t_pool.tile([P, h], F16, tag="t", name="t0")
            nt0 = t_pool.tile([P, h], F16, tag="nt", name="nt0")
            v10 = v_pool.tile([P, h], F16, tag="v1", name="v1_0")
            v20 = v_pool.tile([P, h], F16, tag="v2", name="v2_0")
            u10 = u_pool.tile([P, h], F16, tag="u1", name="u1_0")
            u20 = u_pool.tile([P, h], F16, tag="u2", name="u2_0")
            mv0 = m_pool.tile([P, h], F16, tag="mv", name="mv0")
            O0 = lo_pool.tile([P, w], F16, tag="lo", name="O0")
            Op_f, On_f = _halves(O0[:], 9)
            prev_act = None
            for s in (0, 1):
                msl = slice(2 * s, 2 * s + 2)
                csl = slice(s * h2, (s + 1) * h2)
                mm = slice(s * (w // 4), (s + 1) * (w // 4))
                lpc_i = nc.vector.tensor_copy(Lp_t[:, csl], lv[:, mm, 0])
                lnc_i = nc.scalar.activation(Ln_t[:, csl], lv[:, mm, 1],
                                             ACTF.Copy, bias=0.0, scale=1.0)
                nlp_i = nc.scalar.activation(nLp0[:, csl], lv[:, mm, 0],
                                             ACTF.Copy, bias=0.0, scale=-1.0)
                if prev_act is not None:
                    _after(lnc_i, prev_act)
                _after(nlp_i, lnc_i)
                prev_act = nlp_i
                Lp, Ln = Lp_t[:, csl], Ln_t[:, csl]
                Rp, Rn = Rp_f[:, msl, :], Rn_f[:, msl, :]
                t_i = nc.vector.tensor_tensor(t0[:, csl], Ln, Rn, ALU.add)
                _after(t_i, lpc_i)
                nc.gpsimd.tensor_scalar(nt0[:, csl], t0[:, csl], -1.0, None,
                                        ALU.mult)
                v1_i = nc.vector.tensor_tensor(v10[:, csl], Lp, nRp0[:, csl],
                                               ALU.max)
                _after(v1_i, t_i)
                v2_i = nc.vector.tensor_tensor(v20[:, csl], nLp0[:, csl], Rp,
                                               ALU.max)
                _after(v2_i, v1_i)
                u2_i = nc.vector.tensor_tensor(u20[:, csl], nLp0[:, csl],
                                               t0[:, csl], ALU.max)
                _after(u2_i, v2_i)
                mv_i = nc.vector.tensor_tensor(mv0[:, csl], v10[:, csl],
                                               v20[:, csl], ALU.min)
                _after(mv_i, u2_i)
                if w1 != 1.0:
                    mv_i = nc.vector.tensor_scalar(mv0[:, csl], mv0[:, csl],
                                                   float(w1), None, ALU.mult)
                u1_i = nc.vector.tensor_tensor(u10[:, csl], Lp, nt0[:, csl],
                                               ALU.max)
                _after(u1_i, mv_i)
                nc.gpsimd.tensor_tensor(
                    On_f[:, 2 * s + 1:2 * s + 2, :],
                    mv0[:, s * h2 + h2 // 2:(s + 1) * h2],
                    Ln_t[:, s * h2 + h2 // 2:(s + 1) * h2], ALU.add)
                on_i = nc.vector.tensor_tensor(
                    On_f[:, 2 * s:2 * s + 1, :],
                    mv0[:, s * h2:s * h2 + h2 // 2],
                    Ln_t[:, s * h2:s * h2 + h2 // 2], ALU.add)
                _after(on_i, u1_i)
                op_i = nc.vector.tensor_tensor(Op_f[:, msl, :], u10[:, csl],
                                               u20[:, csl], ALU.min)
                _after(op_i, on_i)
                if w0 != 1.0:
                    nc.vector.tensor_scalar(Op_f[:, msl, :], Op_f[:, msl, :],
                                            float(w0), None, ALU.mult)
                # split writeback: each half leaves while the rest computes
                nc.sync.dma_start(dram_row(out_d, 0)[:, msl, h // g:],
                                  On_f[:, msl, :])
                nc.sync.dma_start(dram_row(out_d, 0)[:, msl, :h // g],
                                  Op_f[:, msl, :])


TRACE = False
LAST_RESULTS = None


def _make_nc(weights, bpc):
    nc = bacc.Bacc("TRN2", target_bir_lowering=False, debug=False)
    build(nc, weights, bpc)
    nc.compile()
    return nc


def kernel(right, left, left_weights, iter):
    right = np.asarray(right)
    left = np.asarray(left)
    wsel = np.asarray(left_weights, dtype=np.float32)[int(iter)]  # [10, 2]
    weights = [(float(wsel[i, 0]), float(wsel[i, 1])) for i in range(NUM_STAGES)]

    bpc = B // N_CORES
    nc = _make_nc(weights, bpc)

    right16 = right[:, :NUM_STAGES, :].astype(np.float16)
    left10_16 = left[:, NUM_STAGES, :].astype(np.float16)
    # stage-0 right row, host-deinterleaved into packed pos|neg halves
    r0 = np.empty((B, CODE), np.float16)
    r0[:, :CODE // 2] = right16[:, 0, 0::2]
    r0[:, CODE // 2:] = right16[:, 0, 1::2]

    in_maps = []
    for c in range(N_CORES):
        sl = slice(c * bpc, (c + 1) * bpc)
        in_maps.append({
            "right": np.ascontiguousarray(right16[sl]),
            "right0": np.ascontiguousarray(r0[sl]),
            "left10": np.ascontiguousarray(left10_16[sl]),
        })
    global LAST_RESULTS
    LAST_RESULTS = run_bass_kernel_spmd(
        nc, in_maps, list(range(N_CORES)), trace=TRACE)
    res = LAST_RESULTS.results

    out16 = np.empty((B, NUM_STAGES, CODE), np.float16)
    for c in range(N_CORES):
        out16[c * bpc:(c + 1) * bpc] = res[c]["out"]
    # row 0 left the device as packed pos|neg halves; re-interleave
    row0 = out16[:, 0, :].copy()
    out16[:, 0, 0::2] = row0[:, :CODE // 2]
    out16[:, 0, 1::2] = row0[:, CODE // 2:]

    out = np.empty((B, NUM_STAGES + 1, CODE), np.float32)
    out[:, :NUM_STAGES, :] = np.clip(out16.astype(np.float32), -CLIP, CLIP)
    out[:, NUM_STAGES, :] = np.clip(left[:, NUM_STAGES, :], -CLIP, CLIP)
    return out


# revision 47
# speedup vs baseline: 2.1875x; 1.0049x over previous
"""Trainium2 Bass kernel: polar-BP left-message butterfly (nn_IterateLeftLayer).

Math per stage i (9..0), with L = left row i+1 (unclipped), R = right row i:
  out[pos] = w0 * ms(L[pos], L[neg] + R[neg])
  out[neg] = w1 * ms(L[pos], R[pos]) + L[neg]
where ms(x,y) = sign(x)sign(y)min(|x|,|y|), pos = {c: bit i of c == 0},
neg = pos + 2^i.  Final output = clip(left, +-10) with rows 0..9 replaced.

ms is computed in pure min/max form (no bit tricks):
  ms(a,b) = min(max(a,-b), max(-a,b))
which is exact (selection + sign flip only).  All on-device compute is fp16:
every DVE tensor_tensor runs in 2x mode and every tensor_scalar in 4x mode,
and DMA bytes halve.  fp16 rounding keeps rel err ~7e-4 vs the f32 oracle.

The +-10 output clip is applied on the host (the recurrence needs UNclipped
values anyway, so the device never clips).

Engine balance per stage (free dim h=2048 per op); the stage recurrence is
serial in L (every next-stage op needs the FULL previous row), so the per-
stage critical chain is what matters.  Assignment keeps the two cheap-engine
ops (Pool) OFF the chain's tail and the DVE queue ordered so it never
stalls more than ~0.4us:
  DVE : t=Ln+Rn, nt=-t, v2=max(nLp,t), mv=min(v1,v2), ON=mv+Ln,
        u2=max(nLp,t)... order [t, nt, v2, mv, ON, u2, OP=min(u1,u2)]
  Act : nLp=-Lp, then nRp for the NEXT stage (R is prefetched, so -Rp is
        computable one stage early)
  Pool: v1=max(Lp,nRp) (ready at row-start), u1=max(Lp,nt) (needed last)
  SP  : R-row prefetch + output writeback DMAs (double buffered)

Stage 0 pairs adjacent elements (stride 1), which would break the DVE 16-bit
packed fast path; instead L row 1 is deinterleaved on-device into packed
pos/neg halves, right row 0 arrives host-deinterleaved, and out row 0 leaves
as packed halves that the host re-interleaves.

Sharding: pure data-parallel over batch, 512 rows per core on 8 cores.
Layout: batch on partitions (4 groups of 128 coalesced along the free axis
-> [128, 4096] tiles); the butterfly is pure strided access patterns.
"""

import sys

for _p in ("/opt/trn_rl_repo",):
    if _p not in sys.path:
        sys.path.insert(0, _p)

import numpy as np

import bass_rust
import concourse.bass as bass
import concourse.tile as tile
from concourse import bacc, mybir
from concourse.bass_utils import run_bass_kernel_spmd


def _after(inst, *prevs):
    """Order `inst` after `prevs` on the same engine (scheduler edge, no
    semaphore) — pins queue order the greedy list scheduler would flip."""
    names = bass_rust.InstructionNameOrderedSet([p.ins.name for p in prevs])
    inst.ins.add_nosync_dependencies_from(names)

NUM_STAGES = 10
CODE = 1024
B = 4096
N_CORES = 8
P = 128
CLIP = 10.0
F16 = mybir.dt.float16
ALU = mybir.AluOpType
ACTF = mybir.ActivationFunctionType


def _halves(ap, i):
    """pos/neg strided views of a [P, W] row for stage i (i >= 1)."""
    r = 1 << i
    v = ap.rearrange("p (m two r) -> p m two r", two=2, r=r)
    return v[:, :, 0, :], v[:, :, 1, :]


def build(nc, weights, bpc):
    """Emit the per-core kernel. weights: [(w0, w1)] * 10, bpc: batch rows/core."""
    g = bpc // P
    w = g * CODE
    h = w // 2

    right_d = nc.dram_tensor("right", [bpc, NUM_STAGES, CODE], F16,
                             kind="ExternalInput")
    r0_d = nc.dram_tensor("right0", [bpc, CODE], F16, kind="ExternalInput")
    left10_d = nc.dram_tensor("left10", [bpc, CODE], F16, kind="ExternalInput")
    out_d = nc.dram_tensor("out", [bpc, NUM_STAGES, CODE], F16,
                           kind="ExternalOutput")

    def dram_row(dt_, i=None):
        ap = dt_.ap() if i is None else dt_.ap()[:, i, :]
        return ap.rearrange("(g p) c -> p g c", p=P)

    def sb(ap):
        return ap.rearrange("p (g c) -> p g c", g=g)

    with tile.TileContext(nc) as tc:
        with (
            tc.tile_pool(name="lo", bufs=3) as lo_pool,
            tc.tile_pool(name="rin", bufs=3) as r_pool,
            tc.tile_pool(name="tt", bufs=2) as t_pool,
            tc.tile_pool(name="ng", bufs=2) as ng_pool,
            tc.tile_pool(name="uu", bufs=2) as u_pool,
            tc.tile_pool(name="vv", bufs=2) as v_pool,
            tc.tile_pool(name="mv", bufs=2) as m_pool,
            tc.tile_pool(name="pk", bufs=1) as pk_pool,
        ):
            # fill: stage 9 is column-split into two batch-group halves
            # (independent rows), each fed by quarter DMAs in dependency
            # order (neg half first), so compute starts ~3.5us in.
            L = lo_pool.tile([P, w], F16, tag="lo", name="L10")
            R_tiles = {9: r_pool.tile([P, w], F16, tag="rin", name="R9")}
            for s in (0, 1):
                for half in (1, 0):   # neg halves first
                    for dt_, tile_ in ((left10_d, L), (right_d, R_tiles[9])):
                        src = dt_.ap() if dt_ is left10_d else dt_.ap()[:, 9, :]
                        src = src.rearrange(
                            "(g p) (two r) -> p g two r",
                            p=P, two=2, r=CODE // 2)[:, 2 * s:2 * s + 2, half, :]
                        dst = tile_[:].rearrange(
                            "p (g two r) -> p g two r",
                            two=2, r=CODE // 2)[:, 2 * s:2 * s + 2, half, :]
                        nc.sync.dma_start(dst, src)
            for i in (8,):
                R_tiles[i] = r_pool.tile([P, w], F16, tag="rin", name=f"R{i}")
                nc.sync.dma_start(sb(R_tiles[i][:]), dram_row(right_d, i))
            def rp_of(j):
                """pos-half view of stage j's R tile."""
                return _halves(R_tiles[j][:], 9 if j == 0 else j)[0]

            nR_tiles = {}

            # ---- stage 9, column-split into two independent batch-group
            # halves so each starts as soon as its quarter-DMAs land.
            # nRp9 runs on DVE (idle during fill) instead of Act.
            w0, w1 = weights[9]
            R9 = R_tiles[9]
            R_tiles[7] = r_pool.tile([P, w], F16, tag="rin", name="R7")
            nc.sync.dma_start(sb(R_tiles[7][:]), dram_row(right_d, 7))
            Lp_f, Ln_f = _halves(L[:], 9)
            Rp_f, Rn_f = _halves(R9[:], 9)
            h2 = h // 2
            nLp9 = ng_pool.tile([P, h], F16, tag="nLp", name="nLp9")
            nRp9 = ng_pool.tile([P, h], F16, tag="nRp9", name="nRp9")
            t9 = t_pool.tile([P, h], F16, tag="t", name="t9")
            nt9 = t_pool.tile([P, h], F16, tag="nt", name="nt9")
            v19 = v_pool.tile([P, h], F16, tag="v1", name="v1_9")
            v29 = v_pool.tile([P, h], F16, tag="v2", name="v2_9")
            u19 = u_pool.tile([P, h], F16, tag="u1", name="u1_9")
            u29 = u_pool.tile([P, h], F16, tag="u2", name="u2_9")
            mv9 = m_pool.tile([P, h], F16, tag="mv", name="mv9")
            O9 = lo_pool.tile([P, w], F16, tag="lo", name="O9")
            Op_f, On_f = _halves(O9[:], 9)
            nlp_insts = []
            for s in (0, 1):
                msl = slice(2 * s, 2 * s + 2)
                csl = slice(s * h2, (s + 1) * h2)
                Lp, Ln = Lp_f[:, msl, :], Ln_f[:, msl, :]
                Rp, Rn = Rp_f[:, msl, :], Rn_f[:, msl, :]
                nlp_insts.append(nc.scalar.activation(
                    nLp9[:, csl], Lp, ACTF.Copy, bias=0.0, scale=-1.0))
                t_i = nc.vector.tensor_tensor(t9[:, csl], Ln, Rn, ALU.add)
                nc.gpsimd.tensor_scalar(nt9[:, csl], t9[:, csl], -1.0, None,
                                        ALU.mult)
                nr_i = nc.vector.tensor_scalar(nRp9[:, csl], Rp, -1.0, None,
                                               ALU.mult)
                _after(nr_i, t_i)
                v1_i = nc.vector.tensor_tensor(v19[:, csl], Lp, nRp9[:, csl],
                                               ALU.max)
                _after(v1_i, nr_i)
                v2_i = nc.vector.tensor_tensor(v29[:, csl], nLp9[:, csl], Rp,
                                               ALU.max)
                _after(v2_i, v1_i)
                u2_i = nc.vector.tensor_tensor(u29[:, csl], nLp9[:, csl],
                                               t9[:, csl], ALU.max)
                _after(u2_i, v2_i)
                mv_i = nc.vector.tensor_tensor(mv9[:, csl], v19[:, csl],
                                               v29[:, csl], ALU.min)
                _after(mv_i, u2_i)
                if w1 != 1.0:
                    mv_i = nc.vector.tensor_scalar(mv9[:, csl], mv9[:, csl],
                                                   float(w1), None, ALU.mult)
                u1_i = nc.vector.tensor_tensor(u19[:, csl], Lp, nt9[:, csl],
                                               ALU.max)
                _after(u1_i, mv_i)
                nc.gpsimd.tensor_tensor(
                    On_f[:, 2 * s + 1:2 * s + 2, :],
                    mv9[:, s * h2 + h2 // 2:(s + 1) * h2],
                    Ln_f[:, 2 * s + 1:2 * s + 2, :], ALU.add)
                on_i = nc.vector.tensor_tensor(
                    On_f[:, 2 * s:2 * s + 1, :],
                    mv9[:, s * h2:s * h2 + h2 // 2],
                    Ln_f[:, 2 * s:2 * s + 1, :], ALU.add)
                _after(on_i, u1_i)
                op_i = nc.vector.tensor_tensor(Op_f[:, msl, :], u19[:, csl],
                                               u29[:, csl], ALU.min)
                _after(op_i, on_i)
                if w0 != 1.0:
                    nc.vector.tensor_scalar(Op_f[:, msl, :], Op_f[:, msl, :],
                                            float(w0), None, ALU.mult)
                nc.sync.dma_start(
                    dram_row(out_d, 9)[:, msl, :],
                    O9[:, s * (w // 2):(s + 1) * (w // 2)].rearrange(
                        "p (g c) -> p g c", g=2))
            # -Rp for stage 8 on Act while stage 9 finishes
            nR_tiles[8] = ng_pool.tile([P, h], F16, tag="nRp", name="nRp8")
            nr8_i = nc.scalar.activation(nR_tiles[8][:], rp_of(8), ACTF.Copy,
                                         bias=0.0, scale=-1.0)
            _after(nr8_i, nlp_insts[1])
            L = O9

            for i in reversed(range(1, NUM_STAGES - 1)):
                w0, w1 = weights[i]
                R = R_tiles[i]
                # prefetch R two stages ahead (stage 0's row arrives
                # host-deinterleaved in r0_d)
                if i - 2 >= 0:
                    R_tiles[i - 2] = r_pool.tile([P, w], F16, tag="rin",
                                                 name=f"R{i - 2}")
                    src = dram_row(r0_d) if i == 2 else dram_row(right_d, i - 2)
                    nc.sync.dma_start(sb(R_tiles[i - 2][:]), src)

                Lp, Ln = _halves(L[:], i)
                Rp, Rn = _halves(R[:], i)
                nRp = nR_tiles.pop(i)

                # DVE queue [t, v1, v2, u2, mv, u1, ON, OP]: no op directly
                # follows its producer (kills the RAW side-effect gaps), the
                # next stage's t needs only ON (2 slots before OP -> seamless
                # stage handoff), and u1 sits late enough that Pool's nt is
                # always ready.
                # nLp gates v2 (DVE slot 3): 3/4 on Act, 1/4 on the DVE in
                # the slot freed by t's Pool share
                m_i = w >> (i + 1)
                nq = m_i - max(1, m_i // 8)
                cq = nq << i
                nLp = ng_pool.tile([P, h], F16, tag="nLp", name=f"nLp{i}")
                nLp_i = nc.scalar.activation(nLp[:, :cq], Lp[:, :nq, :],
                                             ACTF.Copy, bias=0.0, scale=-1.0)

                # t = Ln + Rn, column-split: Pool is idle before nt, so it
                # takes the tail 3/8 there
                tq = m_i - max(1, (m_i * 15) // 32)
                ct = tq << i
                t = t_pool.tile([P, h], F16, tag="t", name=f"t{i}")
                t_i = nc.vector.tensor_tensor(t[:, :ct], Ln[:, :tq, :],
                                              Rn[:, :tq, :], ALU.add)
                nc.gpsimd.tensor_tensor(t[:, ct:], Ln[:, tq:, :],
                                        Rn[:, tq:, :], ALU.add)
                nlpd_i = nc.vector.tensor_scalar(nLp[:, cq:], Lp[:, nq:, :],
                                                 -1.0, None, ALU.mult)
                _after(nlpd_i, t_i)
                v1 = v_pool.tile([P, h], F16, tag="v1", name=f"v1_{i}")
                v1_i = nc.vector.tensor_tensor(v1[:], Lp, nRp[:], ALU.max)
                _after(v1_i, nlpd_i)
                # nt on Pool (tensor_scalar IS implemented on gpsimd): the
                # u-chain has slack, and it takes 594ns off the DVE
                nt = t_pool.tile([P, h], F16, tag="nt", name=f"nt{i}")
                nc.gpsimd.tensor_scalar(nt[:], t[:], -1.0, None, ALU.mult)

                v2 = v_pool.tile([P, h], F16, tag="v2", name=f"v2_{i}")
                v2_i = nc.vector.tensor_tensor(v2[:], nLp[:], Rp, ALU.max)
                _after(v2_i, v1_i)
                u2 = u_pool.tile([P, h], F16, tag="u2", name=f"u2_{i}")
                u2_i = nc.vector.tensor_tensor(u2[:], nLp[:], t[:], ALU.max)
                _after(u2_i, v2_i)

                O = lo_pool.tile([P, w], F16, tag="lo", name=f"O{i}")
                Op, On = _halves(O[:], i)

                mv = m_pool.tile([P, h], F16, tag="mv", name=f"mv{i}")
                mv_i = nc.vector.tensor_tensor(mv[:], v1[:], v2[:], ALU.min)
                _after(mv_i, u2_i)
                if w1 != 1.0:
                    mv_i = nc.vector.tensor_scalar(mv[:], mv[:], float(w1),
                                                   None, ALU.mult)
                u1 = u_pool.tile([P, h], F16, tag="u1", name=f"u1_{i}")
                u1_i = nc.vector.tensor_tensor(u1[:], Lp, nt[:], ALU.max)
                _after(u1_i, mv_i)
                # ON = mv + Ln, column-split: Pool (idle after nt) takes the
                # tail half right after mv while the DVE finishes u1/OP — the
                # only stage op Pool can legally run (fp16 tensor add).
                mh = max(1, (m_i * 3 + 4) // 8)   # ~3/8 on DVE, rest Pool
                ch = mh * (1 << i)
                nc.gpsimd.tensor_tensor(On[:, mh:, :], mv[:, ch:], Ln[:, mh:, :],
                                        ALU.add)
                on_i = nc.vector.tensor_tensor(On[:, :mh, :], mv[:, :ch],
                                               Ln[:, :mh, :], ALU.add)
                _after(on_i, u1_i)
                op_i = nc.vector.tensor_tensor(Op, u1[:], u2[:], ALU.min)
                _after(op_i, on_i)
                if w0 != 1.0:
                    op_i = nc.vector.tensor_scalar(Op, Op, float(w0), None,
                                                   ALU.mult)

                # -Rp for the NEXT stage while DVE finishes this one (kept
                # behind nLp on Act so it can't steal nLp's slot)
                nR_tiles[i - 1] = ng_pool.tile([P, h], F16, tag="nRp",
                                               name=f"nRp{i - 1}")
                nrp_i = nc.scalar.activation(nR_tiles[i - 1][:],
                                             rp_of(i - 1), ACTF.Copy,
                                             bias=0.0, scale=-1.0)
                _after(nrp_i, nLp_i)

                nc.sync.dma_start(dram_row(out_d, i), sb(O[:]))
                L = O

            # ---- stage 0: adjacent pairs -> deinterleave L row 1 into
            # packed halves (r0 arrives host-deinterleaved), column-split
            # into two batch-group halves so the writeback overlaps the
            # second half's compute.
            w0, w1 = weights[0]
            R0 = R_tiles[0]
            lv = L[:].rearrange("p (m two) -> p m two", two=2)
            Rp_f, Rn_f = _halves(R0[:], 9)
            nRp0 = nR_tiles.pop(0)
            Lp_t = pk_pool.tile([P, h], F16, tag="lpk", name="Lp0")
            Ln_t = pk_pool.tile([P, h], F16, tag="lnk", name="Ln0")
            nLp0 = ng_pool.tile([P, h], F16, tag="nLp", name="nLp0")
            t0 = t_pool.tile([P, h], F16, tag="t", name="t0")
            nt0 = t_pool.tile([P, h], F16, tag="nt", name="nt0")
            v10 = v_pool.tile([P, h], F16, tag="v1", name="v1_0")
            v20 = v_pool.tile([P, h], F16, tag="v2", name="v2_0")
            u10 = u_pool.tile([P, h], F16, tag="u1", name="u1_0")
            u20 = u_pool.tile([P, h], F16, tag="u2", name="u2_0")
            mv0 = m_pool.tile([P, h], F16, tag="mv", name="mv0")
            O0 = lo_pool.tile([P, w], F16, tag="lo", name="O0")
            Op_f, On_f = _halves(O0[:], 9)
            prev_act = None
            for s in (0, 1):
                msl = slice(2 * s, 2 * s + 2)
                csl = slice(s * h2, (s + 1) * h2)
                mm = slice(s * (w // 4), (s + 1) * (w // 4))
                lpc_i = nc.vector.tensor_copy(Lp_t[:, csl], lv[:, mm, 0])
                lnc_i = nc.scalar.activation(Ln_t[:, csl], lv[:, mm, 1],
                                             ACTF.Copy, bias=0.0, scale=1.0)
                nlp_i = nc.scalar.activation(nLp0[:, csl], lv[:, mm, 0],
                                             ACTF.Copy, bias=0.0, scale=-1.0)
                if prev_act is not None:
                    _after(lnc_i, prev_act)
                _after(nlp_i, lnc_i)
                prev_act = nlp_i
                Lp, Ln = Lp_t[:, csl], Ln_t[:, csl]
                Rp, Rn = Rp_f[:, msl, :], Rn_f[:, msl, :]
                t_i = nc.vector.tensor_tensor(t0[:, csl], Ln, Rn, ALU.add)
                _after(t_i, lpc_i)
                nc.gpsimd.tensor_scalar(nt0[:, csl], t0[:, csl], -1.0, None,
                                        ALU.mult)
                v1_i = nc.vector.tensor_tensor(v10[:, csl], Lp, nRp0[:, csl],
                                               ALU.max)
                _after(v1_i, t_i)
                v2_i = nc.vector.tensor_tensor(v20[:, csl], nLp0[:, csl], Rp,
                                               ALU.max)
                _after(v2_i, v1_i)
                u2_i = nc.vector.tensor_tensor(u20[:, csl], nLp0[:, csl],
                                               t0[:, csl], ALU.max)
                _after(u2_i, v2_i)
                mv_i = nc.vector.tensor_tensor(mv0[:, csl], v10[:, csl],
                                               v20[:, csl], ALU.min)
                _after(mv_i, u2_i)
                if w1 != 1.0:
                    mv_i = nc.vector.tensor_scalar(mv0[:, csl], mv0[:, csl],
                                                   float(w1), None, ALU.mult)
                u1_i = nc.vector.tensor_tensor(u10[:, csl], Lp, nt0[:, csl],
                                               ALU.max)
                _after(u1_i, mv_i)
                nc.gpsimd.tensor_tensor(
                    On_f[:, 2 * s + 1:2 * s + 2, :],
                    mv0[:, s * h2 + h2 // 2:(s + 1) * h2],
                    Ln_t[:, s * h2 + h2 // 2:(s + 1) * h2], ALU.add)
                on_i = nc.vector.tensor_tensor(
                    On_f[:, 2 * s:2 * s + 1, :],
                    mv0[:, s * h2:s * h2 + h2 // 2],
                    Ln_t[:, s * h2:s * h2 + h2 // 2], ALU.add)
                _after(on_i, u1_i)
                op_i = nc.vector.tensor_tensor(Op_f[:, msl, :], u10[:, csl],
                                               u20[:, csl], ALU.min)
                _after(op_i, on_i)
                if w0 != 1.0:
                    nc.vector.tensor_scalar(Op_f[:, msl, :], Op_f[:, msl, :],
                                            float(w0), None, ALU.mult)
                # split writeback: each half leaves while the rest computes
                nc.sync.dma_start(dram_row(out_d, 0)[:, msl, h // g:],
                                  On_f[:, msl, :])
                nc.sync.dma_start(dram_row(out_d, 0)[:, msl, :h // g],
                                  Op_f[:, msl, :])


TRACE = False
LAST_RESULTS = None


def _make_nc(weights, bpc):
    nc = bacc.Bacc("TRN2", target_bir_lowering=False, debug=False)
    build(nc, weights, bpc)
    nc.compile()
    return nc


def kernel(right, left, left_weights, iter):
    right = np.asarray(right)
    left = np.asarray(left)
    wsel = np.asarray(left_weights, dtype=np.float32)[int(iter)]  # [10, 2]
    weights = [(float(wsel[i, 0]), float(wsel[i, 1])) for i in range(NUM_STAGES)]

    bpc = B // N_CORES
    nc = _make_nc(weights, bpc)

    right16 = right[:, :NUM_STAGES, :].astype(np.float16)
    left10_16 = left[:, NUM_STAGES, :].astype(np.float16)
    # stage-0 right row, host-deinterleaved into packed pos|neg halves
    r0 = np.empty((B, CODE), np.float16)
    r0[:, :CODE // 2] = right16[:, 0, 0::2]
    r0[:, CODE // 2:] = right16[:, 0, 1::2]

    in_maps = []
    for c in range(N_CORES):
        sl = slice(c * bpc, (c + 1) * bpc)
        in_maps.append({
            "right": np.ascontiguousarray(right16[sl]),
            "right0": np.ascontiguousarray(r0[sl]),
            "left10": np.ascontiguousarray(left10_16[sl]),
        })
    global LAST_RESULTS
    LAST_RESULTS = run_bass_kernel_spmd(
        nc, in_maps, list(range(N_CORES)), trace=TRACE)
    res = LAST_RESULTS.results

    out16 = np.empty((B, NUM_STAGES, CODE), np.float16)
    for c in range(N_CORES):
        out16[c * bpc:(c + 1) * bpc] = res[c]["out"]
    # row 0 left the device as packed pos|neg halves; re-interleave
    row0 = out16[:, 0, :].copy()
    out16[:, 0, 0::2] = row0[:, :CODE // 2]
    out16[:, 0, 1::2] = row0[:, CODE // 2:]

    out = np.empty((B, NUM_STAGES + 1, CODE), np.float32)
    out[:, :NUM_STAGES, :] = np.clip(out16.astype(np.float32), -CLIP, CLIP)
    out[:, NUM_STAGES, :] = np.clip(left[:, NUM_STAGES, :], -CLIP, CLIP)
    return out


# revision 48
# speedup vs baseline: 2.1915x; 1.0018x over previous
"""Trainium2 Bass kernel: polar-BP left-message butterfly (nn_IterateLeftLayer).

Math per stage i (9..0), with L = left row i+1 (unclipped), R = right row i:
  out[pos] = w0 * ms(L[pos], L[neg] + R[neg])
  out[neg] = w1 * ms(L[pos], R[pos]) + L[neg]
where ms(x,y) = sign(x)sign(y)min(|x|,|y|), pos = {c: bit i of c == 0},
neg = pos + 2^i.  Final output = clip(left, +-10) with rows 0..9 replaced.

ms is computed in pure min/max form (no bit tricks):
  ms(a,b) = min(max(a,-b), max(-a,b))
which is exact (selection + sign flip only).  All on-device compute is fp16:
every DVE tensor_tensor runs in 2x mode and every tensor_scalar in 4x mode,
and DMA bytes halve.  fp16 rounding keeps rel err ~7e-4 vs the f32 oracle.

The +-10 output clip is applied on the host (the recurrence needs UNclipped
values anyway, so the device never clips).

Engine balance per stage (free dim h=2048 per op); the stage recurrence is
serial in L (every next-stage op needs the FULL previous row), so the per-
stage critical chain is what matters.  Assignment keeps the two cheap-engine
ops (Pool) OFF the chain's tail and the DVE queue ordered so it never
stalls more than ~0.4us:
  DVE : t=Ln+Rn, nt=-t, v2=max(nLp,t), mv=min(v1,v2), ON=mv+Ln,
        u2=max(nLp,t)... order [t, nt, v2, mv, ON, u2, OP=min(u1,u2)]
  Act : nLp=-Lp, then nRp for the NEXT stage (R is prefetched, so -Rp is
        computable one stage early)
  Pool: v1=max(Lp,nRp) (ready at row-start), u1=max(Lp,nt) (needed last)
  SP  : R-row prefetch + output writeback DMAs (double buffered)

Stage 0 pairs adjacent elements (stride 1), which would break the DVE 16-bit
packed fast path; instead L row 1 is deinterleaved on-device into packed
pos/neg halves, right row 0 arrives host-deinterleaved, and out row 0 leaves
as packed halves that the host re-interleaves.

Sharding: pure data-parallel over batch, 512 rows per core on 8 cores.
Layout: batch on partitions (4 groups of 128 coalesced along the free axis
-> [128, 4096] tiles); the butterfly is pure strided access patterns.
"""

import sys

for _p in ("/opt/trn_rl_repo",):
    if _p not in sys.path:
        sys.path.insert(0, _p)

import numpy as np

import bass_rust
import concourse.bass as bass
import concourse.tile as tile
from concourse import bacc, mybir
from concourse.bass_utils import run_bass_kernel_spmd


def _after(inst, *prevs):
    """Order `inst` after `prevs` on the same engine (scheduler edge, no
    semaphore) — pins queue order the greedy list scheduler would flip."""
    names = bass_rust.InstructionNameOrderedSet([p.ins.name for p in prevs])
    inst.ins.add_nosync_dependencies_from(names)

NUM_STAGES = 10
CODE = 1024
B = 4096
N_CORES = 8
P = 128
CLIP = 10.0
F16 = mybir.dt.float16
ALU = mybir.AluOpType
ACTF = mybir.ActivationFunctionType


def _halves(ap, i):
    """pos/neg strided views of a [P, W] row for stage i (i >= 1)."""
    r = 1 << i
    v = ap.rearrange("p (m two r) -> p m two r", two=2, r=r)
    return v[:, :, 0, :], v[:, :, 1, :]


def build(nc, weights, bpc):
    """Emit the per-core kernel. weights: [(w0, w1)] * 10, bpc: batch rows/core."""
    g = bpc // P
    w = g * CODE
    h = w // 2

    right_d = nc.dram_tensor("right", [bpc, NUM_STAGES, CODE], F16,
                             kind="ExternalInput")
    r0_d = nc.dram_tensor("right0", [bpc, CODE], F16, kind="ExternalInput")
    left10_d = nc.dram_tensor("left10", [bpc, CODE], F16, kind="ExternalInput")
    out_d = nc.dram_tensor("out", [bpc, NUM_STAGES, CODE], F16,
                           kind="ExternalOutput")

    def dram_row(dt_, i=None):
        ap = dt_.ap() if i is None else dt_.ap()[:, i, :]
        return ap.rearrange("(g p) c -> p g c", p=P)

    def sb(ap):
        return ap.rearrange("p (g c) -> p g c", g=g)

    with tile.TileContext(nc) as tc:
        with (
            tc.tile_pool(name="lo", bufs=3) as lo_pool,
            tc.tile_pool(name="rin", bufs=3) as r_pool,
            tc.tile_pool(name="tt", bufs=2) as t_pool,
            tc.tile_pool(name="ng", bufs=2) as ng_pool,
            tc.tile_pool(name="uu", bufs=2) as u_pool,
            tc.tile_pool(name="vv", bufs=2) as v_pool,
            tc.tile_pool(name="mv", bufs=2) as m_pool,
            tc.tile_pool(name="pk", bufs=1) as pk_pool,
        ):
            # fill: stage 9 is column-split into two batch-group halves
            # (independent rows), each fed by quarter DMAs in dependency
            # order (neg half first), so compute starts ~3.5us in.
            L = lo_pool.tile([P, w], F16, tag="lo", name="L10")
            R_tiles = {9: r_pool.tile([P, w], F16, tag="rin", name="R9")}
            for s in (0, 1):
                for half in (1, 0):   # neg halves first
                    for dt_, tile_ in ((left10_d, L), (right_d, R_tiles[9])):
                        src = dt_.ap() if dt_ is left10_d else dt_.ap()[:, 9, :]
                        src = src.rearrange(
                            "(g p) (two r) -> p g two r",
                            p=P, two=2, r=CODE // 2)[:, 2 * s:2 * s + 2, half, :]
                        dst = tile_[:].rearrange(
                            "p (g two r) -> p g two r",
                            two=2, r=CODE // 2)[:, 2 * s:2 * s + 2, half, :]
                        nc.sync.dma_start(dst, src)
            for i in (8,):
                R_tiles[i] = r_pool.tile([P, w], F16, tag="rin", name=f"R{i}")
                nc.sync.dma_start(sb(R_tiles[i][:]), dram_row(right_d, i))
            def rp_of(j):
                """pos-half view of stage j's R tile."""
                return _halves(R_tiles[j][:], 9 if j == 0 else j)[0]

            nR_tiles = {}

            # ---- stage 9, column-split into two independent batch-group
            # halves so each starts as soon as its quarter-DMAs land.
            # nRp9 runs on DVE (idle during fill) instead of Act.
            w0, w1 = weights[9]
            R9 = R_tiles[9]
            R_tiles[7] = r_pool.tile([P, w], F16, tag="rin", name="R7")
            nc.sync.dma_start(sb(R_tiles[7][:]), dram_row(right_d, 7))
            Lp_f, Ln_f = _halves(L[:], 9)
            Rp_f, Rn_f = _halves(R9[:], 9)
            h2 = h // 2
            nLp9 = ng_pool.tile([P, h], F16, tag="nLp", name="nLp9")
            nRp9 = ng_pool.tile([P, h], F16, tag="nRp9", name="nRp9")
            t9 = t_pool.tile([P, h], F16, tag="t", name="t9")
            nt9 = t_pool.tile([P, h], F16, tag="nt", name="nt9")
            v19 = v_pool.tile([P, h], F16, tag="v1", name="v1_9")
            v29 = v_pool.tile([P, h], F16, tag="v2", name="v2_9")
            u19 = u_pool.tile([P, h], F16, tag="u1", name="u1_9")
            u29 = u_pool.tile([P, h], F16, tag="u2", name="u2_9")
            mv9 = m_pool.tile([P, h], F16, tag="mv", name="mv9")
            O9 = lo_pool.tile([P, w], F16, tag="lo", name="O9")
            Op_f, On_f = _halves(O9[:], 9)
            nlp_insts = []
            for s in (0, 1):
                msl = slice(2 * s, 2 * s + 2)
                csl = slice(s * h2, (s + 1) * h2)
                Lp, Ln = Lp_f[:, msl, :], Ln_f[:, msl, :]
                Rp, Rn = Rp_f[:, msl, :], Rn_f[:, msl, :]
                nlp_insts.append(nc.scalar.activation(
                    nLp9[:, csl], Lp, ACTF.Copy, bias=0.0, scale=-1.0))
                t_i = nc.vector.tensor_tensor(t9[:, csl], Ln, Rn, ALU.add)
                nc.gpsimd.tensor_scalar(nt9[:, csl], t9[:, csl], -1.0, None,
                                        ALU.mult)
                nr_i = nc.vector.tensor_scalar(nRp9[:, csl], Rp, -1.0, None,
                                               ALU.mult)
                _after(nr_i, t_i)
                v1_i = nc.vector.tensor_tensor(v19[:, csl], Lp, nRp9[:, csl],
                                               ALU.max)
                _after(v1_i, nr_i)
                v2_i = nc.vector.tensor_tensor(v29[:, csl], nLp9[:, csl], Rp,
                                               ALU.max)
                _after(v2_i, v1_i)
                u2_i = nc.vector.tensor_tensor(u29[:, csl], nLp9[:, csl],
                                               t9[:, csl], ALU.max)
                _after(u2_i, v2_i)
                mv_i = nc.vector.tensor_tensor(mv9[:, csl], v19[:, csl],
                                               v29[:, csl], ALU.min)
                _after(mv_i, u2_i)
                if w1 != 1.0:
                    mv_i = nc.vector.tensor_scalar(mv9[:, csl], mv9[:, csl],
                                                   float(w1), None, ALU.mult)
                u1_i = nc.vector.tensor_tensor(u19[:, csl], Lp, nt9[:, csl],
                                               ALU.max)
                _after(u1_i, mv_i)
                nc.gpsimd.tensor_tensor(
                    On_f[:, 2 * s + 1:2 * s + 2, :],
                    mv9[:, s * h2 + h2 // 2:(s + 1) * h2],
                    Ln_f[:, 2 * s + 1:2 * s + 2, :], ALU.add)
                on_i = nc.vector.tensor_tensor(
                    On_f[:, 2 * s:2 * s + 1, :],
                    mv9[:, s * h2:s * h2 + h2 // 2],
                    Ln_f[:, 2 * s:2 * s + 1, :], ALU.add)
                _after(on_i, u1_i)
                op_i = nc.vector.tensor_tensor(Op_f[:, msl, :], u19[:, csl],
                                               u29[:, csl], ALU.min)
                _after(op_i, on_i)
                if w0 != 1.0:
                    nc.vector.tensor_scalar(Op_f[:, msl, :], Op_f[:, msl, :],
                                            float(w0), None, ALU.mult)
                nc.sync.dma_start(
                    dram_row(out_d, 9)[:, msl, :],
                    O9[:, s * (w // 2):(s + 1) * (w // 2)].rearrange(
                        "p (g c) -> p g c", g=2))
            # -Rp for stage 8 on Act while stage 9 finishes
            nR_tiles[8] = ng_pool.tile([P, h], F16, tag="nRp", name="nRp8")
            nr8_i = nc.scalar.activation(nR_tiles[8][:], rp_of(8), ACTF.Copy,
                                         bias=0.0, scale=-1.0)
            _after(nr8_i, nlp_insts[1])
            L = O9

            for i in reversed(range(1, NUM_STAGES - 1)):
                w0, w1 = weights[i]
                R = R_tiles[i]
                # prefetch R two stages ahead (stage 0's row arrives
                # host-deinterleaved in r0_d)
                if i - 2 >= 0:
                    R_tiles[i - 2] = r_pool.tile([P, w], F16, tag="rin",
                                                 name=f"R{i - 2}")
                    src = dram_row(r0_d) if i == 2 else dram_row(right_d, i - 2)
                    nc.sync.dma_start(sb(R_tiles[i - 2][:]), src)

                Lp, Ln = _halves(L[:], i)
                Rp, Rn = _halves(R[:], i)
                nRp = nR_tiles.pop(i)

                # DVE queue [t, v1, v2, u2, mv, u1, ON, OP]: no op directly
                # follows its producer (kills the RAW side-effect gaps), the
                # next stage's t needs only ON (2 slots before OP -> seamless
                # stage handoff), and u1 sits late enough that Pool's nt is
                # always ready.
                # nLp gates v2 (DVE slot 3): 3/4 on Act, 1/4 on the DVE in
                # the slot freed by t's Pool share
                m_i = w >> (i + 1)
                nq = m_i - max(1, m_i // 8)
                cq = nq << i
                nLp = ng_pool.tile([P, h], F16, tag="nLp", name=f"nLp{i}")
                nLp_i = nc.scalar.activation(nLp[:, :cq], Lp[:, :nq, :],
                                             ACTF.Copy, bias=0.0, scale=-1.0)

                # t = Ln + Rn, column-split: Pool is idle before nt, so it
                # takes the tail 3/8 there
                tq = m_i - max(1, m_i // 2)
                ct = tq << i
                t = t_pool.tile([P, h], F16, tag="t", name=f"t{i}")
                t_i = nc.vector.tensor_tensor(t[:, :ct], Ln[:, :tq, :],
                                              Rn[:, :tq, :], ALU.add)
                nc.gpsimd.tensor_tensor(t[:, ct:], Ln[:, tq:, :],
                                        Rn[:, tq:, :], ALU.add)
                nlpd_i = nc.vector.tensor_scalar(nLp[:, cq:], Lp[:, nq:, :],
                                                 -1.0, None, ALU.mult)
                _after(nlpd_i, t_i)
                v1 = v_pool.tile([P, h], F16, tag="v1", name=f"v1_{i}")
                v1_i = nc.vector.tensor_tensor(v1[:], Lp, nRp[:], ALU.max)
                _after(v1_i, nlpd_i)
                # nt on Pool (tensor_scalar IS implemented on gpsimd): the
                # u-chain has slack, and it takes 594ns off the DVE
                nt = t_pool.tile([P, h], F16, tag="nt", name=f"nt{i}")
                nc.gpsimd.tensor_scalar(nt[:], t[:], -1.0, None, ALU.mult)

                v2 = v_pool.tile([P, h], F16, tag="v2", name=f"v2_{i}")
                v2_i = nc.vector.tensor_tensor(v2[:], nLp[:], Rp, ALU.max)
                _after(v2_i, v1_i)
                u2 = u_pool.tile([P, h], F16, tag="u2", name=f"u2_{i}")
                u2_i = nc.vector.tensor_tensor(u2[:], nLp[:], t[:], ALU.max)
                _after(u2_i, v2_i)

                O = lo_pool.tile([P, w], F16, tag="lo", name=f"O{i}")
                Op, On = _halves(O[:], i)

                mv = m_pool.tile([P, h], F16, tag="mv", name=f"mv{i}")
                mv_i = nc.vector.tensor_tensor(mv[:], v1[:], v2[:], ALU.min)
                _after(mv_i, u2_i)
                if w1 != 1.0:
                    mv_i = nc.vector.tensor_scalar(mv[:], mv[:], float(w1),
                                                   None, ALU.mult)
                u1 = u_pool.tile([P, h], F16, tag="u1", name=f"u1_{i}")
                u1_i = nc.vector.tensor_tensor(u1[:], Lp, nt[:], ALU.max)
                _after(u1_i, mv_i)
                # ON = mv + Ln, column-split: Pool (idle after nt) takes the
                # tail half right after mv while the DVE finishes u1/OP — the
                # only stage op Pool can legally run (fp16 tensor add).
                mh = max(1, (m_i * 3 + 4) // 8)   # ~3/8 on DVE, rest Pool
                ch = mh * (1 << i)
                nc.gpsimd.tensor_tensor(On[:, mh:, :], mv[:, ch:], Ln[:, mh:, :],
                                        ALU.add)
                on_i = nc.vector.tensor_tensor(On[:, :mh, :], mv[:, :ch],
                                               Ln[:, :mh, :], ALU.add)
                _after(on_i, u1_i)
                op_i = nc.vector.tensor_tensor(Op, u1[:], u2[:], ALU.min)
                _after(op_i, on_i)
                if w0 != 1.0:
                    op_i = nc.vector.tensor_scalar(Op, Op, float(w0), None,
                                                   ALU.mult)

                # -Rp for the NEXT stage while DVE finishes this one (kept
                # behind nLp on Act so it can't steal nLp's slot)
                nR_tiles[i - 1] = ng_pool.tile([P, h], F16, tag="nRp",
                                               name=f"nRp{i - 1}")
                nrp_i = nc.scalar.activation(nR_tiles[i - 1][:],
                                             rp_of(i - 1), ACTF.Copy,
                                             bias=0.0, scale=-1.0)
                _after(nrp_i, nLp_i)

                nc.sync.dma_start(dram_row(out_d, i), sb(O[:]))
                L = O

            # ---- stage 0: adjacent pairs -> deinterleave L row 1 into
            # packed halves (r0 arrives host-deinterleaved), column-split
            # into two batch-group halves so the writeback overlaps the
            # second half's compute.
            w0, w1 = weights[0]
            R0 = R_tiles[0]
            lv = L[:].rearrange("p (m two) -> p m two", two=2)
            Rp_f, Rn_f = _halves(R0[:], 9)
            nRp0 = nR_tiles.pop(0)
            Lp_t = pk_pool.tile([P, h], F16, tag="lpk", name="Lp0")
            Ln_t = pk_pool.tile([P, h], F16, tag="lnk", name="Ln0")
            nLp0 = ng_pool.tile([P, h], F16, tag="nLp", name="nLp0")
            t0 = t_pool.tile([P, h], F16, tag="t", name="t0")
            nt0 = t_pool.tile([P, h], F16, tag="nt", name="nt0")
            v10 = v_pool.tile([P, h], F16, tag="v1", name="v1_0")
            v20 = v_pool.tile([P, h], F16, tag="v2", name="v2_0")
            u10 = u_pool.tile([P, h], F16, tag="u1", name="u1_0")
            u20 = u_pool.tile([P, h], F16, tag="u2", name="u2_0")
            mv0 = m_pool.tile([P, h], F16, tag="mv", name="mv0")
            O0 = lo_pool.tile([P, w], F16, tag="lo", name="O0")
            Op_f, On_f = _halves(O0[:], 9)
            prev_act = None
            for s in (0, 1):
                msl = slice(2 * s, 2 * s + 2)
                csl = slice(s * h2, (s + 1) * h2)
                mm = slice(s * (w // 4), (s + 1) * (w // 4))
                lpc_i = nc.vector.tensor_copy(Lp_t[:, csl], lv[:, mm, 0])
                lnc_i = nc.scalar.activation(Ln_t[:, csl], lv[:, mm, 1],
                                             ACTF.Copy, bias=0.0, scale=1.0)
                nlp_i = nc.scalar.activation(nLp0[:, csl], lv[:, mm, 0],
                                             ACTF.Copy, bias=0.0, scale=-1.0)
                if prev_act is not None:
                    _after(lnc_i, prev_act)
                _after(nlp_i, lnc_i)
                prev_act = nlp_i
                Lp, Ln = Lp_t[:, csl], Ln_t[:, csl]
                Rp, Rn = Rp_f[:, msl, :], Rn_f[:, msl, :]
                t_i = nc.vector.tensor_tensor(t0[:, csl], Ln, Rn, ALU.add)
                _after(t_i, lpc_i)
                nc.gpsimd.tensor_scalar(nt0[:, csl], t0[:, csl], -1.0, None,
                                        ALU.mult)
                v1_i = nc.vector.tensor_tensor(v10[:, csl], Lp, nRp0[:, csl],
                                               ALU.max)
                _after(v1_i, t_i)
                v2_i = nc.vector.tensor_tensor(v20[:, csl], nLp0[:, csl], Rp,
                                               ALU.max)
                _after(v2_i, v1_i)
                u2_i = nc.vector.tensor_tensor(u20[:, csl], nLp0[:, csl],
                                               t0[:, csl], ALU.max)
                _after(u2_i, v2_i)
                mv_i = nc.vector.tensor_tensor(mv0[:, csl], v10[:, csl],
                                               v20[:, csl], ALU.min)
                _after(mv_i, u2_i)
                if w1 != 1.0:
                    mv_i = nc.vector.tensor_scalar(mv0[:, csl], mv0[:, csl],
                                                   float(w1), None, ALU.mult)
                u1_i = nc.vector.tensor_tensor(u10[:, csl], Lp, nt0[:, csl],
                                               ALU.max)
                _after(u1_i, mv_i)
                nc.gpsimd.tensor_tensor(
                    On_f[:, 2 * s + 1:2 * s + 2, :],
                    mv0[:, s * h2 + h2 // 2:(s + 1) * h2],
                    Ln_t[:, s * h2 + h2 // 2:(s + 1) * h2], ALU.add)
                on_i = nc.vector.tensor_tensor(
                    On_f[:, 2 * s:2 * s + 1, :],
                    mv0[:, s * h2:s * h2 + h2 // 2],
                    Ln_t[:, s * h2:s * h2 + h2 // 2], ALU.add)
                _after(on_i, u1_i)
                op_i = nc.vector.tensor_tensor(Op_f[:, msl, :], u10[:, csl],
                                               u20[:, csl], ALU.min)
                _after(op_i, on_i)
                if w0 != 1.0:
                    nc.vector.tensor_scalar(Op_f[:, msl, :], Op_f[:, msl, :],
                                            float(w0), None, ALU.mult)
                # split writeback: each half leaves while the rest computes
                nc.sync.dma_start(dram_row(out_d, 0)[:, msl, h // g:],
                                  On_f[:, msl, :])
                nc.sync.dma_start(dram_row(out_d, 0)[:, msl, :h // g],
                                  Op_f[:, msl, :])


TRACE = False
LAST_RESULTS = None


def _make_nc(weights, bpc):
    nc = bacc.Bacc("TRN2", target_bir_lowering=False, debug=False)
    build(nc, weights, bpc)
    nc.compile()
    return nc


def kernel(right, left, left_weights, iter):
    right = np.asarray(right)
    left = np.asarray(left)
    wsel = np.asarray(left_weights, dtype=np.float32)[int(iter)]  # [10, 2]
    weights = [(float(wsel[i, 0]), float(wsel[i, 1])) for i in range(NUM_STAGES)]

    bpc = B // N_CORES
    nc = _make_nc(weights, bpc)

    right16 = right[:, :NUM_STAGES, :].astype(np.float16)
    left10_16 = left[:, NUM_STAGES, :].astype(np.float16)
    # stage-0 right row, host-deinterleaved into packed pos|neg halves
    r0 = np.empty((B, CODE), np.float16)
    r0[:, :CODE // 2] = right16[:, 0, 0::2]
    r0[:, CODE // 2:] = right16[:, 0, 1::2]

    in_maps = []
    for c in range(N_CORES):
        sl = slice(c * bpc, (c + 1) * bpc)
        in_maps.append({
            "right": np.ascontiguousarray(right16[sl]),
            "right0": np.ascontiguousarray(r0[sl]),
            "left10": np.ascontiguousarray(left10_16[sl]),
        })
    global LAST_RESULTS
    LAST_RESULTS = run_bass_kernel_spmd(
        nc, in_maps, list(range(N_CORES)), trace=TRACE)
    res = LAST_RESULTS.results

    out16 = np.empty((B, NUM_STAGES, CODE), np.float16)
    for c in range(N_CORES):
        out16[c * bpc:(c + 1) * bpc] = res[c]["out"]
    # row 0 left the device as packed pos|neg halves; re-interleave
    row0 = out16[:, 0, :].copy()
    out16[:, 0, 0::2] = row0[:, :CODE // 2]
    out16[:, 0, 1::2] = row0[:, CODE // 2:]

    out = np.empty((B, NUM_STAGES + 1, CODE), np.float32)
    out[:, :NUM_STAGES, :] = np.clip(out16.astype(np.float32), -CLIP, CLIP)
    out[:, NUM_STAGES, :] = np.clip(left[:, NUM_STAGES, :], -CLIP, CLIP)
    return out


# revision 55
# speedup vs baseline: 2.2025x; 1.0050x over previous
"""Trainium2 Bass kernel: polar-BP left-message butterfly (nn_IterateLeftLayer).

Math per stage i (9..0), with L = left row i+1 (unclipped), R = right row i:
  out[pos] = w0 * ms(L[pos], L[neg] + R[neg])
  out[neg] = w1 * ms(L[pos], R[pos]) + L[neg]
where ms(x,y) = sign(x)sign(y)min(|x|,|y|), pos = {c: bit i of c == 0},
neg = pos + 2^i.  Final output = clip(left, +-10) with rows 0..9 replaced.

ms is computed in pure min/max form (no bit tricks):
  ms(a,b) = min(max(a,-b), max(-a,b))
which is exact (selection + sign flip only).  All on-device compute is fp16:
every DVE tensor_tensor runs in 2x mode and every tensor_scalar in 4x mode,
and DMA bytes halve.  fp16 rounding keeps rel err ~7e-4 vs the f32 oracle.

The +-10 output clip is applied on the host (the recurrence needs UNclipped
values anyway, so the device never clips).

Engine balance per stage (free dim h=2048 per op); the stage recurrence is
serial in L (every next-stage op needs the FULL previous row), so the per-
stage critical chain is what matters.  Assignment keeps the two cheap-engine
ops (Pool) OFF the chain's tail and the DVE queue ordered so it never
stalls more than ~0.4us:
  DVE : t=Ln+Rn, nt=-t, v2=max(nLp,t), mv=min(v1,v2), ON=mv+Ln,
        u2=max(nLp,t)... order [t, nt, v2, mv, ON, u2, OP=min(u1,u2)]
  Act : nLp=-Lp, then nRp for the NEXT stage (R is prefetched, so -Rp is
        computable one stage early)
  Pool: v1=max(Lp,nRp) (ready at row-start), u1=max(Lp,nt) (needed last)
  SP  : R-row prefetch + output writeback DMAs (double buffered)

Stage 0 pairs adjacent elements (stride 1), which would break the DVE 16-bit
packed fast path; instead L row 1 is deinterleaved on-device into packed
pos/neg halves, right row 0 arrives host-deinterleaved, and out row 0 leaves
as packed halves that the host re-interleaves.

Sharding: pure data-parallel over batch, 512 rows per core on 8 cores.
Layout: batch on partitions (4 groups of 128 coalesced along the free axis
-> [128, 4096] tiles); the butterfly is pure strided access patterns.
"""

import sys

for _p in ("/opt/trn_rl_repo",):
    if _p not in sys.path:
        sys.path.insert(0, _p)

import numpy as np

import bass_rust
import concourse.bass as bass
import concourse.tile as tile
from concourse import bacc, mybir
from concourse.bass_utils import run_bass_kernel_spmd


def _after(inst, *prevs):
    """Order `inst` after `prevs` on the same engine (scheduler edge, no
    semaphore) — pins queue order the greedy list scheduler would flip."""
    names = bass_rust.InstructionNameOrderedSet([p.ins.name for p in prevs])
    inst.ins.add_nosync_dependencies_from(names)

NUM_STAGES = 10
CODE = 1024
B = 4096
N_CORES = 8
P = 128
CLIP = 10.0
F16 = mybir.dt.float16
ALU = mybir.AluOpType
ACTF = mybir.ActivationFunctionType


def _halves(ap, i):
    """pos/neg strided views of a [P, W] row for stage i (i >= 1)."""
    r = 1 << i
    v = ap.rearrange("p (m two r) -> p m two r", two=2, r=r)
    return v[:, :, 0, :], v[:, :, 1, :]


def build(nc, weights, bpc):
    """Emit the per-core kernel. weights: [(w0, w1)] * 10, bpc: batch rows/core."""
    g = bpc // P
    w = g * CODE
    h = w // 2

    right_d = nc.dram_tensor("right", [bpc, NUM_STAGES, CODE], F16,
                             kind="ExternalInput")
    r0_d = nc.dram_tensor("right0", [bpc, CODE], F16, kind="ExternalInput")
    left10_d = nc.dram_tensor("left10", [bpc, CODE], F16, kind="ExternalInput")
    out_d = nc.dram_tensor("out", [bpc, NUM_STAGES, CODE], F16,
                           kind="ExternalOutput")

    def dram_row(dt_, i=None):
        ap = dt_.ap() if i is None else dt_.ap()[:, i, :]
        return ap.rearrange("(g p) c -> p g c", p=P)

    def sb(ap):
        return ap.rearrange("p (g c) -> p g c", g=g)

    with tile.TileContext(nc) as tc:
        with (
            tc.tile_pool(name="lo", bufs=3) as lo_pool,
            tc.tile_pool(name="rin", bufs=3) as r_pool,
            tc.tile_pool(name="tt", bufs=2) as t_pool,
            tc.tile_pool(name="ng", bufs=2) as ng_pool,
            tc.tile_pool(name="uu", bufs=2) as u_pool,
            tc.tile_pool(name="vv", bufs=2) as v_pool,
            tc.tile_pool(name="mv", bufs=2) as m_pool,
            tc.tile_pool(name="pk", bufs=1) as pk_pool,
        ):
            # fill: stage 9 is column-split into two batch-group halves
            # (independent rows), each fed by quarter DMAs in dependency
            # order (neg half first), so compute starts ~3.5us in.
            L = lo_pool.tile([P, w], F16, tag="lo", name="L10")
            R_tiles = {9: r_pool.tile([P, w], F16, tag="rin", name="R9")}
            for s in (0, 1):
                for half in (1, 0):   # neg halves first
                    for dt_, tile_ in ((left10_d, L), (right_d, R_tiles[9])):
                        src = dt_.ap() if dt_ is left10_d else dt_.ap()[:, 9, :]
                        src = src.rearrange(
                            "(g p) (two r) -> p g two r",
                            p=P, two=2, r=CODE // 2)[:, 2 * s:2 * s + 2, half, :]
                        dst = tile_[:].rearrange(
                            "p (g two r) -> p g two r",
                            two=2, r=CODE // 2)[:, 2 * s:2 * s + 2, half, :]
                        nc.sync.dma_start(dst, src)
            for i in (8,):
                R_tiles[i] = r_pool.tile([P, w], F16, tag="rin", name=f"R{i}")
                nc.sync.dma_start(sb(R_tiles[i][:]), dram_row(right_d, i))
            def rp_of(j):
                """pos-half view of stage j's R tile."""
                return _halves(R_tiles[j][:], 9 if j == 0 else j)[0]

            nR_tiles = {}

            # ---- stage 9, column-split into two independent batch-group
            # halves so each starts as soon as its quarter-DMAs land.
            # nRp9 runs on DVE (idle during fill) instead of Act.
            w0, w1 = weights[9]
            R9 = R_tiles[9]
            R_tiles[7] = r_pool.tile([P, w], F16, tag="rin", name="R7")
            nc.sync.dma_start(sb(R_tiles[7][:]), dram_row(right_d, 7))
            Lp_f, Ln_f = _halves(L[:], 9)
            Rp_f, Rn_f = _halves(R9[:], 9)
            h2 = h // 2
            nLp9 = ng_pool.tile([P, h], F16, tag="nLp", name="nLp9")
            nRp9 = ng_pool.tile([P, h], F16, tag="nRp9", name="nRp9")
            t9 = t_pool.tile([P, h], F16, tag="t", name="t9")
            nt9 = t_pool.tile([P, h], F16, tag="nt", name="nt9")
            v19 = v_pool.tile([P, h], F16, tag="v1", name="v1_9")
            v29 = v_pool.tile([P, h], F16, tag="v2", name="v2_9")
            u19 = u_pool.tile([P, h], F16, tag="u1", name="u1_9")
            u29 = u_pool.tile([P, h], F16, tag="u2", name="u2_9")
            mv9 = m_pool.tile([P, h], F16, tag="mv", name="mv9")
            O9 = lo_pool.tile([P, w], F16, tag="lo", name="O9")
            Op_f, On_f = _halves(O9[:], 9)
            nlp_insts = []
            for s in (0, 1):
                msl = slice(2 * s, 2 * s + 2)
                csl = slice(s * h2, (s + 1) * h2)
                Lp, Ln = Lp_f[:, msl, :], Ln_f[:, msl, :]
                Rp, Rn = Rp_f[:, msl, :], Rn_f[:, msl, :]
                nlp_insts.append(nc.scalar.activation(
                    nLp9[:, csl], Lp, ACTF.Copy, bias=0.0, scale=-1.0))
                t_i = nc.vector.tensor_tensor(t9[:, csl], Ln, Rn, ALU.add)
                nc.gpsimd.tensor_scalar(nt9[:, csl], t9[:, csl], -1.0, None,
                                        ALU.mult)
                nr_i = nc.vector.tensor_scalar(nRp9[:, csl], Rp, -1.0, None,
                                               ALU.mult)
                _after(nr_i, t_i)
                v1_i = nc.vector.tensor_tensor(v19[:, csl], Lp, nRp9[:, csl],
                                               ALU.max)
                _after(v1_i, nr_i)
                v2_i = nc.vector.tensor_tensor(v29[:, csl], nLp9[:, csl], Rp,
                                               ALU.max)
                _after(v2_i, v1_i)
                u2_i = nc.vector.tensor_tensor(u29[:, csl], nLp9[:, csl],
                                               t9[:, csl], ALU.max)
                _after(u2_i, v2_i)
                mv_i = nc.vector.tensor_tensor(mv9[:, csl], v19[:, csl],
                                               v29[:, csl], ALU.min)
                _after(mv_i, u2_i)
                if w1 != 1.0:
                    mv_i = nc.vector.tensor_scalar(mv9[:, csl], mv9[:, csl],
                                                   float(w1), None, ALU.mult)
                u1_i = nc.vector.tensor_tensor(u19[:, csl], Lp, nt9[:, csl],
                                               ALU.max)
                _after(u1_i, mv_i)
                nc.gpsimd.tensor_tensor(
                    On_f[:, 2 * s + 1:2 * s + 2, :],
                    mv9[:, s * h2 + h2 // 2:(s + 1) * h2],
                    Ln_f[:, 2 * s + 1:2 * s + 2, :], ALU.add)
                on_i = nc.vector.tensor_tensor(
                    On_f[:, 2 * s:2 * s + 1, :],
                    mv9[:, s * h2:s * h2 + h2 // 2],
                    Ln_f[:, 2 * s:2 * s + 1, :], ALU.add)
                _after(on_i, u1_i)
                op_i = nc.vector.tensor_tensor(Op_f[:, msl, :], u19[:, csl],
                                               u29[:, csl], ALU.min)
                _after(op_i, on_i)
                if w0 != 1.0:
                    nc.vector.tensor_scalar(Op_f[:, msl, :], Op_f[:, msl, :],
                                            float(w0), None, ALU.mult)
                nc.sync.dma_start(
                    dram_row(out_d, 9)[:, msl, :],
                    O9[:, s * (w // 2):(s + 1) * (w // 2)].rearrange(
                        "p (g c) -> p g c", g=2))
            # -Rp for stage 8 on Act while stage 9 finishes
            nR_tiles[8] = ng_pool.tile([P, h], F16, tag="nRp", name="nRp8")
            nr8_i = nc.scalar.activation(nR_tiles[8][:], rp_of(8), ACTF.Copy,
                                         bias=0.0, scale=-1.0)
            _after(nr8_i, nlp_insts[1])
            L = O9

            for i in reversed(range(1, NUM_STAGES - 1)):
                w0, w1 = weights[i]
                R = R_tiles[i]
                # prefetch R two stages ahead (stage 0's row arrives
                # host-deinterleaved in r0_d)
                if i - 2 >= 0:
                    R_tiles[i - 2] = r_pool.tile([P, w], F16, tag="rin",
                                                 name=f"R{i - 2}")
                    src = dram_row(r0_d) if i == 2 else dram_row(right_d, i - 2)
                    nc.sync.dma_start(sb(R_tiles[i - 2][:]), src)

                Lp, Ln = _halves(L[:], i)
                Rp, Rn = _halves(R[:], i)
                nRp = nR_tiles.pop(i)

                # DVE queue [t, v1, v2, u2, mv, u1, ON, OP]: no op directly
                # follows its producer (kills the RAW side-effect gaps), the
                # next stage's t needs only ON (2 slots before OP -> seamless
                # stage handoff), and u1 sits late enough that Pool's nt is
                # always ready.
                # nLp gates v2 (DVE slot 3): 3/4 on Act, 1/4 on the DVE in
                # the slot freed by t's Pool share
                m_i = w >> (i + 1)
                nq = m_i - max(1, m_i // 8)
                cq = nq << i
                nLp = ng_pool.tile([P, h], F16, tag="nLp", name=f"nLp{i}")
                nLp_i = nc.scalar.activation(nLp[:, :cq], Lp[:, :nq, :],
                                             ACTF.Copy, bias=0.0, scale=-1.0)

                # t = Ln + Rn, column-split: Pool is idle before nt, so it
                # takes the tail 3/8 there
                tq = m_i - max(1, m_i // 2)
                ct = tq << i
                t = t_pool.tile([P, h], F16, tag="t", name=f"t{i}")
                t_i = nc.vector.tensor_tensor(t[:, :ct], Ln[:, :tq, :],
                                              Rn[:, :tq, :], ALU.add)
                nc.gpsimd.tensor_tensor(t[:, ct:], Ln[:, tq:, :],
                                        Rn[:, tq:, :], ALU.add)
                nlpd_i = nc.vector.tensor_scalar(nLp[:, cq:], Lp[:, nq:, :],
                                                 -1.0, None, ALU.mult)
                _after(nlpd_i, t_i)
                v1 = v_pool.tile([P, h], F16, tag="v1", name=f"v1_{i}")
                v1_i = nc.vector.tensor_tensor(v1[:], Lp, nRp[:], ALU.max)
                _after(v1_i, nlpd_i)
                # nt on Pool (tensor_scalar IS implemented on gpsimd): the
                # u-chain has slack, and it takes 594ns off the DVE
                nt = t_pool.tile([P, h], F16, tag="nt", name=f"nt{i}")
                nc.gpsimd.tensor_scalar(nt[:], t[:], -1.0, None, ALU.mult)

                v2 = v_pool.tile([P, h], F16, tag="v2", name=f"v2_{i}")
                v2_i = nc.vector.tensor_tensor(v2[:], nLp[:], Rp, ALU.max)
                _after(v2_i, v1_i)
                u2 = u_pool.tile([P, h], F16, tag="u2", name=f"u2_{i}")
                u2_i = nc.vector.tensor_tensor(u2[:], nLp[:], t[:], ALU.max)
                _after(u2_i, v2_i)

                O = lo_pool.tile([P, w], F16, tag="lo", name=f"O{i}")
                Op, On = _halves(O[:], i)

                mv = m_pool.tile([P, h], F16, tag="mv", name=f"mv{i}")
                mv_i = nc.vector.tensor_tensor(mv[:], v1[:], v2[:], ALU.min)
                _after(mv_i, u2_i)
                if w1 != 1.0:
                    mv_i = nc.vector.tensor_scalar(mv[:], mv[:], float(w1),
                                                   None, ALU.mult)
                u1 = u_pool.tile([P, h], F16, tag="u1", name=f"u1_{i}")
                u1_i = nc.vector.tensor_tensor(u1[:], Lp, nt[:], ALU.max)
                _after(u1_i, mv_i)
                # ON = mv + Ln, column-split: Pool (idle after nt) takes the
                # tail half right after mv while the DVE finishes u1/OP — the
                # only stage op Pool can legally run (fp16 tensor add).
                mh = max(1, (m_i * 3 + 4) // 8)   # ~3/8 on DVE, rest Pool
                ch = mh * (1 << i)
                nc.gpsimd.tensor_tensor(On[:, mh:, :], mv[:, ch:], Ln[:, mh:, :],
                                        ALU.add)
                on_i = nc.vector.tensor_tensor(On[:, :mh, :], mv[:, :ch],
                                               Ln[:, :mh, :], ALU.add)
                _after(on_i, u1_i)
                op_i = nc.vector.tensor_tensor(Op, u1[:], u2[:], ALU.min)
                _after(op_i, on_i)
                if w0 != 1.0:
                    op_i = nc.vector.tensor_scalar(Op, Op, float(w0), None,
                                                   ALU.mult)

                # -Rp for the NEXT stage while DVE finishes this one (kept
                # behind nLp on Act so it can't steal nLp's slot)
                nR_tiles[i - 1] = ng_pool.tile([P, h], F16, tag="nRp",
                                               name=f"nRp{i - 1}")
                nrp_i = nc.scalar.activation(nR_tiles[i - 1][:],
                                             rp_of(i - 1), ACTF.Copy,
                                             bias=0.0, scale=-1.0)
                _after(nrp_i, nLp_i)

                nc.sync.dma_start(dram_row(out_d, i), sb(O[:]))
                L = O

            # ---- stage 0: adjacent pairs -> deinterleave L row 1 into
            # packed halves (r0 arrives host-deinterleaved), column-split
            # into two batch-group halves so the writeback overlaps the
            # second half's compute.
            w0, w1 = weights[0]
            R0 = R_tiles[0]
            lv = L[:].rearrange("p (m two) -> p m two", two=2)
            Rp_f, Rn_f = _halves(R0[:], 9)
            nRp0 = nR_tiles.pop(0)
            Lp_t = pk_pool.tile([P, h], F16, tag="lpk", name="Lp0")
            Ln_t = pk_pool.tile([P, h], F16, tag="lnk", name="Ln0")
            nLp0 = ng_pool.tile([P, h], F16, tag="nLp", name="nLp0")
            t0 = t_pool.tile([P, h], F16, tag="t", name="t0")
            nt0 = t_pool.tile([P, h], F16, tag="nt", name="nt0")
            v10 = v_pool.tile([P, h], F16, tag="v1", name="v1_0")
            v20 = v_pool.tile([P, h], F16, tag="v2", name="v2_0")
            u10 = u_pool.tile([P, h], F16, tag="u1", name="u1_0")
            u20 = u_pool.tile([P, h], F16, tag="u2", name="u2_0")
            mv0 = m_pool.tile([P, h], F16, tag="mv", name="mv0")
            O0 = lo_pool.tile([P, w], F16, tag="lo", name="O0")
            Op_f, On_f = _halves(O0[:], 9)
            prev_act = None
            for s in (0, 1):
                msl = slice(2 * s, 2 * s + 2)
                csl = slice(s * h2, (s + 1) * h2)
                mm = slice(s * (w // 4), (s + 1) * (w // 4))
                lpc_i = nc.vector.tensor_copy(Lp_t[:, csl], lv[:, mm, 0])
                lnc_i = nc.scalar.activation(Ln_t[:, csl], lv[:, mm, 1],
                                             ACTF.Copy, bias=0.0, scale=1.0)
                nlp_i = nc.scalar.activation(nLp0[:, csl], lv[:, mm, 0],
                                             ACTF.Copy, bias=0.0, scale=-1.0)
                if prev_act is not None:
                    _after(lnc_i, prev_act)
                _after(nlp_i, lnc_i)
                prev_act = nlp_i
                Lp, Ln = Lp_t[:, csl], Ln_t[:, csl]
                Rp, Rn = Rp_f[:, msl, :], Rn_f[:, msl, :]
                cm = s * h2 + h2 // 2
                t_i = nc.vector.tensor_tensor(
                    t0[:, s * h2:cm], Ln_t[:, s * h2:cm],
                    Rn_f[:, 2 * s:2 * s + 1, :], ALU.add)
                _after(t_i, lpc_i)
                nc.gpsimd.tensor_tensor(
                    t0[:, cm:(s + 1) * h2], Ln_t[:, cm:(s + 1) * h2],
                    Rn_f[:, 2 * s + 1:2 * s + 2, :], ALU.add)
                nc.gpsimd.tensor_scalar(nt0[:, csl], t0[:, csl], -1.0, None,
                                        ALU.mult)
                v1_i = nc.vector.tensor_tensor(v10[:, csl], Lp, nRp0[:, csl],
                                               ALU.max)
                _after(v1_i, t_i)
                v2_i = nc.vector.tensor_tensor(v20[:, csl], nLp0[:, csl], Rp,
                                               ALU.max)
                _after(v2_i, v1_i)
                u2_i = nc.vector.tensor_tensor(u20[:, csl], nLp0[:, csl],
                                               t0[:, csl], ALU.max)
                _after(u2_i, v2_i)
                mv_i = nc.vector.tensor_tensor(mv0[:, csl], v10[:, csl],
                                               v20[:, csl], ALU.min)
                _after(mv_i, u2_i)
                if w1 != 1.0:
                    mv_i = nc.vector.tensor_scalar(mv0[:, csl], mv0[:, csl],
                                                   float(w1), None, ALU.mult)
                u1_i = nc.vector.tensor_tensor(u10[:, csl], Lp, nt0[:, csl],
                                               ALU.max)
                _after(u1_i, mv_i)
                nc.gpsimd.tensor_tensor(
                    On_f[:, 2 * s + 1:2 * s + 2, :],
                    mv0[:, s * h2 + h2 // 2:(s + 1) * h2],
                    Ln_t[:, s * h2 + h2 // 2:(s + 1) * h2], ALU.add)
                on_i = nc.vector.tensor_tensor(
                    On_f[:, 2 * s:2 * s + 1, :],
                    mv0[:, s * h2:s * h2 + h2 // 2],
                    Ln_t[:, s * h2:s * h2 + h2 // 2], ALU.add)
                _after(on_i, u1_i)
                op_i = nc.vector.tensor_tensor(Op_f[:, msl, :], u10[:, csl],
                                               u20[:, csl], ALU.min)
                _after(op_i, on_i)
                if w0 != 1.0:
                    nc.vector.tensor_scalar(Op_f[:, msl, :], Op_f[:, msl, :],
                                            float(w0), None, ALU.mult)
                # split writeback: each half leaves while the rest computes
                nc.sync.dma_start(dram_row(out_d, 0)[:, msl, h // g:],
                                  On_f[:, msl, :])
                nc.sync.dma_start(dram_row(out_d, 0)[:, msl, :h // g],
                                  Op_f[:, msl, :])


TRACE = False
LAST_RESULTS = None


def _make_nc(weights, bpc):
    nc = bacc.Bacc("TRN2", target_bir_lowering=False, debug=False)
    build(nc, weights, bpc)
    nc.compile()
    return nc


def kernel(right, left, left_weights, iter):
    right = np.asarray(right)
    left = np.asarray(left)
    wsel = np.asarray(left_weights, dtype=np.float32)[int(iter)]  # [10, 2]
    weights = [(float(wsel[i, 0]), float(wsel[i, 1])) for i in range(NUM_STAGES)]

    bpc = B // N_CORES
    nc = _make_nc(weights, bpc)

    right16 = right[:, :NUM_STAGES, :].astype(np.float16)
    left10_16 = left[:, NUM_STAGES, :].astype(np.float16)
    # stage-0 right row, host-deinterleaved into packed pos|neg halves
    r0 = np.empty((B, CODE), np.float16)
    r0[:, :CODE // 2] = right16[:, 0, 0::2]
    r0[:, CODE // 2:] = right16[:, 0, 1::2]

    in_maps = []
    for c in range(N_CORES):
        sl = slice(c * bpc, (c + 1) * bpc)
        in_maps.append({
            "right": np.ascontiguousarray(right16[sl]),
            "right0": np.ascontiguousarray(r0[sl]),
            "left10": np.ascontiguousarray(left10_16[sl]),
        })
    global LAST_RESULTS
    LAST_RESULTS = run_bass_kernel_spmd(
        nc, in_maps, list(range(N_CORES)), trace=TRACE)
    res = LAST_RESULTS.results

    out16 = np.empty((B, NUM_STAGES, CODE), np.float16)
    for c in range(N_CORES):
        out16[c * bpc:(c + 1) * bpc] = res[c]["out"]
    # row 0 left the device as packed pos|neg halves; re-interleave
    row0 = out16[:, 0, :].copy()
    out16[:, 0, 0::2] = row0[:, :CODE // 2]
    out16[:, 0, 1::2] = row0[:, CODE // 2:]

    out = np.empty((B, NUM_STAGES + 1, CODE), np.float32)
    out[:, :NUM_STAGES, :] = np.clip(out16.astype(np.float32), -CLIP, CLIP)
    out[:, NUM_STAGES, :] = np.clip(left[:, NUM_STAGES, :], -CLIP, CLIP)
    return out


# revision 56
# speedup vs baseline: 2.2087x; 1.0028x over previous
"""Trainium2 Bass kernel: polar-BP left-message butterfly (nn_IterateLeftLayer).

Math per stage i (9..0), with L = left row i+1 (unclipped), R = right row i:
  out[pos] = w0 * ms(L[pos], L[neg] + R[neg])
  out[neg] = w1 * ms(L[pos], R[pos]) + L[neg]
where ms(x,y) = sign(x)sign(y)min(|x|,|y|), pos = {c: bit i of c == 0},
neg = pos + 2^i.  Final output = clip(left, +-10) with rows 0..9 replaced.

ms is computed in pure min/max form (no bit tricks):
  ms(a,b) = min(max(a,-b), max(-a,b))
which is exact (selection + sign flip only).  All on-device compute is fp16:
every DVE tensor_tensor runs in 2x mode and every tensor_scalar in 4x mode,
and DMA bytes halve.  fp16 rounding keeps rel err ~7e-4 vs the f32 oracle.

The +-10 output clip is applied on the host (the recurrence needs UNclipped
values anyway, so the device never clips).

Engine balance per stage (free dim h=2048 per op); the stage recurrence is
serial in L (every next-stage op needs the FULL previous row), so the per-
stage critical chain is what matters.  Assignment keeps the two cheap-engine
ops (Pool) OFF the chain's tail and the DVE queue ordered so it never
stalls more than ~0.4us:
  DVE : t=Ln+Rn, nt=-t, v2=max(nLp,t), mv=min(v1,v2), ON=mv+Ln,
        u2=max(nLp,t)... order [t, nt, v2, mv, ON, u2, OP=min(u1,u2)]
  Act : nLp=-Lp, then nRp for the NEXT stage (R is prefetched, so -Rp is
        computable one stage early)
  Pool: v1=max(Lp,nRp) (ready at row-start), u1=max(Lp,nt) (needed last)
  SP  : R-row prefetch + output writeback DMAs (double buffered)

Stage 0 pairs adjacent elements (stride 1), which would break the DVE 16-bit
packed fast path; instead L row 1 is deinterleaved on-device into packed
pos/neg halves, right row 0 arrives host-deinterleaved, and out row 0 leaves
as packed halves that the host re-interleaves.

Sharding: pure data-parallel over batch, 512 rows per core on 8 cores.
Layout: batch on partitions (4 groups of 128 coalesced along the free axis
-> [128, 4096] tiles); the butterfly is pure strided access patterns.
"""

import sys

for _p in ("/opt/trn_rl_repo",):
    if _p not in sys.path:
        sys.path.insert(0, _p)

import numpy as np

import bass_rust
import concourse.bass as bass
import concourse.tile as tile
from concourse import bacc, mybir
from concourse.bass_utils import run_bass_kernel_spmd


def _after(inst, *prevs):
    """Order `inst` after `prevs` on the same engine (scheduler edge, no
    semaphore) — pins queue order the greedy list scheduler would flip."""
    names = bass_rust.InstructionNameOrderedSet([p.ins.name for p in prevs])
    inst.ins.add_nosync_dependencies_from(names)

NUM_STAGES = 10
CODE = 1024
B = 4096
N_CORES = 8
P = 128
CLIP = 10.0
F16 = mybir.dt.float16
ALU = mybir.AluOpType
ACTF = mybir.ActivationFunctionType


def _halves(ap, i):
    """pos/neg strided views of a [P, W] row for stage i (i >= 1)."""
    r = 1 << i
    v = ap.rearrange("p (m two r) -> p m two r", two=2, r=r)
    return v[:, :, 0, :], v[:, :, 1, :]


def build(nc, weights, bpc):
    """Emit the per-core kernel. weights: [(w0, w1)] * 10, bpc: batch rows/core."""
    g = bpc // P
    w = g * CODE
    h = w // 2

    right_d = nc.dram_tensor("right", [bpc, NUM_STAGES, CODE], F16,
                             kind="ExternalInput")
    r0_d = nc.dram_tensor("right0", [bpc, CODE], F16, kind="ExternalInput")
    left10_d = nc.dram_tensor("left10", [bpc, CODE], F16, kind="ExternalInput")
    out_d = nc.dram_tensor("out", [bpc, NUM_STAGES, CODE], F16,
                           kind="ExternalOutput")

    def dram_row(dt_, i=None):
        ap = dt_.ap() if i is None else dt_.ap()[:, i, :]
        return ap.rearrange("(g p) c -> p g c", p=P)

    def sb(ap):
        return ap.rearrange("p (g c) -> p g c", g=g)

    with tile.TileContext(nc) as tc:
        with (
            tc.tile_pool(name="lo", bufs=3) as lo_pool,
            tc.tile_pool(name="rin", bufs=3) as r_pool,
            tc.tile_pool(name="tt", bufs=2) as t_pool,
            tc.tile_pool(name="ng", bufs=2) as ng_pool,
            tc.tile_pool(name="uu", bufs=2) as u_pool,
            tc.tile_pool(name="vv", bufs=2) as v_pool,
            tc.tile_pool(name="mv", bufs=2) as m_pool,
            tc.tile_pool(name="pk", bufs=1) as pk_pool,
        ):
            # fill: stage 9 is column-split into two batch-group halves
            # (independent rows), each fed by quarter DMAs in dependency
            # order (neg half first), so compute starts ~3.5us in.
            L = lo_pool.tile([P, w], F16, tag="lo", name="L10")
            R_tiles = {9: r_pool.tile([P, w], F16, tag="rin", name="R9")}
            for s in (0, 1):
                for half in (1, 0):   # neg halves first
                    for dt_, tile_ in ((left10_d, L), (right_d, R_tiles[9])):
                        src = dt_.ap() if dt_ is left10_d else dt_.ap()[:, 9, :]
                        src = src.rearrange(
                            "(g p) (two r) -> p g two r",
                            p=P, two=2, r=CODE // 2)[:, 2 * s:2 * s + 2, half, :]
                        dst = tile_[:].rearrange(
                            "p (g two r) -> p g two r",
                            two=2, r=CODE // 2)[:, 2 * s:2 * s + 2, half, :]
                        nc.sync.dma_start(dst, src)
            for i in (8,):
                R_tiles[i] = r_pool.tile([P, w], F16, tag="rin", name=f"R{i}")
                nc.sync.dma_start(sb(R_tiles[i][:]), dram_row(right_d, i))
            def rp_of(j):
                """pos-half view of stage j's R tile."""
                return _halves(R_tiles[j][:], 9 if j == 0 else j)[0]

            nR_tiles = {}

            # ---- stage 9, column-split into two independent batch-group
            # halves so each starts as soon as its quarter-DMAs land.
            # nRp9 runs on DVE (idle during fill) instead of Act.
            w0, w1 = weights[9]
            R9 = R_tiles[9]
            R_tiles[7] = r_pool.tile([P, w], F16, tag="rin", name="R7")
            nc.sync.dma_start(sb(R_tiles[7][:]), dram_row(right_d, 7))
            Lp_f, Ln_f = _halves(L[:], 9)
            Rp_f, Rn_f = _halves(R9[:], 9)
            h2 = h // 2
            nLp9 = ng_pool.tile([P, h], F16, tag="nLp", name="nLp9")
            nRp9 = ng_pool.tile([P, h], F16, tag="nRp9", name="nRp9")
            t9 = t_pool.tile([P, h], F16, tag="t", name="t9")
            nt9 = t_pool.tile([P, h], F16, tag="nt", name="nt9")
            v19 = v_pool.tile([P, h], F16, tag="v1", name="v1_9")
            v29 = v_pool.tile([P, h], F16, tag="v2", name="v2_9")
            u19 = u_pool.tile([P, h], F16, tag="u1", name="u1_9")
            u29 = u_pool.tile([P, h], F16, tag="u2", name="u2_9")
            mv9 = m_pool.tile([P, h], F16, tag="mv", name="mv9")
            O9 = lo_pool.tile([P, w], F16, tag="lo", name="O9")
            Op_f, On_f = _halves(O9[:], 9)
            nlp_insts = []
            for s in (0, 1):
                msl = slice(2 * s, 2 * s + 2)
                csl = slice(s * h2, (s + 1) * h2)
                Lp, Ln = Lp_f[:, msl, :], Ln_f[:, msl, :]
                Rp, Rn = Rp_f[:, msl, :], Rn_f[:, msl, :]
                nlp_insts.append(nc.scalar.activation(
                    nLp9[:, csl], Lp, ACTF.Copy, bias=0.0, scale=-1.0))
                cm9 = s * h2 + h2 // 2
                t_i = nc.vector.tensor_tensor(
                    t9[:, s * h2:cm9], Ln_f[:, 2 * s:2 * s + 1, :],
                    Rn_f[:, 2 * s:2 * s + 1, :], ALU.add)
                nc.gpsimd.tensor_tensor(
                    t9[:, cm9:(s + 1) * h2], Ln_f[:, 2 * s + 1:2 * s + 2, :],
                    Rn_f[:, 2 * s + 1:2 * s + 2, :], ALU.add)
                nc.gpsimd.tensor_scalar(nt9[:, csl], t9[:, csl], -1.0, None,
                                        ALU.mult)
                nr_i = nc.vector.tensor_scalar(nRp9[:, csl], Rp, -1.0, None,
                                               ALU.mult)
                _after(nr_i, t_i)
                v1_i = nc.vector.tensor_tensor(v19[:, csl], Lp, nRp9[:, csl],
                                               ALU.max)
                _after(v1_i, nr_i)
                v2_i = nc.vector.tensor_tensor(v29[:, csl], nLp9[:, csl], Rp,
                                               ALU.max)
                _after(v2_i, v1_i)
                u2_i = nc.vector.tensor_tensor(u29[:, csl], nLp9[:, csl],
                                               t9[:, csl], ALU.max)
                _after(u2_i, v2_i)
                mv_i = nc.vector.tensor_tensor(mv9[:, csl], v19[:, csl],
                                               v29[:, csl], ALU.min)
                _after(mv_i, u2_i)
                if w1 != 1.0:
                    mv_i = nc.vector.tensor_scalar(mv9[:, csl], mv9[:, csl],
                                                   float(w1), None, ALU.mult)
                u1_i = nc.vector.tensor_tensor(u19[:, csl], Lp, nt9[:, csl],
                                               ALU.max)
                _after(u1_i, mv_i)
                nc.gpsimd.tensor_tensor(
                    On_f[:, 2 * s + 1:2 * s + 2, :],
                    mv9[:, s * h2 + h2 // 2:(s + 1) * h2],
                    Ln_f[:, 2 * s + 1:2 * s + 2, :], ALU.add)
                on_i = nc.vector.tensor_tensor(
                    On_f[:, 2 * s:2 * s + 1, :],
                    mv9[:, s * h2:s * h2 + h2 // 2],
                    Ln_f[:, 2 * s:2 * s + 1, :], ALU.add)
                _after(on_i, u1_i)
                op_i = nc.vector.tensor_tensor(Op_f[:, msl, :], u19[:, csl],
                                               u29[:, csl], ALU.min)
                _after(op_i, on_i)
                if w0 != 1.0:
                    nc.vector.tensor_scalar(Op_f[:, msl, :], Op_f[:, msl, :],
                                            float(w0), None, ALU.mult)
                nc.sync.dma_start(
                    dram_row(out_d, 9)[:, msl, :],
                    O9[:, s * (w // 2):(s + 1) * (w // 2)].rearrange(
                        "p (g c) -> p g c", g=2))
            # -Rp for stage 8 on Act while stage 9 finishes
            nR_tiles[8] = ng_pool.tile([P, h], F16, tag="nRp", name="nRp8")
            nr8_i = nc.scalar.activation(nR_tiles[8][:], rp_of(8), ACTF.Copy,
                                         bias=0.0, scale=-1.0)
            _after(nr8_i, nlp_insts[1])
            L = O9

            for i in reversed(range(1, NUM_STAGES - 1)):
                w0, w1 = weights[i]
                R = R_tiles[i]
                # prefetch R two stages ahead (stage 0's row arrives
                # host-deinterleaved in r0_d)
                if i - 2 >= 0:
                    R_tiles[i - 2] = r_pool.tile([P, w], F16, tag="rin",
                                                 name=f"R{i - 2}")
                    src = dram_row(r0_d) if i == 2 else dram_row(right_d, i - 2)
                    nc.sync.dma_start(sb(R_tiles[i - 2][:]), src)

                Lp, Ln = _halves(L[:], i)
                Rp, Rn = _halves(R[:], i)
                nRp = nR_tiles.pop(i)

                # DVE queue [t, v1, v2, u2, mv, u1, ON, OP]: no op directly
                # follows its producer (kills the RAW side-effect gaps), the
                # next stage's t needs only ON (2 slots before OP -> seamless
                # stage handoff), and u1 sits late enough that Pool's nt is
                # always ready.
                # nLp gates v2 (DVE slot 3): 3/4 on Act, 1/4 on the DVE in
                # the slot freed by t's Pool share
                m_i = w >> (i + 1)
                nq = m_i - max(1, m_i // 8)
                cq = nq << i
                nLp = ng_pool.tile([P, h], F16, tag="nLp", name=f"nLp{i}")
                nLp_i = nc.scalar.activation(nLp[:, :cq], Lp[:, :nq, :],
                                             ACTF.Copy, bias=0.0, scale=-1.0)

                # t = Ln + Rn, column-split: Pool is idle before nt, so it
                # takes the tail 3/8 there
                tq = m_i - max(1, m_i // 2)
                ct = tq << i
                t = t_pool.tile([P, h], F16, tag="t", name=f"t{i}")
                t_i = nc.vector.tensor_tensor(t[:, :ct], Ln[:, :tq, :],
                                              Rn[:, :tq, :], ALU.add)
                nc.gpsimd.tensor_tensor(t[:, ct:], Ln[:, tq:, :],
                                        Rn[:, tq:, :], ALU.add)
                nlpd_i = nc.vector.tensor_scalar(nLp[:, cq:], Lp[:, nq:, :],
                                                 -1.0, None, ALU.mult)
                _after(nlpd_i, t_i)
                v1 = v_pool.tile([P, h], F16, tag="v1", name=f"v1_{i}")
                v1_i = nc.vector.tensor_tensor(v1[:], Lp, nRp[:], ALU.max)
                _after(v1_i, nlpd_i)
                # nt on Pool (tensor_scalar IS implemented on gpsimd): the
                # u-chain has slack, and it takes 594ns off the DVE
                nt = t_pool.tile([P, h], F16, tag="nt", name=f"nt{i}")
                nc.gpsimd.tensor_scalar(nt[:], t[:], -1.0, None, ALU.mult)

                v2 = v_pool.tile([P, h], F16, tag="v2", name=f"v2_{i}")
                v2_i = nc.vector.tensor_tensor(v2[:], nLp[:], Rp, ALU.max)
                _after(v2_i, v1_i)
                u2 = u_pool.tile([P, h], F16, tag="u2", name=f"u2_{i}")
                u2_i = nc.vector.tensor_tensor(u2[:], nLp[:], t[:], ALU.max)
                _after(u2_i, v2_i)

                O = lo_pool.tile([P, w], F16, tag="lo", name=f"O{i}")
                Op, On = _halves(O[:], i)

                mv = m_pool.tile([P, h], F16, tag="mv", name=f"mv{i}")
                mv_i = nc.vector.tensor_tensor(mv[:], v1[:], v2[:], ALU.min)
                _after(mv_i, u2_i)
                if w1 != 1.0:
                    mv_i = nc.vector.tensor_scalar(mv[:], mv[:], float(w1),
                                                   None, ALU.mult)
                u1 = u_pool.tile([P, h], F16, tag="u1", name=f"u1_{i}")
                u1_i = nc.vector.tensor_tensor(u1[:], Lp, nt[:], ALU.max)
                _after(u1_i, mv_i)
                # ON = mv + Ln, column-split: Pool (idle after nt) takes the
                # tail half right after mv while the DVE finishes u1/OP — the
                # only stage op Pool can legally run (fp16 tensor add).
                mh = max(1, (m_i * 3 + 4) // 8)   # ~3/8 on DVE, rest Pool
                ch = mh * (1 << i)
                nc.gpsimd.tensor_tensor(On[:, mh:, :], mv[:, ch:], Ln[:, mh:, :],
                                        ALU.add)
                on_i = nc.vector.tensor_tensor(On[:, :mh, :], mv[:, :ch],
                                               Ln[:, :mh, :], ALU.add)
                _after(on_i, u1_i)
                op_i = nc.vector.tensor_tensor(Op, u1[:], u2[:], ALU.min)
                _after(op_i, on_i)
                if w0 != 1.0:
                    op_i = nc.vector.tensor_scalar(Op, Op, float(w0), None,
                                                   ALU.mult)

                # -Rp for the NEXT stage while DVE finishes this one (kept
                # behind nLp on Act so it can't steal nLp's slot)
                nR_tiles[i - 1] = ng_pool.tile([P, h], F16, tag="nRp",
                                               name=f"nRp{i - 1}")
                nrp_i = nc.scalar.activation(nR_tiles[i - 1][:],
                                             rp_of(i - 1), ACTF.Copy,
                                             bias=0.0, scale=-1.0)
                _after(nrp_i, nLp_i)

                nc.sync.dma_start(dram_row(out_d, i), sb(O[:]))
                L = O

            # ---- stage 0: adjacent pairs -> deinterleave L row 1 into
            # packed halves (r0 arrives host-deinterleaved), column-split
            # into two batch-group halves so the writeback overlaps the
            # second half's compute.
            w0, w1 = weights[0]
            R0 = R_tiles[0]
            lv = L[:].rearrange("p (m two) -> p m two", two=2)
            Rp_f, Rn_f = _halves(R0[:], 9)
            nRp0 = nR_tiles.pop(0)
            Lp_t = pk_pool.tile([P, h], F16, tag="lpk", name="Lp0")
            Ln_t = pk_pool.tile([P, h], F16, tag="lnk", name="Ln0")
            nLp0 = ng_pool.tile([P, h], F16, tag="nLp", name="nLp0")
            t0 = t_pool.tile([P, h], F16, tag="t", name="t0")
            nt0 = t_pool.tile([P, h], F16, tag="nt", name="nt0")
            v10 = v_pool.tile([P, h], F16, tag="v1", name="v1_0")
            v20 = v_pool.tile([P, h], F16, tag="v2", name="v2_0")
            u10 = u_pool.tile([P, h], F16, tag="u1", name="u1_0")
            u20 = u_pool.tile([P, h], F16, tag="u2", name="u2_0")
            mv0 = m_pool.tile([P, h], F16, tag="mv", name="mv0")
            O0 = lo_pool.tile([P, w], F16, tag="lo", name="O0")
            Op_f, On_f = _halves(O0[:], 9)
            prev_act = None
            for s in (0, 1):
                msl = slice(2 * s, 2 * s + 2)
                csl = slice(s * h2, (s + 1) * h2)
                mm = slice(s * (w // 4), (s + 1) * (w // 4))
                lpc_i = nc.vector.tensor_copy(Lp_t[:, csl], lv[:, mm, 0])
                lnc_i = nc.scalar.activation(Ln_t[:, csl], lv[:, mm, 1],
                                             ACTF.Copy, bias=0.0, scale=1.0)
                nlp_i = nc.scalar.activation(nLp0[:, csl], lv[:, mm, 0],
                                             ACTF.Copy, bias=0.0, scale=-1.0)
                if prev_act is not None:
                    _after(lnc_i, prev_act)
                _after(nlp_i, lnc_i)
                prev_act = nlp_i
                Lp, Ln = Lp_t[:, csl], Ln_t[:, csl]
                Rp, Rn = Rp_f[:, msl, :], Rn_f[:, msl, :]
                cm = s * h2 + h2 // 2
                t_i = nc.vector.tensor_tensor(
                    t0[:, s * h2:cm], Ln_t[:, s * h2:cm],
                    Rn_f[:, 2 * s:2 * s + 1, :], ALU.add)
                _after(t_i, lpc_i)
                nc.gpsimd.tensor_tensor(
                    t0[:, cm:(s + 1) * h2], Ln_t[:, cm:(s + 1) * h2],
                    Rn_f[:, 2 * s + 1:2 * s + 2, :], ALU.add)
                nc.gpsimd.tensor_scalar(nt0[:, csl], t0[:, csl], -1.0, None,
                                        ALU.mult)
                v1_i = nc.vector.tensor_tensor(v10[:, csl], Lp, nRp0[:, csl],
                                               ALU.max)
                _after(v1_i, t_i)
                v2_i = nc.vector.tensor_tensor(v20[:, csl], nLp0[:, csl], Rp,
                                               ALU.max)
                _after(v2_i, v1_i)
                u2_i = nc.vector.tensor_tensor(u20[:, csl], nLp0[:, csl],
                                               t0[:, csl], ALU.max)
                _after(u2_i, v2_i)
                mv_i = nc.vector.tensor_tensor(mv0[:, csl], v10[:, csl],
                                               v20[:, csl], ALU.min)
                _after(mv_i, u2_i)
                if w1 != 1.0:
                    mv_i = nc.vector.tensor_scalar(mv0[:, csl], mv0[:, csl],
                                                   float(w1), None, ALU.mult)
                u1_i = nc.vector.tensor_tensor(u10[:, csl], Lp, nt0[:, csl],
                                               ALU.max)
                _after(u1_i, mv_i)
                nc.gpsimd.tensor_tensor(
                    On_f[:, 2 * s + 1:2 * s + 2, :],
                    mv0[:, s * h2 + h2 // 2:(s + 1) * h2],
                    Ln_t[:, s * h2 + h2 // 2:(s + 1) * h2], ALU.add)
                on_i = nc.vector.tensor_tensor(
                    On_f[:, 2 * s:2 * s + 1, :],
                    mv0[:, s * h2:s * h2 + h2 // 2],
                    Ln_t[:, s * h2:s * h2 + h2 // 2], ALU.add)
                _after(on_i, u1_i)
                op_i = nc.vector.tensor_tensor(Op_f[:, msl, :], u10[:, csl],
                                               u20[:, csl], ALU.min)
                _after(op_i, on_i)
                if w0 != 1.0:
                    nc.vector.tensor_scalar(Op_f[:, msl, :], Op_f[:, msl, :],
                                            float(w0), None, ALU.mult)
                # split writeback: each half leaves while the rest computes
                nc.sync.dma_start(dram_row(out_d, 0)[:, msl, h // g:],
                                  On_f[:, msl, :])
                nc.sync.dma_start(dram_row(out_d, 0)[:, msl, :h // g],
                                  Op_f[:, msl, :])


TRACE = False
LAST_RESULTS = None


def _make_nc(weights, bpc):
    nc = bacc.Bacc("TRN2", target_bir_lowering=False, debug=False)
    build(nc, weights, bpc)
    nc.compile()
    return nc


def kernel(right, left, left_weights, iter):
    right = np.asarray(right)
    left = np.asarray(left)
    wsel = np.asarray(left_weights, dtype=np.float32)[int(iter)]  # [10, 2]
    weights = [(float(wsel[i, 0]), float(wsel[i, 1])) for i in range(NUM_STAGES)]

    bpc = B // N_CORES
    nc = _make_nc(weights, bpc)

    right16 = right[:, :NUM_STAGES, :].astype(np.float16)
    left10_16 = left[:, NUM_STAGES, :].astype(np.float16)
    # stage-0 right row, host-deinterleaved into packed pos|neg halves
    r0 = np.empty((B, CODE), np.float16)
    r0[:, :CODE // 2] = right16[:, 0, 0::2]
    r0[:, CODE // 2:] = right16[:, 0, 1::2]

    in_maps = []
    for c in range(N_CORES):
        sl = slice(c * bpc, (c + 1) * bpc)
        in_maps.append({
            "right": np.ascontiguousarray(right16[sl]),
            "right0": np.ascontiguousarray(r0[sl]),
            "left10": np.ascontiguousarray(left10_16[sl]),
        })
    global LAST_RESULTS
    LAST_RESULTS = run_bass_kernel_spmd(
        nc, in_maps, list(range(N_CORES)), trace=TRACE)
    res = LAST_RESULTS.results

    out16 = np.empty((B, NUM_STAGES, CODE), np.float16)
    for c in range(N_CORES):
        out16[c * bpc:(c + 1) * bpc] = res[c]["out"]
    # row 0 left the device as packed pos|neg halves; re-interleave
    row0 = out16[:, 0, :].copy()
    out16[:, 0, 0::2] = row0[:, :CODE // 2]
    out16[:, 0, 1::2] = row0[:, CODE // 2:]

    out = np.empty((B, NUM_STAGES + 1, CODE), np.float32)
    out[:, :NUM_STAGES, :] = np.clip(out16.astype(np.float32), -CLIP, CLIP)
    out[:, NUM_STAGES, :] = np.clip(left[:, NUM_STAGES, :], -CLIP, CLIP)
    return out
